# revision 1
# baseline (speedup 1.0000x reference)
"""Causal multi-head attention with RoPE for Trainium2, 8-core SPMD.

Problem: B=2, S=2048, D_MODEL=1024, H=16, HD=64, causal softmax(QK^T/8)V
with interleaved-pair RoPE on q/k, projections Wq/Wk/Wv/Wo.

Sharding (host side): batch x head-group. Core c handles batch b=c//4 and
head group g=c%4 (heads 4g..4g+3, a 256-wide slice of the projection dims).
Each core computes a full [S, D_MODEL] partial of the output (its head
group's contribution through Wo); host sums 4 partials per batch.

Device layout strategy (all matmuls bf16, fp32 accumulate):
 - host passes x[b].T so the d-contraction sits on SBUF partitions
 - Q,K projected in [s, o] layout -> RoPE on DVE along free dim (pairs are
   adjacent columns) -> bf16 -> DMA-transposed (XBAR, bf16) into [o, s]
 - scores^T[k, q] = Kt.T @ Qt per 128-key block (K=64 contraction); the
   two heads of a pair are issued to PE row groups 0/64 (tile_position)
   and run concurrently. Blocks land in wide PSUM tiles, one Exp per wide
   tile (ACT amortizes its 352-cycle fixed cost), causal-masked by
   multiplying the diagonal 128x128 block; q-columns below the diagonal
   are never computed or consumed
 - PV: lhsT = [V | 1] per key block (M=65) so row 64 of the PSUM output
   accumulates the softmax denominator for free; DVE normalizes
 - o_proj consumes the attention output, PSUM is DMA'd straight to DRAM
"""

import numpy as np
import ml_dtypes

B, S, D, H = 2, 2048, 1024, 16
HD = 64
NCORES = 8
HEADS_PER_CORE = 4
GDIM = HEADS_PER_CORE * HD          # 256 projection cols per core
SB = S // 128                        # 16 s-tiles
KD = D // 128                        # 8 k-tiles over d
QCHUNK = 512
NQC = S // QCHUNK                    # 4 q-chunks
WIDE = 1024                          # wide scores psum tile (2 banks)

_BF16 = ml_dtypes.bfloat16
_cache = {}


def _build(use_rope: bool, reps: int = 1, timing: bool = False, phases=(1, 2, 3)):
    import concourse.bass as bass
    import concourse.mybir as mybir
    import concourse.tile as tile
    from concourse import bacc

    F32 = mybir.dt.float32
    BF16 = mybir.dt.bfloat16
    EXP = mybir.ActivationFunctionType.Exp

    nc = bacc.Bacc(None, target_bir_lowering=False)

    xt_d = nc.dram_tensor("xt", [D, S], BF16, kind="ExternalInput")
    wqk_d = nc.dram_tensor("wqk", [D, 2 * GDIM], BF16, kind="ExternalInput")
    wv_d = nc.dram_tensor("wv", [D, GDIM], BF16, kind="ExternalInput")
    wo_d = nc.dram_tensor("wo", [GDIM, D], BF16, kind="ExternalInput")
    cos_d = nc.dram_tensor("cos8", [S, 256], BF16, kind="ExternalInput")
    sin_d = nc.dram_tensor("sin8", [S, 256], BF16, kind="ExternalInput")
    mask_d = nc.dram_tensor("maskT", [128, 128], BF16, kind="ExternalInput")
    if timing:
        # timing builds: full-size output stays on device (internal DRAM);
        # tiny external output avoids 64MB host transfers per timed call
        out_d = nc.dram_tensor("oscratch", [S, D], F32)
        out_small = nc.dram_tensor("out", [128, 512], F32, kind="ExternalOutput")
    else:
        out_d = nc.dram_tensor("out", [S, D], F32, kind="ExternalOutput")
        out_small = None

    with tile.TileContext(nc) as tc:
        with tc.tile_pool(name="big", bufs=1) as big, \
             tc.tile_pool(name="work", bufs=3) as work, \
             tc.tile_pool(name="ropet", bufs=4) as ropet, \
             tc.tile_pool(name="pex", bufs=4) as pex:
            # ---- resident tensors ----
            xt = big.tile([128, KD, S], BF16)
            nc.sync.dma_start(xt[:], xt_d.rearrange("(k p) s -> p k s", p=128))
            wqk = big.tile([128, KD, 2 * GDIM], BF16)
            nc.sync.dma_start(wqk[:], wqk_d.rearrange("(k p) o -> p k o", p=128))
            wv = big.tile([128, KD, GDIM], BF16)
            nc.sync.dma_start(wv[:], wv_d.rearrange("(k p) o -> p k o", p=128))
            wo = big.tile([128, 2, D], BF16)
            nc.sync.dma_start(wo[:], wo_d.rearrange("(k p) o -> p k o", p=128))
            maskT = big.tile([128, 128], BF16)
            nc.sync.dma_start(maskT[:], mask_d[:])
            if use_rope:
                cos8 = big.tile([128, SB, 256], BF16)
                nc.sync.dma_start(cos8[:], cos_d.rearrange("(m p) f -> p m f", p=128))
                sin8 = big.tile([128, SB, 256], BF16)
                nc.sync.dma_start(sin8[:], sin_d.rearrange("(m p) f -> p m f", p=128))

            # attention-side resident tiles
            qkt = [big.tile([128, S], BF16, tag=f"qkt{i}", name=f"qkt{i}")
                   for i in range(4)]
            # qkt[0]: Qt heads 0-1, qkt[1]: Qt heads 2-3, qkt[2]: Kt 0-1, qkt[3]: Kt 2-3
            vsb = big.tile([128, SB, HEADS_PER_CORE * 65], BF16)
            yt2 = [big.tile([128, S], BF16, tag=f"yt2{i}", name=f"yt2{i}")
                   for i in range(2)]

            for _rep in range(reps):
                # PSUM plan: sc(4 banks)+yt(2)+pp(2) live together; pp closes
                # after phase 1 and op(2) reuses its banks, so projections,
                # attention and o_proj can overlap on separate banks.
                with tc.tile_pool(name="sc", bufs=1, space="PSUM") as scp, \
                     tc.tile_pool(name="yt", bufs=1, space="PSUM") as ytp:
                    # ---- phase 1: projections + rope + transpose + V ----
                    if 1 in phases:
                        with tc.tile_pool(name="pp", bufs=1, space="PSUM") as pp:
                            ones_set = False
                            for m in range(SB):
                                ms = slice(m * 128, (m + 1) * 128)
                                # QK projection: [128 s, 512] = x_m @ [Wq|Wk]
                                ps = pp.tile([128, 2 * GDIM], F32, tag="ps_qk")
                                for k in range(KD):
                                    nc.tensor.matmul(ps[:], xt[:, k, ms], wqk[:, k, :],
                                                     start=(k == 0), stop=(k == KD - 1))
                                qkr = ropet.tile([128, 2 * GDIM], BF16, tag="qkr")
                                if use_rope:
                                    # single fast cast-copy releases the psum
                                    # slot; rope runs in bf16 on SBUF (2x DVE)
                                    qkf = ropet.tile([128, 2 * GDIM], BF16,
                                                     tag="qkf")
                                    nc.vector.tensor_copy(qkf[:], ps[:])
                                    pv = qkf.rearrange("p (x two) -> p two x", two=2)
                                    ov = qkr.rearrange("p (x two) -> p two x", two=2)
                                    E, O = pv[:, 0, :], pv[:, 1, :]
                                    C, Sn = cos8[:, m, :], sin8[:, m, :]
                                    ta = ropet.tile([128, 256], BF16, tag="ta")
                                    tb = ropet.tile([128, 256], BF16, tag="tb")
                                    nc.vector.tensor_mul(ta[:], E, C)
                                    nc.vector.tensor_mul(tb[:], O, Sn)
                                    nc.vector.tensor_sub(ov[:, 0, :], ta[:], tb[:])
                                    tc_ = ropet.tile([128, 256], BF16, tag="tc")
                                    td = ropet.tile([128, 256], BF16, tag="td")
                                    nc.vector.tensor_mul(tc_[:], O, C)
                                    nc.vector.tensor_mul(td[:], E, Sn)
                                    nc.vector.tensor_add(ov[:, 1, :], tc_[:], td[:])
                                else:
                                    nc.vector.tensor_copy(qkr[:], ps[:])
                                # transpose 128x128 blocks into qkt tiles
                                for cb in range(4):
                                    nc.sync.dma_start_transpose(
                                        qkt[cb][:, ms], qkr[:, cb * 128:(cb + 1) * 128])

                                # V projection: [128 s, 256]
                                psv = pp.tile([128, GDIM], F32, tag="ps_v")
                                for k in range(KD):
                                    nc.tensor.matmul(psv[:], xt[:, k, ms], wv[:, k, :],
                                                     start=(k == 0), stop=(k == KD - 1))
                                if not ones_set:
                                    nc.vector.memset(vsb[:], 1.0)
                                    ones_set = True
                                # copy 4 head blocks of 64 into stride-65 slots
                                dst = vsb[:, m, :].rearrange("p (h c) -> p h c", h=4)[:, :, 0:64]
                                src = psv.rearrange("p (h c) -> p h c", h=4)
                                nc.vector.tensor_copy(dst, src)

                    # ---- phase 2: attention, head pairs row-packed on PE ----
                    if 2 in phases:
                        # Heads 2p and 2p+1 share qkt tiles (partitions 0-63 / 64-127);
                        # their scoresT matmuls are issued to PE row groups 0 and 64 via
                        # tile_position auto-derivation and run concurrently.
                        for qc in range(NQC):
                            for hp in range(2):
                                qt = qkt[hp]
                                kt = qkt[2 + hp]
                                q0 = qc * QCHUNK
                                # Pack kb blocks into wide psum tiles of
                                # WIDE cols. A matmul may not cross a 512-col
                                # psum bank, so emit widths in order
                                # 512,...,512,384,128,256 (384+128=512 tiles
                                # banks exactly; 256 trails).
                                order = list(range(4 * qc)) + \
                                    [4 * qc, 4 * qc + 1, 4 * qc + 3, 4 * qc + 2]
                                groups, cur = [], []
                                cols = 0
                                for kb in order:
                                    r = max(0, kb - 4 * qc)
                                    qoff, n = q0 + r * 128, QCHUNK - r * 128
                                    if cols + n > WIDE:
                                        groups.append(cur)
                                        cur, cols = [], 0
                                    cur.append((kb, qoff, n, cols))
                                    cols += n
                                groups.append(cur)
                                last_kb = groups[-1][-1][0]


                                ytps = [ytp.tile([65, QCHUNK], F32,
                                                 tag=f"ytps{i}", name=f"ytps{i}")
                                        for i in range(2)]
                                for grp in groups:
                                    gcols = grp[-1][3] + grp[-1][2]
                                    scs = [scp.tile([128, WIDE], F32, tag=f"sc{i}",
                                                    name=f"sc{i}") for i in range(2)]
                                    for i in range(2):
                                        rows = slice(i * 64, i * 64 + 64)
                                        for (kb, qoff, n, o) in grp:
                                            nc.tensor.matmul(
                                                scs[i][:, o:o + n],
                                                kt[rows, kb * 128:(kb + 1) * 128],
                                                qt[rows, qoff:qoff + n],
                                                start=True, stop=True)
                                    for i in range(2):
                                        h = 2 * hp + i
                                        vcol = slice(h * 65, h * 65 + 65)
                                        pe = pex.tile([128, WIDE], BF16,
                                                      tag=f"pe{i}", name=f"pe{i}")
                                        nc.scalar.activation(pe[:, :gcols],
                                                             scs[i][:, :gcols],
                                                             EXP, scale=0.125)
                                        for (kb, qoff, n, o) in grp:
                                            if kb >= 4 * qc:  # diagonal: causal mask
                                                nc.vector.tensor_mul(
                                                    pe[:, o:o + 128], pe[:, o:o + 128],
                                                    maskT[:])
                                            # kb==0 always has n=512: start clears
                                            # the whole [65, QCHUNK] accumulator
                                            nc.tensor.matmul(
                                                ytps[i][:, qoff - q0:qoff - q0 + n],
                                                vsb[:, kb, vcol],
                                                pe[:, o:o + n],
                                                start=(kb == 0), stop=(kb == last_kb))
                                for i in range(2):
                                    h = 2 * hp + i
                                    # single copy releases the psum bank for
                                    # the next chunk's PV; normalize from SBUF
                                    ytu = work.tile([65, QCHUNK], F32, tag="ytu")
                                    nc.vector.tensor_copy(ytu[:], ytps[i][:])
                                    rc = work.tile([1, QCHUNK], F32, tag="rc")
                                    nc.vector.reciprocal(rc[:], ytu[64:65, :])
                                    bc = work.tile([64, QCHUNK], F32, tag="bc")
                                    nc.gpsimd.partition_broadcast(bc[:], rc[0:1, :])
                                    nc.vector.tensor_mul(
                                        yt2[hp][i * 64:i * 64 + 64, q0:q0 + QCHUNK],
                                        ytu[0:64, :], bc[:])

                    # ---- phase 3: o_proj ----
                    if 3 in phases:
                        with tc.tile_pool(name="op", bufs=2, space="PSUM") as op:
                            for m in range(SB):
                                ms = slice(m * 128, (m + 1) * 128)
                                for nb in range(2):
                                    po = op.tile([128, 512], F32, tag="po")
                                    for k in range(2):
                                        nc.tensor.matmul(po[:], yt2[k][:, ms],
                                                         wo[:, k, nb * 512:(nb + 1) * 512],
                                                         start=(k == 0), stop=(k == 1))
                                    so = work.tile([128, 512], F32, tag="so")
                                    nc.vector.tensor_copy(so[:], po[:])
                                    nc.sync.dma_start(
                                        out_d[ms, nb * 512:(nb + 1) * 512], so[:])
                                    if timing and out_small is not None and m == 0 and nb == 0:
                                        nc.sync.dma_start(out_small[:], so[:])
    nc.compile()
    return nc


def _prep_core_inputs(x, Wq, Wk, Wv, Wo, cos_g, sin_g, use_rope):
    """Host-side shard + layout prep. Returns list of 8 input dicts."""
    maskT = np.tril(np.ones((128, 128), np.float32)).T.astype(_BF16)
    # interleave cos/sin to the 256-wide repeating pattern used by rope
    cos8 = np.tile(cos_g, (1, 8)).astype(_BF16)
    sin8 = np.tile(sin_g, (1, 8)).astype(_BF16)
    maps = []
    for c in range(NCORES):
        b, g = divmod(c, HEADS_PER_CORE)
        rows = slice(g * GDIM, (g + 1) * GDIM)
        wqk = np.concatenate([Wq[rows], Wk[rows]], axis=0).T  # [D, 512]
        maps.append({
            "xt": np.ascontiguousarray(x[b].T).astype(_BF16),
            "wqk": np.ascontiguousarray(wqk).astype(_BF16),
            "wv": np.ascontiguousarray(Wv[rows].T).astype(_BF16),
            "wo": np.ascontiguousarray(Wo[:, rows].T).astype(_BF16),
            "cos8": cos8,
            "sin8": sin8,
            "maskT": maskT,
        })
    return maps


def kernel(x, token_positions, use_rope, Wq, Wk, Wv, Wo, cos, sin):
    from concourse.bass_utils import run_bass_kernel_spmd

    x = np.asarray(x, np.float32)
    token_positions = np.asarray(token_positions)
    Wq = np.asarray(Wq, np.float32)
    Wk = np.asarray(Wk, np.float32)
    Wv = np.asarray(Wv, np.float32)
    Wo = np.asarray(Wo, np.float32)
    cos = np.asarray(cos, np.float32)
    sin = np.asarray(sin, np.float32)
    rope = bool(int(use_rope))

    cos_g = cos[token_positions]  # [S, 32]
    sin_g = sin[token_positions]

    if rope not in _cache:
        _cache[rope] = _build(rope)
    nc = _cache[rope]

    in_maps = _prep_core_inputs(x, Wq, Wk, Wv, Wo, cos_g, sin_g, rope)
    res = run_bass_kernel_spmd(nc, in_maps, list(range(NCORES)))

    out = np.zeros((B, S, D), np.float32)
    for c in range(NCORES):
        out[c // HEADS_PER_CORE] += res.results[c]["out"]
    return out



# revision 12
# speedup vs baseline: 1.1295x; 1.1295x over previous
"""Causal multi-head attention with RoPE for Trainium2, 8-core SPMD.

Problem: B=2, S=2048, D_MODEL=1024, H=16, HD=64, causal softmax(QK^T/8)V
with interleaved-pair RoPE on q/k, projections Wq/Wk/Wv/Wo.

Sharding (host side): batch x head-group. Core c handles batch b=c//4 and
head group g=c%4 (heads 4g..4g+3, a 256-wide slice of the projection dims).
Each core computes a full [S, D_MODEL] partial of the output (its head
group's contribution through Wo); host sums 4 partials per batch.

Device schedule (all matmuls bf16, fp32 accumulate):
 - host passes x[b].T so the d-contraction sits on SBUF partitions
 - Wq/Wk rows are permuted per head to [evens | odds] so RoPE pairs are
   (col j, col j+32) within each head: the DVE ops run on contiguous
   32-wide groups (packed, 2x mode) and cos/sin tables are the raw
   [S, 32] tables read through stride-0 broadcast views
 - phase A: Q,K projected in [s, o] layout -> ACT cast-copy to bf16 ->
   RoPE on DVE -> one wide DMA transpose per s-tile into qkt4
 - phase B: per q-chunk, scores^T[k, q] = Kt.T @ Qt per 128-key block
   (K=64 contraction) into 1024-wide PSUM units; one Exp per unit (ACT),
   causal mask on the diagonal blocks (DVE); PV with lhsT = [V | 1]
   (M=65) so row 64 accumulates the softmax denominator for free;
   normalization reads PSUM directly (reciprocal + gpsimd broadcast +
   DVE mul)
 - the V projection runs inside phase B as PE filler at q-chunk
   boundaries (vsb s-tiles are only consumed by the diagonal units,
   several pipeline steps later), so PE never idles while ACT streams
   exps; o_proj of the previous chunk slots in the same place
 - PV accumulators and o_proj PSUM share banks (tags big0/big1); the
   tail o_proj accumulates in the freed scores banks with the
   normalize chain split per m-tile to shorten the critical path
"""

import numpy as np
import ml_dtypes

B, S, D, H = 2, 2048, 1024, 16
HD = 64
NCORES = 8
HEADS_PER_CORE = 4
GDIM = HEADS_PER_CORE * HD          # 256 projection cols per core
SB = S // 128                        # 16 s-tiles
KD = D // 128                        # 8 k-tiles over d
QCHUNK = 512
NQC = S // QCHUNK                    # 4 q-chunks
UNIT = 1024                          # scores psum unit (2 banks)

_BF16 = ml_dtypes.bfloat16
_cache = {}


def _build(use_rope: bool):
    import concourse.bass as bass
    import concourse.mybir as mybir
    import concourse.tile as tile
    from concourse import bacc

    F32 = mybir.dt.float32
    BF16 = mybir.dt.bfloat16
    EXP = mybir.ActivationFunctionType.Exp

    nc = bacc.Bacc(None, target_bir_lowering=False)

    xt_d = nc.dram_tensor("xt", [D, S], BF16, kind="ExternalInput")
    wqk_d = nc.dram_tensor("wqk", [D, 2 * GDIM], BF16, kind="ExternalInput")
    wv_d = nc.dram_tensor("wv", [D, GDIM], BF16, kind="ExternalInput")
    wo_d = nc.dram_tensor("wo", [GDIM, D], BF16, kind="ExternalInput")
    cos_d = nc.dram_tensor("cosr", [S, 32], BF16, kind="ExternalInput")
    sin_d = nc.dram_tensor("sinr", [S, 32], BF16, kind="ExternalInput")
    mask_d = nc.dram_tensor("maskT", [128, 128], BF16, kind="ExternalInput")
    # bf16 output halves the output-DMA bytes; the host accumulates the
    # 4 per-batch partials in fp32
    out_d = nc.dram_tensor("out", [S, D], BF16, kind="ExternalOutput")

    with tile.TileContext(nc) as tc:
        with tc.tile_pool(name="big", bufs=1) as big, \
             tc.tile_pool(name="work", bufs=3) as work, \
             tc.tile_pool(name="ropet", bufs=3) as ropet, \
             tc.tile_pool(name="pex", bufs=3) as pex:
            # ---- resident tensors ----
            xt = big.tile([128, KD, S], BF16)
            wqk = big.tile([128, KD, 2 * GDIM], BF16)
            wv = big.tile([128, KD, GDIM], BF16)
            wo = big.tile([128, 2, D], BF16)
            maskT = big.tile([128, 128], BF16)
            cosr = big.tile([128, SB, 32], BF16)
            sinr = big.tile([128, SB, 32], BF16)
            qkt4 = big.tile([128, 4, S], BF16)
            # qkt4 blocks: 0: Qt heads 0-1, 1: Qt heads 2-3, 2: Kt 0-1, 3: Kt 2-3
            vsb = big.tile([128, SB, HEADS_PER_CORE * 65], BF16)
            yt2 = [big.tile([128, S], BF16, tag=f"yt2{i}", name=f"yt2{i}")
                   for i in range(2)]

            # ones-rows of vsb (column 64 of each 65-wide head slot)
            ones_view = vsb[:].rearrange("p m (h c) -> p m h c", h=4)[:, :, :, 64:65]
            nc.vector.memset(ones_view, 1.0)

            # ---- chunked input loads, ordered so x streams ahead of use ----
            xt_r = xt_d.rearrange("(k p) s -> p k s", p=128)
            wqk_r = wqk_d.rearrange("(k p) o -> p k o", p=128)
            for kq in range(2):
                ks = slice(kq * 4, kq * 4 + 4)
                nc.sync.dma_start(wqk[:, ks, :], wqk_r[:, ks, :])
                nc.sync.dma_start(xt[:, ks, 0:128], xt_r[:, ks, 0:128])
            nc.sync.dma_start(xt[:, :, 128:640], xt_r[:, :, 128:640])
            nc.sync.dma_start(cosr[:], cos_d.rearrange("(m p) f -> p m f", p=128))
            nc.sync.dma_start(sinr[:], sin_d.rearrange("(m p) f -> p m f", p=128))
            nc.sync.dma_start(wv[:], wv_d.rearrange("(k p) o -> p k o", p=128))
            nc.sync.dma_start(maskT[:], mask_d[:])
            nc.sync.dma_start(xt[:, :, 640:1536], xt_r[:, :, 640:1536])
            nc.sync.dma_start(xt[:, :, 1536:2048], xt_r[:, :, 1536:2048])
            nc.sync.dma_start(wo[:], wo_d.rearrange("(k p) o -> p k o", p=128))

            # ---- phase A: QK projections + rope + transpose ----
            act_warm = False
            with tc.tile_pool(name="pa", bufs=2, space="PSUM") as pa:
                for m in range(SB):
                    ms = slice(m * 128, (m + 1) * 128)
                    ps = pa.tile([128, 2 * GDIM], F32, tag="qk")
                    for k in range(KD):
                        nc.tensor.matmul(ps[:], xt[:, k, ms], wqk[:, k, :],
                                         start=(k == 0), stop=(k == KD - 1))
                    qkr = ropet.tile([128, 2 * GDIM], BF16, tag="qkr")
                    if use_rope:
                        qkf = ropet.tile([128, 2 * GDIM], BF16, tag="qkf")
                        nc.scalar.copy(qkf[:], ps[:])
                        qv = qkf[:].rearrange("p (g e c) -> p g e c", e=2, c=32)
                        ov = qkr[:].rearrange("p (g e c) -> p g e c", e=2, c=32)
                        E, O = qv[:, :, 0, :], qv[:, :, 1, :]
                        C = cosr[:, m, :].unsqueeze(1).broadcast_to((128, 8, 32))
                        Sn = sinr[:, m, :].unsqueeze(1).broadcast_to((128, 8, 32))
                        g32 = lambda t: t[:].rearrange("p (g c) -> p g c", c=32)
                        ta = ropet.tile([128, 256], BF16, tag="ta")
                        tb = ropet.tile([128, 256], BF16, tag="tb")
                        nc.vector.tensor_mul(g32(ta), E, C)
                        nc.vector.tensor_mul(g32(tb), O, Sn)
                        nc.vector.tensor_sub(ov[:, :, 0, :], g32(ta), g32(tb))
                        tc_ = ropet.tile([128, 256], BF16, tag="tc")
                        td = ropet.tile([128, 256], BF16, tag="td")
                        nc.vector.tensor_mul(g32(tc_), O, C)
                        nc.vector.tensor_mul(g32(td), E, Sn)
                        nc.vector.tensor_add(ov[:, :, 1, :], g32(tc_), g32(td))
                    else:
                        nc.scalar.copy(qkr[:], ps[:])
                    nc.sync.dma_start_transpose(qkt4[:, :, ms], qkr[:])
                    if not act_warm:
                        # preload the Exp table while ACT is idle so the
                        # first phase-B exp doesn't pay the 1.3us load
                        wa = work.tile([1, 1], BF16, tag="wa")
                        nc.scalar.activation(wa[:], ta[0:1, 0:1] if use_rope
                                             else qkr[0:1, 0:1], EXP, scale=0.125)
                        act_warm = True

            # ---- phase B: attention + V projection + interleaved o_proj ----
            with tc.tile_pool(name="sc", bufs=1, space="PSUM") as scp, \
                 tc.tile_pool(name="yb", bufs=1, space="PSUM") as ybp, \
                 tc.tile_pool(name="pv", bufs=2, space="PSUM") as pvp:

                def emit_vproj_tile(m):
                    ms = slice(m * 128, (m + 1) * 128)
                    psv = pvp.tile([128, GDIM], F32, tag="v")
                    for k in range(KD):
                        nc.tensor.matmul(psv[:], xt[:, k, ms], wv[:, k, :],
                                         start=(k == 0), stop=(k == KD - 1))
                    dst = vsb[:, m, :].rearrange("p (h c) -> p h c", h=4)[:, :, 0:64]
                    nc.scalar.copy(dst, psv[:].rearrange("p (h c) -> p h c", h=4))

                def emit_oproj_tile(m):
                    ms = slice(m * 128, (m + 1) * 128)
                    so = work.tile([128, 1024], BF16, tag="so")
                    for nb in range(2):
                        po = ybp.tile([128, QCHUNK], F32, tag=f"big{nb}")
                        for k2 in range(2):
                            nc.tensor.matmul(po[:], yt2[k2][:, ms],
                                             wo[:, k2, nb * 512:(nb + 1) * 512],
                                             start=(k2 == 0), stop=(k2 == 1))
                        nc.vector.tensor_copy(so[:, nb * 512:(nb + 1) * 512], po[:])
                    nc.sync.dma_start(out_d[ms, :], so[:])

                for qc in range(NQC):
                    q0 = qc * QCHUNK
                    # block list: full key blocks then the diagonal in order
                    # 512,384,128,256 so no matmul crosses a 512-col bank
                    order = list(range(4 * qc)) + \
                        [4 * qc, 4 * qc + 1, 4 * qc + 3, 4 * qc + 2]
                    blocks = []
                    for kb in order:
                        r = max(0, kb - 4 * qc)
                        blocks.append((kb, q0 + r * 128, QCHUNK - r * 128))
                    units, cur, cols = [], [], 0
                    for kb, qoff, n in blocks:
                        if cols + n > UNIT:
                            units.append(cur)
                            cur, cols = [], 0
                        cur.append((kb, qoff, n, cols))
                        cols += n
                    units.append(cur)
                    last_kb = units[-1][-1][0]

                    for hp in range(2):
                        ytps = None
                        for ui, unit in enumerate(units):
                            ucols = unit[-1][3] + unit[-1][2]
                            scs = [scp.tile([128, UNIT], F32, tag=f"sc{i}",
                                            name=f"sc{i}") for i in range(2)]
                            for i in range(2):
                                rows = slice(i * 64, i * 64 + 64)
                                for (kb, qoff, n, o) in unit:
                                    nc.tensor.matmul(
                                        scs[i][:, o:o + n],
                                        qkt4[rows, 2 + hp, kb * 128:(kb + 1) * 128],
                                        qkt4[rows, hp, qoff:qoff + n],
                                        start=True, stop=True)
                            # PE filler between scores and PV: previous
                            # chunk's o_proj and this chunk's V projection.
                            # Both must precede this chunk's ytps allocation
                            # (the po tiles reuse the big0/big1 banks); the
                            # vsb tiles feed only the diagonal units, which
                            # sit several pipeline steps later.
                            if hp == 0 and ui == 0:
                                if qc > 0:
                                    for mi in range(4):
                                        emit_oproj_tile((qc - 1) * 4 + mi)
                                for mi in range(4):
                                    emit_vproj_tile(qc * 4 + mi)
                            if ytps is None:
                                ytps = [ybp.tile([128, QCHUNK], F32,
                                                 tag=f"big{i}", name=f"big{i}")
                                        for i in range(2)]
                            for i in range(2):
                                h = 2 * hp + i
                                vcol = slice(h * 65, h * 65 + 65)
                                pe = pex.tile([128, UNIT], BF16,
                                              tag=f"pe{i}", name=f"pe{i}")
                                nc.scalar.activation(pe[:, :ucols],
                                                     scs[i][:, :ucols],
                                                     EXP, scale=0.125)
                                for (kb, qoff, n, o) in unit:
                                    if kb >= 4 * qc:  # diagonal: causal mask
                                        nc.vector.tensor_mul(
                                            pe[:, o:o + 128], pe[:, o:o + 128],
                                            maskT[:])
                                    # kb==0 always has n=512: start clears
                                    # the whole [65, QCHUNK] accumulator
                                    nc.tensor.matmul(
                                        ytps[i][0:65, qoff - q0:qoff - q0 + n],
                                        vsb[:, kb, vcol],
                                        pe[:, o:o + n],
                                        start=(kb == 0), stop=(kb == last_kb))
                        if qc == NQC - 1 and hp == 1:
                            # tail: full-width reciprocals (safe: after the
                            # accumulation stop), then per-m-tile broadcast/
                            # mul chains staggered with the o_proj pairs so
                            # the last output DMA starts as early as possible
                            rcs = []
                            for i in range(2):
                                rc = work.tile([1, QCHUNK], F32, tag=f"rct{i}")
                                nc.vector.reciprocal(rc[:], ytps[i][64:65, :])
                                rcs.append(rc)
                            for mi in range(4):
                                mc = slice(mi * 128, (mi + 1) * 128)
                                for i in range(2):
                                    bc = work.tile([64, 128], F32, tag="bct")
                                    nc.gpsimd.partition_broadcast(
                                        bc[:], rcs[i][0:1, mc])
                                    nc.vector.tensor_mul(
                                        yt2[hp][i * 64:i * 64 + 64,
                                                q0 + mi * 128:q0 + (mi + 1) * 128],
                                        ytps[i][0:64, mc], bc[:])
                        else:
                            for i in range(2):
                                # normalize straight from PSUM
                                rc = work.tile([1, QCHUNK], F32, tag="rc")
                                nc.vector.reciprocal(rc[:], ytps[i][64:65, :])
                                bc = work.tile([64, QCHUNK], F32, tag="bc")
                                nc.gpsimd.partition_broadcast(bc[:], rc[0:1, :])
                                nc.vector.tensor_mul(
                                    yt2[hp][i * 64:i * 64 + 64, q0:q0 + QCHUNK],
                                    ytps[i][0:64, :], bc[:])
                    if qc == NQC - 1:
                        # tail o_proj in the freed scores banks; k2=0
                        # matmuls run during the normalize chains
                        for mi in range(4):
                            m = qc * 4 + mi
                            ms = slice(m * 128, (m + 1) * 128)
                            pot = scp.tile([128, UNIT], F32,
                                           tag=f"sc{mi % 2}", name=f"sc{mi % 2}")
                            for k2 in range(2):
                                for nb in range(2):
                                    nc.tensor.matmul(
                                        pot[:, nb * 512:(nb + 1) * 512],
                                        yt2[k2][:, ms],
                                        wo[:, k2, nb * 512:(nb + 1) * 512],
                                        start=(k2 == 0), stop=(k2 == 1))
                            for nb in range(2):
                                # copies alternate DVE/ACT: both are idle at
                                # the tail and this halves the serial chain
                                so = work.tile([128, 512], BF16, tag="sot")
                                if nb == 0:
                                    nc.vector.tensor_copy(
                                        so[:], pot[:, 0:512])
                                else:
                                    nc.scalar.copy(so[:], pot[:, 512:1024])
                                nc.sync.dma_start(
                                    out_d[ms, nb * 512:(nb + 1) * 512], so[:])
    nc.compile()
    return nc


def _prep_core_inputs(x, Wq, Wk, Wv, Wo, cos_g, sin_g, use_rope):
    """Host-side shard + layout prep. Returns list of 8 input dicts."""
    maskT = np.tril(np.ones((128, 128), np.float32)).T.astype(_BF16)
    # per-head row permutation: [evens | odds] so rope pairs are
    # (j, j+32) within each head's 64 projection dims
    perm = np.concatenate([np.arange(h * 64, (h + 1) * 64).reshape(32, 2).T.reshape(64)
                           for h in range(H)])
    Wq_p = Wq[perm]
    Wk_p = Wk[perm]
    maps = []
    for c in range(NCORES):
        b, g = divmod(c, HEADS_PER_CORE)
        rows = slice(g * GDIM, (g + 1) * GDIM)
        wqk = np.concatenate([Wq_p[rows], Wk_p[rows]], axis=0).T  # [D, 512]
        maps.append({
            "xt": np.ascontiguousarray(x[b].T).astype(_BF16),
            "wqk": np.ascontiguousarray(wqk).astype(_BF16),
            "wv": np.ascontiguousarray(Wv[rows].T).astype(_BF16),
            "wo": np.ascontiguousarray(Wo[:, rows].T).astype(_BF16),
            "cosr": cos_g.astype(_BF16),
            "sinr": sin_g.astype(_BF16),
            "maskT": maskT,
        })
    return maps


def kernel(x, token_positions, use_rope, Wq, Wk, Wv, Wo, cos, sin):
    from concourse.bass_utils import run_bass_kernel_spmd

    x = np.asarray(x, np.float32)
    token_positions = np.asarray(token_positions)
    Wq = np.asarray(Wq, np.float32)
    Wk = np.asarray(Wk, np.float32)
    Wv = np.asarray(Wv, np.float32)
    Wo = np.asarray(Wo, np.float32)
    cos = np.asarray(cos, np.float32)
    sin = np.asarray(sin, np.float32)
    rope = bool(int(use_rope))

    cos_g = cos[token_positions]  # [S, 32]
    sin_g = sin[token_positions]

    if rope not in _cache:
        _cache[rope] = _build(rope)
    nc = _cache[rope]

    in_maps = _prep_core_inputs(x, Wq, Wk, Wv, Wo, cos_g, sin_g, rope)
    res = run_bass_kernel_spmd(nc, in_maps, list(range(NCORES)))

    out = np.zeros((B, S, D), np.float32)
    for c in range(NCORES):
        out[c // HEADS_PER_CORE] += res.results[c]["out"].astype(np.float32)
    return out


# revision 25
# speedup vs baseline: 1.1815x; 1.0460x over previous
"""Causal multi-head attention with RoPE for Trainium2, 8-core SPMD.

Problem: B=2, S=2048, D_MODEL=1024, H=16, HD=64, causal softmax(QK^T/8)V
with interleaved-pair RoPE on q/k, projections Wq/Wk/Wv/Wo.

Sharding (host side): batch x head-group. Core c handles batch b=c//4 and
head group g=c%4 (heads 4g..4g+3, a 256-wide slice of the projection dims).
Each core computes a full [S, D_MODEL] partial of the output (its head
group's contribution through Wo); host sums 4 partials per batch.

Device schedule (all matmuls bf16, fp32 accumulate):
 - host passes x[b].T so the d-contraction sits on SBUF partitions
 - Wq/Wk rows are permuted per head to [evens | odds] so RoPE pairs are
   (col j, col j+32) within each head: the DVE ops run on contiguous
   32-wide groups (packed, 2x mode) and cos/sin tables are the raw
   [S, 32] tables read through stride-0 broadcast views
 - phase A: Q,K projected in [s, o] layout -> ACT cast-copy to bf16 ->
   RoPE on DVE -> one wide DMA transpose per s-tile into qkt4
 - phase B: per q-chunk, scores^T[k, q] = Kt.T @ Qt per 128-key block
   (K=64 contraction) into 1024-wide PSUM units; one Exp per unit (ACT),
   causal mask on the diagonal blocks (DVE); PV with lhsT = [V | 1]
   (M=65) so row 64 accumulates the softmax denominator for free;
   normalization reads PSUM directly (reciprocal + gpsimd broadcast +
   DVE mul)
 - the V projection runs inside phase B as PE filler at q-chunk
   boundaries (vsb s-tiles are only consumed by the diagonal units,
   several pipeline steps later), so PE never idles while ACT streams
   exps; o_proj of the previous chunk slots in the same place
 - PV accumulators and o_proj PSUM share banks (tags big0/big1); the
   tail o_proj accumulates in the freed scores banks with the
   normalize chain split per m-tile to shorten the critical path
"""

import numpy as np
import ml_dtypes

B, S, D, H = 2, 2048, 1024, 16
HD = 64
NCORES = 8
HEADS_PER_CORE = 4
GDIM = HEADS_PER_CORE * HD          # 256 projection cols per core
SB = S // 128                        # 16 s-tiles
KD = D // 128                        # 8 k-tiles over d
QCHUNK = 512
NQC = S // QCHUNK                    # 4 q-chunks
UNIT = 1024                          # scores psum unit (2 banks)

_BF16 = ml_dtypes.bfloat16
_cache = {}


def _build(use_rope: bool):
    import concourse.bass as bass
    import concourse.mybir as mybir
    import concourse.tile as tile
    from concourse import bacc

    F32 = mybir.dt.float32
    BF16 = mybir.dt.bfloat16
    EXP = mybir.ActivationFunctionType.Exp

    nc = bacc.Bacc(None, target_bir_lowering=False)

    xt_d = nc.dram_tensor("xt", [D, S], BF16, kind="ExternalInput")
    wqk_d = nc.dram_tensor("wqk", [D, 2 * GDIM], BF16, kind="ExternalInput")
    wv_d = nc.dram_tensor("wv", [D, GDIM], BF16, kind="ExternalInput")
    wo_d = nc.dram_tensor("wo", [GDIM, D], BF16, kind="ExternalInput")
    cos_d = nc.dram_tensor("cosr", [S, 32], BF16, kind="ExternalInput")
    sin_d = nc.dram_tensor("sinr", [S, 32], BF16, kind="ExternalInput")
    mask_d = nc.dram_tensor("maskT", [128, 128], BF16, kind="ExternalInput")
    # bf16 output halves the output-DMA bytes; the host accumulates the
    # 4 per-batch partials in fp32
    out_d = nc.dram_tensor("out", [S, D], BF16, kind="ExternalOutput")

    with tile.TileContext(nc) as tc:
        with tc.tile_pool(name="big", bufs=1) as big, \
             tc.tile_pool(name="work", bufs=3) as work, \
             tc.tile_pool(name="ropet", bufs=3) as ropet, \
             tc.tile_pool(name="pex", bufs=3) as pex:
            # ---- resident tensors ----
            xt = big.tile([128, KD, S], BF16)
            wqk = big.tile([128, KD, 2 * GDIM], BF16)
            wv = big.tile([128, KD, GDIM], BF16)
            wo = big.tile([128, 2, D], BF16)
            maskT = big.tile([128, 128], BF16)
            cosr = big.tile([128, SB, 32], BF16)
            sinr = big.tile([128, SB, 32], BF16)
            qkt4 = big.tile([128, 4, S], BF16)
            # qkt4 blocks: 0: Qt heads 0-1, 1: Qt heads 2-3, 2: Kt 0-1, 3: Kt 2-3
            vsb = big.tile([128, SB, HEADS_PER_CORE * 65], BF16)
            yt2 = [big.tile([128, S], BF16, tag=f"yt2{i}", name=f"yt2{i}")
                   for i in range(2)]

            # ones-rows of vsb (column 64 of each 65-wide head slot)
            ones_view = vsb[:].rearrange("p m (h c) -> p m h c", h=4)[:, :, :, 64:65]
            nc.vector.memset(ones_view, 1.0)

            # ---- chunked input loads, ordered so x streams ahead of use ----
            xt_r = xt_d.rearrange("(k p) s -> p k s", p=128)
            wqk_r = wqk_d.rearrange("(k p) o -> p k o", p=128)
            nc.sync.dma_start(xt[:, 0:4, 0:128], xt_r[:, 0:4, 0:128])
            nc.sync.dma_start(wqk[:, 0:2, :], wqk_r[:, 0:2, :])
            nc.sync.dma_start(wqk[:, 2:4, :], wqk_r[:, 2:4, :])
            nc.sync.dma_start(wqk[:, 4:6, :], wqk_r[:, 4:6, :])
            nc.sync.dma_start(wqk[:, 6:8, :], wqk_r[:, 6:8, :])
            nc.sync.dma_start(xt[:, 4:8, 0:128], xt_r[:, 4:8, 0:128])
            nc.sync.dma_start(xt[:, :, 128:384], xt_r[:, :, 128:384])
            nc.sync.dma_start(xt[:, :, 384:640], xt_r[:, :, 384:640])
            nc.sync.dma_start(cosr[:], cos_d.rearrange("(m p) f -> p m f", p=128))
            nc.sync.dma_start(sinr[:], sin_d.rearrange("(m p) f -> p m f", p=128))
            nc.sync.dma_start(wv[:], wv_d.rearrange("(k p) o -> p k o", p=128))
            nc.sync.dma_start(maskT[:], mask_d[:])
            nc.sync.dma_start(xt[:, :, 640:1536], xt_r[:, :, 640:1536])
            nc.sync.dma_start(xt[:, :, 1536:2048], xt_r[:, :, 1536:2048])
            nc.sync.dma_start(wo[:], wo_d.rearrange("(k p) o -> p k o", p=128))

            # ---- phase A: QK projections + rope + transpose ----
            act_warm = False
            with tc.tile_pool(name="pa", bufs=2, space="PSUM") as pa:
                for m in range(SB):
                    ms = slice(m * 128, (m + 1) * 128)
                    ps = pa.tile([128, 2 * GDIM], F32, tag="qk")
                    for k in range(KD):
                        nc.tensor.matmul(ps[:], xt[:, k, ms], wqk[:, k, :],
                                         start=(k == 0), stop=(k == KD - 1))
                    qkr = ropet.tile([128, 2 * GDIM], BF16, tag="qkr")
                    if use_rope:
                        qkf = ropet.tile([128, 2 * GDIM], BF16, tag="qkf")
                        nc.scalar.copy(qkf[:], ps[:])
                        qv = qkf[:].rearrange("p (g e c) -> p g e c", e=2, c=32)
                        ov = qkr[:].rearrange("p (g e c) -> p g e c", e=2, c=32)
                        E, O = qv[:, :, 0, :], qv[:, :, 1, :]
                        C = cosr[:, m, :].unsqueeze(1).broadcast_to((128, 8, 32))
                        Sn = sinr[:, m, :].unsqueeze(1).broadcast_to((128, 8, 32))
                        g32 = lambda t: t[:].rearrange("p (g c) -> p g c", c=32)
                        ta = ropet.tile([128, 256], BF16, tag="ta")
                        tb = ropet.tile([128, 256], BF16, tag="tb")
                        nc.vector.tensor_mul(g32(ta), E, C)
                        nc.vector.tensor_mul(g32(tb), O, Sn)
                        nc.vector.tensor_sub(ov[:, :, 0, :], g32(ta), g32(tb))
                        tc_ = ropet.tile([128, 256], BF16, tag="tc")
                        td = ropet.tile([128, 256], BF16, tag="td")
                        nc.vector.tensor_mul(g32(tc_), O, C)
                        nc.vector.tensor_mul(g32(td), E, Sn)
                        nc.vector.tensor_add(ov[:, :, 1, :], g32(tc_), g32(td))
                    else:
                        nc.scalar.copy(qkr[:], ps[:])
                    nc.sync.dma_start_transpose(qkt4[:, :, ms], qkr[:])
                    if not act_warm:
                        # preload the Exp table while ACT is idle so the
                        # first phase-B exp doesn't pay the 1.3us load
                        wa = work.tile([1, 1], BF16, tag="wa")
                        nc.scalar.activation(wa[:], ta[0:1, 0:1] if use_rope
                                             else qkr[0:1, 0:1], EXP, scale=0.125)
                        act_warm = True

            # ---- phase B: attention + V projection + interleaved o_proj ----
            with tc.tile_pool(name="sc", bufs=1, space="PSUM") as scp, \
                 tc.tile_pool(name="yb", bufs=1, space="PSUM") as ybp, \
                 tc.tile_pool(name="pv", bufs=2, space="PSUM") as pvp:

                def emit_vproj_tile(m):
                    ms = slice(m * 128, (m + 1) * 128)
                    psv = pvp.tile([128, QCHUNK], F32, tag="v")
                    for k in range(KD):
                        nc.tensor.matmul(psv[:, 0:GDIM], xt[:, k, ms], wv[:, k, :],
                                         start=(k == 0), stop=(k == KD - 1))
                    dst = vsb[:, m, :].rearrange("p (h c) -> p h c", h=4)[:, :, 0:64]
                    nc.scalar.copy(dst, psv[:, 0:GDIM].rearrange("p (h c) -> p h c", h=4))

                def emit_oproj_tile(m, use_pv=False):
                    ms = slice(m * 128, (m + 1) * 128)
                    so = work.tile([128, 1024], BF16, tag="so")
                    for nb in range(2):
                        if use_pv:
                            # pv-pool accumulator: avoids waiting on the
                            # ytps (big-tag) release behind the normalize
                            po = pvp.tile([128, QCHUNK], F32, tag="v")
                        else:
                            po = ybp.tile([128, QCHUNK], F32, tag=f"big{nb}")
                        for k2 in range(2):
                            nc.tensor.matmul(po[:], yt2[k2][:, ms],
                                             wo[:, k2, nb * 512:(nb + 1) * 512],
                                             start=(k2 == 0), stop=(k2 == 1))
                        # copies alternate DVE/ACT to halve the serial chain
                        if nb == 0:
                            nc.vector.tensor_copy(so[:, 0:512], po[:])
                        else:
                            nc.scalar.copy(so[:, 512:1024], po[:])
                    nc.sync.dma_start(out_d[ms, :], so[:])

                # qc0's diagonal PV needs vsb s-tiles 0-3 immediately;
                # project them first (also keeps PE warm across the
                # phase boundary)
                for mi in range(4):
                    emit_vproj_tile(mi)

                for qc in range(NQC):
                    q0 = qc * QCHUNK
                    # block list: full key blocks then the diagonal in order
                    # 512,384,128,256 so no matmul crosses a 512-col bank
                    order = list(range(4 * qc)) + \
                        [4 * qc, 4 * qc + 1, 4 * qc + 3, 4 * qc + 2]
                    blocks = []
                    for kb in order:
                        r = max(0, kb - 4 * qc)
                        blocks.append((kb, q0 + r * 128, QCHUNK - r * 128))
                    units, cur, cols = [], [], 0
                    for kb, qoff, n in blocks:
                        if cols + n > UNIT:
                            units.append(cur)
                            cur, cols = [], 0
                        cur.append((kb, qoff, n, cols))
                        cols += n
                    units.append(cur)
                    last_kb = units[-1][-1][0]

                    for hp in range(2):
                        ytps = None
                        for ui, unit in enumerate(units):
                            ucols = unit[-1][3] + unit[-1][2]
                            scs = [scp.tile([128, UNIT], F32, tag=f"sc{i}",
                                            name=f"sc{i}") for i in range(2)]
                            for i in range(2):
                                rows = slice(i * 64, i * 64 + 64)
                                for (kb, qoff, n, o) in unit:
                                    nc.tensor.matmul(
                                        scs[i][:, o:o + n],
                                        qkt4[rows, 2 + hp, kb * 128:(kb + 1) * 128],
                                        qkt4[rows, hp, qoff:qoff + n],
                                        start=True, stop=True)
                            # PE filler between scores and PV: previous
                            # chunk's o_proj (must precede this chunk's ytps
                            # allocation — the po tiles reuse the big banks)
                            # and this chunk's V projection, spread across
                            # unit boundaries so ACT's exp stream never
                            # starves. The vsb tiles feed only the diagonal
                            # units, several pipeline steps later.
                            if ui == 0 and qc > 0:
                                # 2 o_proj tiles per hp boundary (hp0's po
                                # allocations must precede its ytps; hp1's
                                # ride the pv pool). The last chunk defers
                                # hp1's pair to the tail as stagger filler.
                                if hp == 0:
                                    for mi in range(2):
                                        emit_oproj_tile((qc - 1) * 4 + mi)
                                elif qc < NQC - 1:
                                    for mi in range(2):
                                        emit_oproj_tile((qc - 1) * 4 + 2 + mi,
                                                        use_pv=True)
                            if hp == 0 and qc > 0 and ui in (1, 2):
                                for mi in range(2):
                                    emit_vproj_tile(qc * 4 + (ui - 1) * 2 + mi)
                            if ytps is None:
                                ytps = [ybp.tile([128, QCHUNK], F32,
                                                 tag=f"big{i}", name=f"big{i}")
                                        for i in range(2)]
                            for i in range(2):
                                h = 2 * hp + i
                                vcol = slice(h * 65, h * 65 + 65)
                                pe = pex.tile([128, UNIT], BF16,
                                              tag=f"pe{i}", name=f"pe{i}")
                                nc.scalar.activation(pe[:, :ucols],
                                                     scs[i][:, :ucols],
                                                     EXP, scale=0.125)
                                for (kb, qoff, n, o) in unit:
                                    if kb >= 4 * qc:  # diagonal: causal mask
                                        nc.vector.tensor_mul(
                                            pe[:, o:o + 128], pe[:, o:o + 128],
                                            maskT[:])
                                    # kb==0 always has n=512: start clears
                                    # the whole [65, QCHUNK] accumulator
                                    nc.tensor.matmul(
                                        ytps[i][0:65, qoff - q0:qoff - q0 + n],
                                        vsb[:, kb, vcol],
                                        pe[:, o:o + n],
                                        start=(kb == 0), stop=(kb == last_kb))
                        if qc == NQC - 1 and hp == 1:
                            # tail: per-m-tile recip/broadcast/mul chains
                            # (safe: all PV accumulation has stopped) so the
                            # o_proj k2=1 pairs stagger in as early as
                            # possible
                            for mi in range(4):
                                mc = slice(mi * 128, (mi + 1) * 128)
                                for i in range(2):
                                    rc = work.tile([1, 128], F32, tag="rcm")
                                    nc.vector.reciprocal(rc[:], ytps[i][64:65, mc])
                                    bc = work.tile([64, 128], F32, tag="bct")
                                    nc.gpsimd.partition_broadcast(
                                        bc[:], rc[0:1, :])
                                    nc.vector.tensor_mul(
                                        yt2[hp][i * 64:i * 64 + 64,
                                                q0 + mi * 128:q0 + (mi + 1) * 128],
                                        ytps[i][0:64, mc], bc[:])
                        else:
                            for i in range(2):
                                # normalize straight from PSUM
                                rc = work.tile([1, QCHUNK], F32, tag="rc")
                                nc.vector.reciprocal(rc[:], ytps[i][64:65, :])
                                bc = work.tile([64, QCHUNK], F32, tag="bc")
                                nc.gpsimd.partition_broadcast(bc[:], rc[0:1, :])
                                nc.vector.tensor_mul(
                                    yt2[hp][i * 64:i * 64 + 64, q0:q0 + QCHUNK],
                                    ytps[i][0:64, :], bc[:])
                    if qc == NQC - 1:
                        # deferred previous-chunk pair: free-running PE
                        # filler that overlaps the tail normalize chains
                        for mi in range(2):
                            emit_oproj_tile((qc - 1) * 4 + 2 + mi, use_pv=True)
                        # tail o_proj with 8 independent PSUM accumulators
                        # (freed scores banks + pv pool + big banks) so no
                        # matmul waits on a copy; all k2=0 matmuls run
                        # during the normalize chains, k2=1 staggers in as
                        # the per-m muls land; copies alternate DVE/ACT
                        slots = []
                        for mi in range(2):
                            pot = scp.tile([128, UNIT], F32,
                                           tag=f"sc{mi}", name=f"sc{mi}")
                            slots.append((pot[:, 0:512], pot[:, 512:1024]))
                        for mi in range(2):
                            a = pvp.tile([128, QCHUNK], F32, tag="v")
                            b = ybp.tile([128, QCHUNK], F32,
                                         tag=f"big{mi}", name=f"big{mi}")
                            slots.append((a[:], b[:]))
                        for k2 in range(2):
                            for mi in range(4):
                                m = qc * 4 + mi
                                ms = slice(m * 128, (m + 1) * 128)
                                for nb in range(2):
                                    nc.tensor.matmul(
                                        slots[mi][nb], yt2[k2][:, ms],
                                        wo[:, k2, nb * 512:(nb + 1) * 512],
                                        start=(k2 == 0), stop=(k2 == 1))
                        for mi in range(4):
                            m = qc * 4 + mi
                            ms = slice(m * 128, (m + 1) * 128)
                            so = work.tile([128, 1024], BF16, tag=f"sot{mi}")
                            for nb in range(2):
                                if nb == 0:
                                    nc.vector.tensor_copy(
                                        so[:, 0:512], slots[mi][nb])
                                else:
                                    nc.scalar.copy(so[:, 512:1024], slots[mi][nb])
                            nc.sync.dma_start(out_d[ms, :], so[:])
    nc.compile()
    return nc


def _prep_core_inputs(x, Wq, Wk, Wv, Wo, cos_g, sin_g, use_rope):
    """Host-side shard + layout prep. Returns list of 8 input dicts."""
    maskT = np.tril(np.ones((128, 128), np.float32)).T.astype(_BF16)
    # per-head row permutation: [evens | odds] so rope pairs are
    # (j, j+32) within each head's 64 projection dims
    perm = np.concatenate([np.arange(h * 64, (h + 1) * 64).reshape(32, 2).T.reshape(64)
                           for h in range(H)])
    Wq_p = Wq[perm]
    Wk_p = Wk[perm]
    maps = []
    for c in range(NCORES):
        b, g = divmod(c, HEADS_PER_CORE)
        rows = slice(g * GDIM, (g + 1) * GDIM)
        wqk = np.concatenate([Wq_p[rows], Wk_p[rows]], axis=0).T  # [D, 512]
        maps.append({
            "xt": np.ascontiguousarray(x[b].T).astype(_BF16),
            "wqk": np.ascontiguousarray(wqk).astype(_BF16),
            "wv": np.ascontiguousarray(Wv[rows].T).astype(_BF16),
            "wo": np.ascontiguousarray(Wo[:, rows].T).astype(_BF16),
            "cosr": cos_g.astype(_BF16),
            "sinr": sin_g.astype(_BF16),
            "maskT": maskT,
        })
    return maps


def kernel(x, token_positions, use_rope, Wq, Wk, Wv, Wo, cos, sin):
    from concourse.bass_utils import run_bass_kernel_spmd

    x = np.asarray(x, np.float32)
    token_positions = np.asarray(token_positions)
    Wq = np.asarray(Wq, np.float32)
    Wk = np.asarray(Wk, np.float32)
    Wv = np.asarray(Wv, np.float32)
    Wo = np.asarray(Wo, np.float32)
    cos = np.asarray(cos, np.float32)
    sin = np.asarray(sin, np.float32)
    rope = bool(int(use_rope))

    cos_g = cos[token_positions]  # [S, 32]
    sin_g = sin[token_positions]

    if rope not in _cache:
        _cache[rope] = _build(rope)
    nc = _cache[rope]

    in_maps = _prep_core_inputs(x, Wq, Wk, Wv, Wo, cos_g, sin_g, rope)
    res = run_bass_kernel_spmd(nc, in_maps, list(range(NCORES)))

    out = np.zeros((B, S, D), np.float32)
    for c in range(NCORES):
        out[c // HEADS_PER_CORE] += res.results[c]["out"].astype(np.float32)
    return out


# revision 37
# speedup vs baseline: 1.1958x; 1.0120x over previous
"""Causal multi-head attention with RoPE for Trainium2, 8-core SPMD.

Problem: B=2, S=2048, D_MODEL=1024, H=16, HD=64, causal softmax(QK^T/8)V
with interleaved-pair RoPE on q/k, projections Wq/Wk/Wv/Wo.

Sharding (host side): batch x head-group. Core c handles batch b=c//4 and
head group g=c%4 (heads 4g..4g+3, a 256-wide slice of the projection dims).
Each core computes a full [S, D_MODEL] partial of the output (its head
group's contribution through Wo); host sums 4 partials per batch.

Device schedule (all matmuls bf16, fp32 accumulate):
 - host passes x[b].T so the d-contraction sits on SBUF partitions
 - Wq/Wk rows are permuted per head to [evens | odds] so RoPE pairs are
   (col j, col j+32) within each head: the DVE ops run on contiguous
   32-wide groups (packed, 2x mode) and cos/sin tables are the raw
   [S, 32] tables read through stride-0 broadcast views
 - phase A: Q,K projected in [s, o] layout -> ACT cast-copy to bf16 ->
   RoPE on DVE -> one wide DMA transpose per s-tile into qkt4
 - phase B: per q-chunk, scores^T[k, q] = Kt.T @ Qt per 128-key block
   (K=64 contraction) into 1024-wide PSUM units; one Exp per unit (ACT),
   causal mask on the diagonal blocks (DVE); PV with lhsT = [V | 1]
   (M=65) so row 64 accumulates the softmax denominator for free;
   normalization reads PSUM directly (reciprocal + gpsimd broadcast +
   DVE mul)
 - the V projection runs inside phase B as PE filler at q-chunk
   boundaries (vsb s-tiles are only consumed by the diagonal units,
   several pipeline steps later), so PE never idles while ACT streams
   exps; o_proj of the previous chunk slots in the same place
 - PV accumulators and o_proj PSUM share banks (tags big0/big1); the
   tail o_proj accumulates in the freed scores banks with the
   normalize chain split per m-tile to shorten the critical path
"""

import numpy as np
import ml_dtypes

B, S, D, H = 2, 2048, 1024, 16
HD = 64
NCORES = 8
HEADS_PER_CORE = 4
GDIM = HEADS_PER_CORE * HD          # 256 projection cols per core
SB = S // 128                        # 16 s-tiles
KD = D // 128                        # 8 k-tiles over d
QCHUNK = 512
NQC = S // QCHUNK                    # 4 q-chunks
UNIT = 1024                          # scores psum unit (2 banks)

_BF16 = ml_dtypes.bfloat16
_cache = {}


def _build(use_rope: bool):
    import concourse.bass as bass
    import concourse.mybir as mybir
    import concourse.tile as tile
    from concourse import bacc

    F32 = mybir.dt.float32
    BF16 = mybir.dt.bfloat16
    EXP = mybir.ActivationFunctionType.Exp

    nc = bacc.Bacc(None, target_bir_lowering=False)

    xt_d = nc.dram_tensor("xt", [D, S], BF16, kind="ExternalInput")
    wqk_d = nc.dram_tensor("wqk", [D, 2 * GDIM], BF16, kind="ExternalInput")
    wv_d = nc.dram_tensor("wv", [D, GDIM], BF16, kind="ExternalInput")
    wo_d = nc.dram_tensor("wo", [GDIM, D], BF16, kind="ExternalInput")
    cos_d = nc.dram_tensor("cosr", [S, 32], BF16, kind="ExternalInput")
    sin_d = nc.dram_tensor("sinr", [S, 32], BF16, kind="ExternalInput")
    mask_d = nc.dram_tensor("maskT", [128, 128], BF16, kind="ExternalInput")
    # bf16 output halves the output-DMA bytes; the host accumulates the
    # 4 per-batch partials in fp32
    out_d = nc.dram_tensor("out", [S, D], BF16, kind="ExternalOutput")

    with tile.TileContext(nc) as tc:
        with tc.tile_pool(name="big", bufs=1) as big, \
             tc.tile_pool(name="work", bufs=4) as work, \
             tc.tile_pool(name="ropet", bufs=3) as ropet, \
             tc.tile_pool(name="pex", bufs=4) as pex:
            # ---- resident tensors ----
            xt = big.tile([128, KD, S], BF16)
            wqk = big.tile([128, KD, 2 * GDIM], BF16)
            wv = big.tile([128, KD, GDIM], BF16)
            wo = big.tile([128, 2, D], BF16)
            maskT = big.tile([128, 128], BF16)
            cosr = big.tile([128, SB, 32], BF16)
            sinr = big.tile([128, SB, 32], BF16)
            qkt4 = big.tile([128, 4, S], BF16)
            # qkt4 blocks: 0: Qt heads 0-1, 1: Qt heads 2-3, 2: Kt 0-1, 3: Kt 2-3
            vsb = big.tile([128, SB, HEADS_PER_CORE * 65], BF16)
            yt2 = [big.tile([128, S], BF16, tag=f"yt2{i}", name=f"yt2{i}")
                   for i in range(2)]

            # ones-rows of vsb (column 64 of each 65-wide head slot)
            ones_view = vsb[:].rearrange("p m (h c) -> p m h c", h=4)[:, :, :, 64:65]
            nc.vector.memset(ones_view, 1.0)

            # ---- chunked input loads, ordered so x streams ahead of use ----
            xt_r = xt_d.rearrange("(k p) s -> p k s", p=128)
            wqk_r = wqk_d.rearrange("(k p) o -> p k o", p=128)
            nc.sync.dma_start(xt[:, 0:4, 0:128], xt_r[:, 0:4, 0:128])
            nc.sync.dma_start(wqk[:, 0:2, :], wqk_r[:, 0:2, :])
            nc.sync.dma_start(wqk[:, 2:4, :], wqk_r[:, 2:4, :])
            nc.sync.dma_start(xt[:, 4:8, 0:128], xt_r[:, 4:8, 0:128])
            nc.sync.dma_start(wqk[:, 4:6, :], wqk_r[:, 4:6, :])
            nc.sync.dma_start(wqk[:, 6:8, :], wqk_r[:, 6:8, :])
            nc.sync.dma_start(xt[:, :, 128:384], xt_r[:, :, 128:384])
            nc.sync.dma_start(xt[:, :, 384:640], xt_r[:, :, 384:640])
            nc.sync.dma_start(cosr[:], cos_d.rearrange("(m p) f -> p m f", p=128))
            nc.sync.dma_start(sinr[:], sin_d.rearrange("(m p) f -> p m f", p=128))
            nc.sync.dma_start(wv[:], wv_d.rearrange("(k p) o -> p k o", p=128))
            nc.sync.dma_start(maskT[:], mask_d[:])
            nc.sync.dma_start(xt[:, :, 640:1536], xt_r[:, :, 640:1536])
            nc.sync.dma_start(xt[:, :, 1536:2048], xt_r[:, :, 1536:2048])
            nc.sync.dma_start(wo[:], wo_d.rearrange("(k p) o -> p k o", p=128))

            # ---- phase A: QK projections + rope + transpose ----
            act_warm = False
            with tc.tile_pool(name="pa", bufs=2, space="PSUM") as pa:
                for m in range(SB):
                    ms = slice(m * 128, (m + 1) * 128)
                    ps = pa.tile([128, 2 * GDIM], F32, tag="qk")
                    for k in range(KD):
                        nc.tensor.matmul(ps[:], xt[:, k, ms], wqk[:, k, :],
                                         start=(k == 0), stop=(k == KD - 1))
                    qkr = ropet.tile([128, 2 * GDIM], BF16, tag="qkr")
                    if use_rope:
                        qkf = ropet.tile([128, 2 * GDIM], BF16, tag="qkf")
                        nc.scalar.copy(qkf[:], ps[:])
                        qv = qkf[:].rearrange("p (g e c) -> p g e c", e=2, c=32)
                        ov = qkr[:].rearrange("p (g e c) -> p g e c", e=2, c=32)
                        E, O = qv[:, :, 0, :], qv[:, :, 1, :]
                        C = cosr[:, m, :].unsqueeze(1).broadcast_to((128, 8, 32))
                        Sn = sinr[:, m, :].unsqueeze(1).broadcast_to((128, 8, 32))
                        g32 = lambda t: t[:].rearrange("p (g c) -> p g c", c=32)
                        ta = ropet.tile([128, 256], BF16, tag="ta")
                        tb = ropet.tile([128, 256], BF16, tag="tb")
                        nc.vector.tensor_mul(g32(ta), E, C)
                        nc.vector.tensor_mul(g32(tb), O, Sn)
                        nc.vector.tensor_sub(ov[:, :, 0, :], g32(ta), g32(tb))
                        tc_ = ropet.tile([128, 256], BF16, tag="tc")
                        td = ropet.tile([128, 256], BF16, tag="td")
                        nc.vector.tensor_mul(g32(tc_), O, C)
                        nc.vector.tensor_mul(g32(td), E, Sn)
                        nc.vector.tensor_add(ov[:, :, 1, :], g32(tc_), g32(td))
                    else:
                        nc.scalar.copy(qkr[:], ps[:])
                    nc.sync.dma_start_transpose(qkt4[:, :, ms], qkr[:])
                    if not act_warm:
                        # preload the Exp table while ACT is idle so the
                        # first phase-B exp doesn't pay the 1.3us load
                        wa = work.tile([1, 1], BF16, tag="wa")
                        nc.scalar.activation(wa[:], ta[0:1, 0:1] if use_rope
                                             else qkr[0:1, 0:1], EXP, scale=0.125)
                        act_warm = True

            # ---- phase B: attention + V projection + interleaved o_proj ----
            with tc.tile_pool(name="sc", bufs=1, space="PSUM") as scp, \
                 tc.tile_pool(name="yb", bufs=1, space="PSUM") as ybp, \
                 tc.tile_pool(name="pv", bufs=2, space="PSUM") as pvp:

                def emit_vproj_tile(m):
                    ms = slice(m * 128, (m + 1) * 128)
                    psv = pvp.tile([128, QCHUNK], F32, tag="v")
                    for k in range(KD):
                        nc.tensor.matmul(psv[:, 0:GDIM], xt[:, k, ms], wv[:, k, :],
                                         start=(k == 0), stop=(k == KD - 1))
                    dst = vsb[:, m, :].rearrange("p (h c) -> p h c", h=4)[:, :, 0:64]
                    nc.scalar.copy(dst, psv[:, 0:GDIM].rearrange("p (h c) -> p h c", h=4))

                def emit_oproj_tile(m, use_pv=False):
                    ms = slice(m * 128, (m + 1) * 128)
                    so = work.tile([128, 1024], BF16, tag="so")
                    for nb in range(2):
                        if use_pv:
                            # pv-pool accumulator: avoids waiting on the
                            # ytps (big-tag) release behind the normalize
                            po = pvp.tile([128, QCHUNK], F32, tag="v")
                        else:
                            po = ybp.tile([128, QCHUNK], F32, tag=f"big{nb}")
                        for k2 in range(2):
                            nc.tensor.matmul(po[:], yt2[k2][:, ms],
                                             wo[:, k2, nb * 512:(nb + 1) * 512],
                                             start=(k2 == 0), stop=(k2 == 1))
                        # copies alternate DVE/ACT to halve the serial chain
                        if nb == 0:
                            nc.vector.tensor_copy(so[:, 0:512], po[:])
                        else:
                            nc.scalar.copy(so[:, 512:1024], po[:])
                    nc.sync.dma_start(out_d[ms, :], so[:])

                # qc0's diagonal PV needs vsb s-tiles 0-3 immediately;
                # project them first (also keeps PE warm across the
                # phase boundary)
                for mi in range(4):
                    emit_vproj_tile(mi)

                for qc in range(NQC):
                    q0 = qc * QCHUNK
                    # block list: full key blocks then the diagonal in order
                    # 512,384,128,256 so no matmul crosses a 512-col bank
                    order = list(range(4 * qc)) + \
                        [4 * qc, 4 * qc + 1, 4 * qc + 3, 4 * qc + 2]
                    blocks = []
                    for kb in order:
                        r = max(0, kb - 4 * qc)
                        blocks.append((kb, q0 + r * 128, QCHUNK - r * 128))
                    units, cur, cols = [], [], 0
                    for kb, qoff, n in blocks:
                        if cols + n > UNIT:
                            units.append(cur)
                            cur, cols = [], 0
                        cur.append((kb, qoff, n, cols))
                        cols += n
                    units.append(cur)
                    last_kb = units[-1][-1][0]

                    for hp in range(2):
                        ytps = None
                        for ui, unit in enumerate(units):
                            ucols = unit[-1][3] + unit[-1][2]
                            scs = [scp.tile([128, UNIT], F32, tag=f"sc{i}",
                                            name=f"sc{i}") for i in range(2)]
                            for i in range(2):
                                rows = slice(i * 64, i * 64 + 64)
                                for (kb, qoff, n, o) in unit:
                                    nc.tensor.matmul(
                                        scs[i][:, o:o + n],
                                        qkt4[rows, 2 + hp, kb * 128:(kb + 1) * 128],
                                        qkt4[rows, hp, qoff:qoff + n],
                                        start=True, stop=True)
                            # PE filler between scores and PV: previous
                            # chunk's o_proj (must precede this chunk's ytps
                            # allocation — the po tiles reuse the big banks)
                            # and this chunk's V projection, spread across
                            # unit boundaries so ACT's exp stream never
                            # starves. The vsb tiles feed only the diagonal
                            # units, several pipeline steps later.
                            if ui == 0 and qc > 0:
                                # 2 o_proj tiles per hp boundary (hp0's po
                                # allocations must precede its ytps; hp1's
                                # ride the pv pool). The last chunk defers
                                # hp1's pair to the tail as stagger filler.
                                if hp == 0:
                                    for mi in range(2):
                                        emit_oproj_tile((qc - 1) * 4 + mi)
                                elif qc < NQC - 1:
                                    for mi in range(2):
                                        emit_oproj_tile((qc - 1) * 4 + 2 + mi,
                                                        use_pv=True)
                            if hp == 0 and qc > 0:
                                nu = len(units)
                                if nu >= 5 and ui in (1, 2, 3, 4):
                                    emit_vproj_tile(qc * 4 + ui - 1)
                                elif nu < 5 and ui in (1, 2):
                                    for mi in range(2):
                                        emit_vproj_tile(
                                            qc * 4 + (ui - 1) * 2 + mi)
                            if ytps is None:
                                ytps = [ybp.tile([128, QCHUNK], F32,
                                                 tag=f"big{i}", name=f"big{i}")
                                        for i in range(2)]
                            for i in range(2):
                                h = 2 * hp + i
                                vcol = slice(h * 65, h * 65 + 65)
                                pe = pex.tile([128, UNIT], BF16,
                                              tag=f"pe{i}", name=f"pe{i}")
                                nc.scalar.activation(pe[:, :ucols],
                                                     scs[i][:, :ucols],
                                                     EXP, scale=0.125)
                                for (kb, qoff, n, o) in unit:
                                    if kb >= 4 * qc:  # diagonal: causal mask
                                        nc.vector.tensor_mul(
                                            pe[:, o:o + 128], pe[:, o:o + 128],
                                            maskT[:])
                                    # kb==0 always has n=512: start clears
                                    # the whole [65, QCHUNK] accumulator
                                    nc.tensor.matmul(
                                        ytps[i][0:65, qoff - q0:qoff - q0 + n],
                                        vsb[:, kb, vcol],
                                        pe[:, o:o + n],
                                        start=(kb == 0), stop=(kb == last_kb))
                        if qc == NQC - 1 and hp == 1:
                            # tail: bf16 normalize (denominator precision
                            # cost ~0.4%, well inside the error budget).
                            # ACT casts y to bf16 while DVE does the
                            # reciprocals; the per-m-tile muls then run in
                            # 2x mode and stagger the o_proj k2=1 pairs.
                            # Reading ytps via ybf also releases the big
                            # banks for the tail slots sooner.
                            rcs, ybfs = [], []
                            for i in range(2):
                                rc = work.tile([1, QCHUNK], BF16, tag=f"rcb{i}")
                                with nc.allow_low_precision(
                                        reason="bf16 softmax denominator"):
                                    nc.vector.reciprocal(rc[:],
                                                         ytps[i][64:65, :])
                                rcs.append(rc)
                                ybf = work.tile([64, QCHUNK], BF16,
                                                tag=f"ybf{i}")
                                nc.scalar.copy(ybf[:], ytps[i][0:64, :])
                                ybfs.append(ybf)
                            for mi in range(4):
                                mc = slice(mi * 128, (mi + 1) * 128)
                                for i in range(2):
                                    bc = work.tile([64, 128], BF16, tag="bct")
                                    nc.gpsimd.partition_broadcast(
                                        bc[:], rcs[i][0:1, mc])
                                    nc.vector.tensor_mul(
                                        yt2[hp][i * 64:i * 64 + 64,
                                                q0 + mi * 128:
                                                q0 + (mi + 1) * 128],
                                        ybfs[i][:, mc], bc[:])
                        else:
                            for i in range(2):
                                # normalize straight from PSUM
                                rc = work.tile([1, QCHUNK], F32, tag="rc")
                                nc.vector.reciprocal(rc[:], ytps[i][64:65, :])
                                bc = work.tile([64, QCHUNK], F32, tag="bc")
                                nc.gpsimd.partition_broadcast(bc[:], rc[0:1, :])
                                nc.vector.tensor_mul(
                                    yt2[hp][i * 64:i * 64 + 64, q0:q0 + QCHUNK],
                                    ytps[i][0:64, :], bc[:])
                    if qc == NQC - 1:
                        # deferred previous-chunk pair: free-running PE
                        # filler that overlaps the tail normalize chains
                        for mi in range(2):
                            emit_oproj_tile((qc - 1) * 4 + 2 + mi, use_pv=True)
                        # tail o_proj with 8 independent PSUM accumulators
                        # (freed scores banks + pv pool + big banks) so no
                        # matmul waits on a copy; all k2=0 matmuls run
                        # during the normalize chains, k2=1 staggers in as
                        # the per-m muls land; copies alternate DVE/ACT
                        slots = []
                        for mi in range(2):
                            pot = scp.tile([128, UNIT], F32,
                                           tag=f"sc{mi}", name=f"sc{mi}")
                            slots.append((pot[:, 0:512], pot[:, 512:1024]))
                        for mi in range(2):
                            a = pvp.tile([128, QCHUNK], F32, tag="v")
                            b = ybp.tile([128, QCHUNK], F32,
                                         tag=f"big{mi}", name=f"big{mi}")
                            slots.append((a[:], b[:]))
                        for k2 in range(2):
                            for mi in range(4):
                                m = qc * 4 + mi
                                ms = slice(m * 128, (m + 1) * 128)
                                for nb in range(2):
                                    nc.tensor.matmul(
                                        slots[mi][nb], yt2[k2][:, ms],
                                        wo[:, k2, nb * 512:(nb + 1) * 512],
                                        start=(k2 == 0), stop=(k2 == 1))
                        for mi in range(4):
                            m = qc * 4 + mi
                            ms = slice(m * 128, (m + 1) * 128)
                            so = work.tile([128, 1024], BF16, tag=f"sot{mi}")
                            for nb in range(2):
                                if (mi + nb) % 2 == 0:
                                    nc.vector.tensor_copy(
                                        so[:, nb * 512:(nb + 1) * 512],
                                        slots[mi][nb])
                                else:
                                    nc.scalar.copy(
                                        so[:, nb * 512:(nb + 1) * 512],
                                        slots[mi][nb])
                            nc.sync.dma_start(out_d[ms, :], so[:])
    nc.compile()
    return nc


def _prep_core_inputs(x, Wq, Wk, Wv, Wo, cos_g, sin_g, use_rope):
    """Host-side shard + layout prep. Returns list of 8 input dicts."""
    maskT = np.tril(np.ones((128, 128), np.float32)).T.astype(_BF16)
    # per-head row permutation: [evens | odds] so rope pairs are
    # (j, j+32) within each head's 64 projection dims
    perm = np.concatenate([np.arange(h * 64, (h + 1) * 64).reshape(32, 2).T.reshape(64)
                           for h in range(H)])
    Wq_p = Wq[perm]
    Wk_p = Wk[perm]
    maps = []
    for c in range(NCORES):
        b, g = divmod(c, HEADS_PER_CORE)
        rows = slice(g * GDIM, (g + 1) * GDIM)
        wqk = np.concatenate([Wq_p[rows], Wk_p[rows]], axis=0).T  # [D, 512]
        maps.append({
            "xt": np.ascontiguousarray(x[b].T).astype(_BF16),
            "wqk": np.ascontiguousarray(wqk).astype(_BF16),
            "wv": np.ascontiguousarray(Wv[rows].T).astype(_BF16),
            "wo": np.ascontiguousarray(Wo[:, rows].T).astype(_BF16),
            "cosr": cos_g.astype(_BF16),
            "sinr": sin_g.astype(_BF16),
            "maskT": maskT,
        })
    return maps


def kernel(x, token_positions, use_rope, Wq, Wk, Wv, Wo, cos, sin):
    from concourse.bass_utils import run_bass_kernel_spmd

    x = np.asarray(x, np.float32)
    token_positions = np.asarray(token_positions)
    Wq = np.asarray(Wq, np.float32)
    Wk = np.asarray(Wk, np.float32)
    Wv = np.asarray(Wv, np.float32)
    Wo = np.asarray(Wo, np.float32)
    cos = np.asarray(cos, np.float32)
    sin = np.asarray(sin, np.float32)
    rope = bool(int(use_rope))

    cos_g = cos[token_positions]  # [S, 32]
    sin_g = sin[token_positions]

    if rope not in _cache:
        _cache[rope] = _build(rope)
    nc = _cache[rope]

    in_maps = _prep_core_inputs(x, Wq, Wk, Wv, Wo, cos_g, sin_g, rope)
    res = run_bass_kernel_spmd(nc, in_maps, list(range(NCORES)))

    out = np.zeros((B, S, D), np.float32)
    for c in range(NCORES):
        out[c // HEADS_PER_CORE] += res.results[c]["out"].astype(np.float32)
    return out


# revision 44
# speedup vs baseline: 1.2002x; 1.0037x over previous
"""Causal multi-head attention with RoPE for Trainium2, 8-core SPMD.

Problem: B=2, S=2048, D_MODEL=1024, H=16, HD=64, causal softmax(QK^T/8)V
with interleaved-pair RoPE on q/k, projections Wq/Wk/Wv/Wo.

Sharding (host side): batch x head-group. Core c handles batch b=c//4 and
head group g=c%4 (heads 4g..4g+3, a 256-wide slice of the projection dims).
Each core computes a full [S, D_MODEL] partial of the output (its head
group's contribution through Wo); host sums 4 partials per batch.

Device schedule (all matmuls bf16, fp32 accumulate):
 - host passes x[b].T so the d-contraction sits on SBUF partitions
 - Wq/Wk rows are permuted per head to [evens | odds] so RoPE pairs are
   (col j, col j+32) within each head: the DVE ops run on contiguous
   32-wide groups (packed, 2x mode) and cos/sin tables are the raw
   [S, 32] tables read through stride-0 broadcast views
 - phase A: Q,K projected in [s, o] layout -> ACT cast-copy to bf16 ->
   RoPE on DVE -> one wide DMA transpose per s-tile into qkt4
 - phase B: per q-chunk, scores^T[k, q] = Kt.T @ Qt per 128-key block
   (K=64 contraction) into 1024-wide PSUM units; one Exp per unit (ACT),
   causal mask on the diagonal blocks (DVE); PV with lhsT = [V | 1]
   (M=65) so row 64 accumulates the softmax denominator for free;
   normalization reads PSUM directly (reciprocal + gpsimd broadcast +
   DVE mul)
 - the V projection runs inside phase B as PE filler at q-chunk
   boundaries (vsb s-tiles are only consumed by the diagonal units,
   several pipeline steps later), so PE never idles while ACT streams
   exps; o_proj of the previous chunk slots in the same place
 - PV accumulators and o_proj PSUM share banks (tags big0/big1); the
   tail o_proj accumulates in the freed scores banks with the
   normalize chain split per m-tile to shorten the critical path
"""

import numpy as np
import ml_dtypes

B, S, D, H = 2, 2048, 1024, 16
HD = 64
NCORES = 8
HEADS_PER_CORE = 4
GDIM = HEADS_PER_CORE * HD          # 256 projection cols per core
SB = S // 128                        # 16 s-tiles
KD = D // 128                        # 8 k-tiles over d
QCHUNK = 512
NQC = S // QCHUNK                    # 4 q-chunks
UNIT = 1024                          # scores psum unit (2 banks)

_BF16 = ml_dtypes.bfloat16
_cache = {}


def _build(use_rope: bool):
    import concourse.bass as bass
    import concourse.mybir as mybir
    import concourse.tile as tile
    from concourse import bacc

    F32 = mybir.dt.float32
    BF16 = mybir.dt.bfloat16
    EXP = mybir.ActivationFunctionType.Exp

    nc = bacc.Bacc(None, target_bir_lowering=False)

    xt_d = nc.dram_tensor("xt", [D, S], BF16, kind="ExternalInput")
    wqk_d = nc.dram_tensor("wqk", [D, 2 * GDIM], BF16, kind="ExternalInput")
    wv_d = nc.dram_tensor("wv", [D, GDIM], BF16, kind="ExternalInput")
    wo_d = nc.dram_tensor("wo", [GDIM, D], BF16, kind="ExternalInput")
    cs_d = nc.dram_tensor("csr", [S, 64], BF16, kind="ExternalInput")
    mask_d = nc.dram_tensor("maskT", [128, 128], BF16, kind="ExternalInput")
    # bf16 output halves the output-DMA bytes; the host accumulates the
    # 4 per-batch partials in fp32
    out_d = nc.dram_tensor("out", [S, D], BF16, kind="ExternalOutput")

    with tile.TileContext(nc) as tc:
        with tc.tile_pool(name="big", bufs=1) as big, \
             tc.tile_pool(name="work", bufs=4) as work, \
             tc.tile_pool(name="ropet", bufs=3) as ropet, \
             tc.tile_pool(name="pex", bufs=4) as pex:
            # ---- resident tensors ----
            xt = big.tile([128, KD, S], BF16)
            wqk = big.tile([128, KD, 2 * GDIM], BF16)
            wv = big.tile([128, KD, GDIM], BF16)
            wo = big.tile([128, 2, D], BF16)
            maskT = big.tile([128, 128], BF16)
            csr = big.tile([128, SB, 64], BF16)
            qkt4 = big.tile([128, 4, S], BF16)
            # qkt4 blocks: 0: Qt heads 0-1, 1: Qt heads 2-3, 2: Kt 0-1, 3: Kt 2-3
            vsb = big.tile([128, SB, HEADS_PER_CORE * 65], BF16)
            yt2 = [big.tile([128, S], BF16, tag=f"yt2{i}", name=f"yt2{i}")
                   for i in range(2)]

            # ones-rows of vsb (column 64 of each 65-wide head slot)
            ones_view = vsb[:].rearrange("p m (h c) -> p m h c", h=4)[:, :, :, 64:65]
            nc.vector.memset(ones_view, 1.0)

            # ---- chunked input loads, ordered so x streams ahead of use ----
            xt_r = xt_d.rearrange("(k p) s -> p k s", p=128)
            wqk_r = wqk_d.rearrange("(k p) o -> p k o", p=128)
            nc.sync.dma_start(xt[:, 0:4, 0:128], xt_r[:, 0:4, 0:128])
            nc.sync.dma_start(wqk[:, 0:2, :], wqk_r[:, 0:2, :])
            nc.sync.dma_start(csr[:], cs_d.rearrange("(m p) f -> p m f", p=128))
            nc.sync.dma_start(wqk[:, 2:4, :], wqk_r[:, 2:4, :])
            nc.sync.dma_start(xt[:, 4:8, 0:128], xt_r[:, 4:8, 0:128])
            nc.sync.dma_start(wqk[:, 4:6, :], wqk_r[:, 4:6, :])
            nc.sync.dma_start(wqk[:, 6:8, :], wqk_r[:, 6:8, :])
            nc.sync.dma_start(xt[:, :, 128:384], xt_r[:, :, 128:384])
            nc.sync.dma_start(xt[:, :, 384:640], xt_r[:, :, 384:640])
            nc.sync.dma_start(wv[:], wv_d.rearrange("(k p) o -> p k o", p=128))
            nc.sync.dma_start(maskT[:], mask_d[:])
            nc.sync.dma_start(xt[:, :, 640:1536], xt_r[:, :, 640:1536])
            nc.sync.dma_start(xt[:, :, 1536:2048], xt_r[:, :, 1536:2048])
            nc.sync.dma_start(wo[:], wo_d.rearrange("(k p) o -> p k o", p=128))

            # shared QK-tile body: projection psum -> bf16 -> rope -> one
            # wide transpose into qkt4. copy_dve picks the cast engine so
            # phase-B filler tiles stay off the exp-loaded ACT queue.
            def emit_rope_transpose(ps, m, copy_dve):
                ms = slice(m * 128, (m + 1) * 128)
                qkr = ropet.tile([128, 2 * GDIM], BF16, tag="qkr")
                if use_rope:
                    qkf = ropet.tile([128, 2 * GDIM], BF16, tag="qkf")
                    if copy_dve:
                        nc.vector.tensor_copy(qkf[:], ps)
                    else:
                        nc.scalar.copy(qkf[:], ps)
                    qv = qkf[:].rearrange("p (g e c) -> p g e c", e=2, c=32)
                    ov = qkr[:].rearrange("p (g e c) -> p g e c", e=2, c=32)
                    E, O = qv[:, :, 0, :], qv[:, :, 1, :]
                    C = csr[:, m, 0:32].unsqueeze(1).broadcast_to((128, 8, 32))
                    Sn = csr[:, m, 32:64].unsqueeze(1).broadcast_to((128, 8, 32))
                    g32 = lambda t: t[:].rearrange("p (g c) -> p g c", c=32)
                    ta = ropet.tile([128, 256], BF16, tag="ta")
                    tb = ropet.tile([128, 256], BF16, tag="tb")
                    nc.vector.tensor_mul(g32(ta), E, C)
                    nc.vector.tensor_mul(g32(tb), O, Sn)
                    nc.vector.tensor_sub(ov[:, :, 0, :], g32(ta), g32(tb))
                    tc_ = ropet.tile([128, 256], BF16, tag="tc")
                    td = ropet.tile([128, 256], BF16, tag="td")
                    nc.vector.tensor_mul(g32(tc_), O, C)
                    nc.vector.tensor_mul(g32(td), E, Sn)
                    nc.vector.tensor_add(ov[:, :, 1, :], g32(tc_), g32(td))
                else:
                    if copy_dve:
                        nc.vector.tensor_copy(qkr[:], ps)
                    else:
                        nc.scalar.copy(qkr[:], ps)
                return nc.sync.dma_start_transpose(qkt4[:, :, ms], qkr[:])

            # ---- phase A: QK projections for the first 8 s-tiles (the
            # rest run inside phase B as PE filler) ----
            act_warm = False
            with tc.tile_pool(name="pa", bufs=2, space="PSUM") as pa:
                for m in range(SB // 2):
                    ms = slice(m * 128, (m + 1) * 128)
                    ps = pa.tile([128, 2 * GDIM], F32, tag="qk")
                    for k in range(KD):
                        nc.tensor.matmul(ps[:], xt[:, k, ms], wqk[:, k, :],
                                         start=(k == 0), stop=(k == KD - 1))
                    emit_rope_transpose(ps[:], m, copy_dve=False)
                    if not act_warm:
                        # preload the Exp table while ACT is idle so the
                        # first phase-B exp doesn't pay the 1.3us load
                        wa = work.tile([1, 1], BF16, tag="wa")
                        nc.scalar.activation(wa[:], wqk[0:1, 0, 0:1],
                                             EXP, scale=0.125)
                        act_warm = True

            # ---- phase B: attention + V projection + interleaved o_proj ----
            with tc.tile_pool(name="sc", bufs=1, space="PSUM") as scp, \
                 tc.tile_pool(name="yb", bufs=1, space="PSUM") as ybp, \
                 tc.tile_pool(name="pv", bufs=2, space="PSUM") as pvp:

                def emit_vproj_tile(m):
                    ms = slice(m * 128, (m + 1) * 128)
                    psv = pvp.tile([128, QCHUNK], F32, tag="v")
                    for k in range(KD):
                        nc.tensor.matmul(psv[:, 0:GDIM], xt[:, k, ms], wv[:, k, :],
                                         start=(k == 0), stop=(k == KD - 1))
                    dst = vsb[:, m, :].rearrange("p (h c) -> p h c", h=4)[:, :, 0:64]
                    nc.scalar.copy(dst, psv[:, 0:GDIM].rearrange("p (h c) -> p h c", h=4))

                def emit_qkproj_tile(m):
                    ms = slice(m * 128, (m + 1) * 128)
                    ps = pvp.tile([128, QCHUNK], F32, tag="v")
                    for k in range(KD):
                        nc.tensor.matmul(ps[:], xt[:, k, ms], wqk[:, k, :],
                                         start=(k == 0), stop=(k == KD - 1))
                    emit_rope_transpose(ps[:], m, copy_dve=True)

                def emit_oproj_tile(m, use_pv=False):
                    ms = slice(m * 128, (m + 1) * 128)
                    so = work.tile([128, 1024], BF16, tag="so")
                    for nb in range(2):
                        if use_pv:
                            # pv-pool accumulator: avoids waiting on the
                            # ytps (big-tag) release behind the normalize
                            po = pvp.tile([128, QCHUNK], F32, tag="v")
                        else:
                            po = ybp.tile([128, QCHUNK], F32, tag=f"big{nb}")
                        for k2 in range(2):
                            nc.tensor.matmul(po[:], yt2[k2][:, ms],
                                             wo[:, k2, nb * 512:(nb + 1) * 512],
                                             start=(k2 == 0), stop=(k2 == 1))
                        # copies alternate DVE/ACT to halve the serial chain
                        if nb == 0:
                            nc.vector.tensor_copy(so[:, 0:512], po[:])
                        else:
                            nc.scalar.copy(so[:, 512:1024], po[:])
                    nc.sync.dma_start(out_d[ms, :], so[:])

                # qc0's diagonal PV needs vsb s-tiles 0-3 immediately;
                # project them first (also keeps PE warm across the
                # phase boundary)
                for mi in range(4):
                    emit_vproj_tile(mi)

                # filler schedule: one PE tile (o_proj of an earlier chunk,
                # V projection, or a deferred QK projection) per unit
                # boundary so ACT's exp stream and PE advance in lockstep.
                # Deadlines: vsb[m] before its chunk's diagonal units, and
                # qkt4[m] transposed before the chunk whose q-range needs it.
                FILLER = {
                    (0, 0, 0): [("v", 4)], (0, 0, 1): [("v", 5)],
                    (0, 1, 0): [("v", 6)], (0, 1, 1): [("v", 7)],
                    (1, 0, 0): [("op", 0), ("op", 1)],
                    (1, 0, 1): [("qk", 8)], (1, 0, 2): [("qk", 9)],
                    (1, 0, 3): [("qk", 10)],
                    (1, 1, 0): [("oppv", 2), ("oppv", 3)],
                    (1, 1, 1): [("qk", 11)], (1, 1, 2): [("v", 8)],
                    (1, 1, 3): [("v", 9)],
                    (2, 0, 0): [("op", 4), ("op", 5)],
                    (2, 0, 1): [("v", 10)], (2, 0, 2): [("v", 11)],
                    (2, 0, 3): [("qk", 12)], (2, 0, 4): [("qk", 13)],
                    (2, 1, 0): [("oppv", 6), ("oppv", 7)],
                    (2, 1, 1): [("qk", 14)], (2, 1, 2): [("qk", 15)],
                    (3, 0, 0): [("op", 8), ("op", 9)],
                    (3, 1, 0): [("oppv", 10), ("oppv", 11)],
                    (3, 0, 1): [("v", 12)], (3, 0, 2): [("v", 13)],
                    (3, 0, 3): [("v", 14)], (3, 0, 4): [("v", 15)],
                }

                for qc in range(NQC):
                    q0 = qc * QCHUNK
                    # block list: full key blocks then the diagonal in order
                    # 512,384,128,256 so no matmul crosses a 512-col bank
                    order = list(range(4 * qc)) + \
                        [4 * qc, 4 * qc + 1, 4 * qc + 3, 4 * qc + 2]
                    blocks = []
                    for kb in order:
                        r = max(0, kb - 4 * qc)
                        blocks.append((kb, q0 + r * 128, QCHUNK - r * 128))
                    units, cur, cols = [], [], 0
                    for kb, qoff, n in blocks:
                        if cols + n > UNIT:
                            units.append(cur)
                            cur, cols = [], 0
                        cur.append((kb, qoff, n, cols))
                        cols += n
                    units.append(cur)
                    last_kb = units[-1][-1][0]

                    for hp in range(2):
                        ytps = None
                        for ui, unit in enumerate(units):
                            ucols = unit[-1][3] + unit[-1][2]
                            scs = [scp.tile([128, UNIT], F32, tag=f"sc{i}",
                                            name=f"sc{i}") for i in range(2)]
                            for i in range(2):
                                rows = slice(i * 64, i * 64 + 64)
                                for (kb, qoff, n, o) in unit:
                                    nc.tensor.matmul(
                                        scs[i][:, o:o + n],
                                        qkt4[rows, 2 + hp, kb * 128:(kb + 1) * 128],
                                        qkt4[rows, hp, qoff:qoff + n],
                                        start=True, stop=True)
                            # PE filler between scores and PV: previous
                            # chunk's o_proj (must precede this chunk's ytps
                            # allocation — the po tiles reuse the big banks)
                            # and this chunk's V projection, spread across
                            # unit boundaries so ACT's exp stream never
                            # starves. The vsb tiles feed only the diagonal
                            # units, several pipeline steps later.
                            for kind, fm in FILLER.get((qc, hp, ui), ()):
                                if kind == "op":
                                    emit_oproj_tile(fm)
                                elif kind == "oppv":
                                    emit_oproj_tile(fm, use_pv=True)
                                elif kind == "v":
                                    emit_vproj_tile(fm)
                                else:
                                    emit_qkproj_tile(fm)
                            if ytps is None:
                                ytps = [ybp.tile([128, QCHUNK], F32,
                                                 tag=f"big{i}", name=f"big{i}")
                                        for i in range(2)]
                            for i in range(2):
                                h = 2 * hp + i
                                vcol = slice(h * 65, h * 65 + 65)
                                pe = pex.tile([128, UNIT], BF16,
                                              tag=f"pe{i}", name=f"pe{i}")
                                nc.scalar.activation(pe[:, :ucols],
                                                     scs[i][:, :ucols],
                                                     EXP, scale=0.125)
                                for (kb, qoff, n, o) in unit:
                                    if kb >= 4 * qc:  # diagonal: causal mask
                                        nc.vector.tensor_mul(
                                            pe[:, o:o + 128], pe[:, o:o + 128],
                                            maskT[:])
                                    # kb==0 always has n=512: start clears
                                    # the whole [65, QCHUNK] accumulator
                                    nc.tensor.matmul(
                                        ytps[i][0:65, qoff - q0:qoff - q0 + n],
                                        vsb[:, kb, vcol],
                                        pe[:, o:o + n],
                                        start=(kb == 0), stop=(kb == last_kb))
                        if qc == NQC - 1 and hp == 1:
                            # tail: bf16 normalize (denominator precision
                            # cost ~0.4%, well inside the error budget).
                            # ACT casts y to bf16 while DVE does the
                            # reciprocals; the per-m-tile muls then run in
                            # 2x mode and stagger the o_proj k2=1 pairs.
                            # Reading ytps via ybf also releases the big
                            # banks for the tail slots sooner.
                            rcs, ybfs = [], []
                            for i in range(2):
                                rc = work.tile([1, QCHUNK], BF16, tag=f"rcb{i}")
                                with nc.allow_low_precision(
                                        reason="bf16 softmax denominator"):
                                    nc.vector.reciprocal(rc[:],
                                                         ytps[i][64:65, :])
                                rcs.append(rc)
                                ybf = work.tile([64, QCHUNK], BF16,
                                                tag=f"ybf{i}")
                                nc.scalar.copy(ybf[:], ytps[i][0:64, :])
                                ybfs.append(ybf)
                            for mi in range(4):
                                mc = slice(mi * 128, (mi + 1) * 128)
                                for i in range(2):
                                    bc = work.tile([64, 128], BF16, tag="bct")
                                    nc.gpsimd.partition_broadcast(
                                        bc[:], rcs[i][0:1, mc])
                                    nc.vector.tensor_mul(
                                        yt2[hp][i * 64:i * 64 + 64,
                                                q0 + mi * 128:
                                                q0 + (mi + 1) * 128],
                                        ybfs[i][:, mc], bc[:])
                        else:
                            for i in range(2):
                                # normalize straight from PSUM
                                rc = work.tile([1, QCHUNK], F32, tag="rc")
                                nc.vector.reciprocal(rc[:], ytps[i][64:65, :])
                                bc = work.tile([64, QCHUNK], F32, tag="bc")
                                nc.gpsimd.partition_broadcast(bc[:], rc[0:1, :])
                                nc.vector.tensor_mul(
                                    yt2[hp][i * 64:i * 64 + 64, q0:q0 + QCHUNK],
                                    ytps[i][0:64, :], bc[:])
                    if qc == NQC - 1:
                        # tail o_proj with 8 independent PSUM accumulators
                        # (freed scores banks + pv pool + big banks) so no
                        # matmul waits on a copy; all k2=0 matmuls run
                        # during the normalize chains, k2=1 staggers in as
                        # the per-m muls land; copies alternate DVE/ACT
                        slots = []
                        for mi in range(2):
                            pot = scp.tile([128, UNIT], F32,
                                           tag=f"sc{mi}", name=f"sc{mi}")
                            slots.append((pot[:, 0:512], pot[:, 512:1024]))
                        for mi in range(2):
                            a = pvp.tile([128, QCHUNK], F32, tag="v")
                            b = ybp.tile([128, QCHUNK], F32,
                                         tag=f"big{mi}", name=f"big{mi}")
                            slots.append((a[:], b[:]))
                        for k2 in range(2):
                            for mi in range(4):
                                m = qc * 4 + mi
                                ms = slice(m * 128, (m + 1) * 128)
                                for nb in range(2):
                                    nc.tensor.matmul(
                                        slots[mi][nb], yt2[k2][:, ms],
                                        wo[:, k2, nb * 512:(nb + 1) * 512],
                                        start=(k2 == 0), stop=(k2 == 1))
                        for mi in range(4):
                            m = qc * 4 + mi
                            ms = slice(m * 128, (m + 1) * 128)
                            so = work.tile([128, 1024], BF16, tag=f"sot{mi}")
                            for nb in range(2):
                                if (mi + nb) % 2 == 0:
                                    nc.vector.tensor_copy(
                                        so[:, nb * 512:(nb + 1) * 512],
                                        slots[mi][nb])
                                else:
                                    nc.scalar.copy(
                                        so[:, nb * 512:(nb + 1) * 512],
                                        slots[mi][nb])
                            nc.sync.dma_start(out_d[ms, :], so[:])
    nc.compile()
    return nc


def _prep_core_inputs(x, Wq, Wk, Wv, Wo, cos_g, sin_g, use_rope):
    """Host-side shard + layout prep. Returns list of 8 input dicts."""
    maskT = np.tril(np.ones((128, 128), np.float32)).T.astype(_BF16)
    # per-head row permutation: [evens | odds] so rope pairs are
    # (j, j+32) within each head's 64 projection dims
    perm = np.concatenate([np.arange(h * 64, (h + 1) * 64).reshape(32, 2).T.reshape(64)
                           for h in range(H)])
    Wq_p = Wq[perm]
    Wk_p = Wk[perm]
    maps = []
    for c in range(NCORES):
        b, g = divmod(c, HEADS_PER_CORE)
        rows = slice(g * GDIM, (g + 1) * GDIM)
        wqk = np.concatenate([Wq_p[rows], Wk_p[rows]], axis=0).T  # [D, 512]
        maps.append({
            "xt": np.ascontiguousarray(x[b].T).astype(_BF16),
            "wqk": np.ascontiguousarray(wqk).astype(_BF16),
            "wv": np.ascontiguousarray(Wv[rows].T).astype(_BF16),
            "wo": np.ascontiguousarray(Wo[:, rows].T).astype(_BF16),
            "csr": np.concatenate([cos_g, sin_g], axis=1).astype(_BF16),
            "maskT": maskT,
        })
    return maps


def kernel(x, token_positions, use_rope, Wq, Wk, Wv, Wo, cos, sin):
    from concourse.bass_utils import run_bass_kernel_spmd

    x = np.asarray(x, np.float32)
    token_positions = np.asarray(token_positions)
    Wq = np.asarray(Wq, np.float32)
    Wk = np.asarray(Wk, np.float32)
    Wv = np.asarray(Wv, np.float32)
    Wo = np.asarray(Wo, np.float32)
    cos = np.asarray(cos, np.float32)
    sin = np.asarray(sin, np.float32)
    rope = bool(int(use_rope))

    cos_g = cos[token_positions]  # [S, 32]
    sin_g = sin[token_positions]

    if rope not in _cache:
        _cache[rope] = _build(rope)
    nc = _cache[rope]

    in_maps = _prep_core_inputs(x, Wq, Wk, Wv, Wo, cos_g, sin_g, rope)
    res = run_bass_kernel_spmd(nc, in_maps, list(range(NCORES)))

    out = np.zeros((B, S, D), np.float32)
    for c in range(NCORES):
        out[c // HEADS_PER_CORE] += res.results[c]["out"].astype(np.float32)
    return out


# revision 47
# speedup vs baseline: 1.2095x; 1.0078x over previous
"""Causal multi-head attention with RoPE for Trainium2, 8-core SPMD.

Problem: B=2, S=2048, D_MODEL=1024, H=16, HD=64, causal softmax(QK^T/8)V
with interleaved-pair RoPE on q/k, projections Wq/Wk/Wv/Wo.

Sharding (host side): batch x head-group. Core c handles batch b=c//4 and
head group g=c%4 (heads 4g..4g+3, a 256-wide slice of the projection dims).
Each core computes a full [S, D_MODEL] partial of the output (its head
group's contribution through Wo); host sums 4 partials per batch.

Device schedule (all matmuls bf16, fp32 accumulate):
 - host passes x[b].T so the d-contraction sits on SBUF partitions
 - Wq/Wk rows are permuted per head to [evens | odds] so RoPE pairs are
   (col j, col j+32) within each head: the DVE ops run on contiguous
   32-wide groups (packed, 2x mode) and cos/sin tables are the raw
   [S, 32] tables read through stride-0 broadcast views
 - phase A: Q,K projected in [s, o] layout for the first 8 s-tiles only
   (cast to bf16 -> RoPE on DVE -> one wide DMA transpose per s-tile
   into qkt4); the other 8 QK tiles run inside phase B as PE filler
 - phase B: per q-chunk, scores^T[k, q] = Kt.T @ Qt per 128-key block
   (K=64 contraction) into 1024-wide PSUM units; one Exp per unit (ACT),
   causal mask on the diagonal blocks (DVE); PV with lhsT = [V | 1]
   (M=65) so row 64 accumulates the softmax denominator for free;
   normalization reads PSUM directly (reciprocal + gpsimd broadcast +
   DVE mul)
 - the FILLER schedule places one PE tile (deferred QK projection, V
   projection, or the previous chunk's o_proj) at each scores-unit
   boundary so PE and ACT's exp stream advance in lockstep; deadlines:
   vsb[m] before its chunk's diagonal units, qkt4[m] transposed before
   the chunk whose q-range reads it
 - PV accumulators, o_proj PSUM and the filler-projection PSUM share
   banks (tags big0/big1 and the pv pool); the tail o_proj accumulates
   in the freed scores banks with a bf16 normalize chain split per
   m-tile to shorten the critical path
"""

import numpy as np
import ml_dtypes

B, S, D, H = 2, 2048, 1024, 16
HD = 64
NCORES = 8
HEADS_PER_CORE = 4
GDIM = HEADS_PER_CORE * HD          # 256 projection cols per core
SB = S // 128                        # 16 s-tiles
KD = D // 128                        # 8 k-tiles over d
QCHUNK = 512
NQC = S // QCHUNK                    # 4 q-chunks
UNIT = 1024                          # scores psum unit (2 banks)

_BF16 = ml_dtypes.bfloat16
_cache = {}


def _build(use_rope: bool):
    import concourse.bass as bass
    import concourse.mybir as mybir
    import concourse.tile as tile
    from concourse import bacc

    F32 = mybir.dt.float32
    BF16 = mybir.dt.bfloat16
    EXP = mybir.ActivationFunctionType.Exp

    nc = bacc.Bacc(None, target_bir_lowering=False)

    xt_d = nc.dram_tensor("xt", [D, S], BF16, kind="ExternalInput")
    wqk_d = nc.dram_tensor("wqk", [D, 2 * GDIM], BF16, kind="ExternalInput")
    wv_d = nc.dram_tensor("wv", [D, GDIM], BF16, kind="ExternalInput")
    wo_d = nc.dram_tensor("wo", [GDIM, D], BF16, kind="ExternalInput")
    cs_d = nc.dram_tensor("csr", [S, 64], BF16, kind="ExternalInput")
    mask_d = nc.dram_tensor("maskT", [128, 128], BF16, kind="ExternalInput")
    # bf16 output halves the output-DMA bytes; the host accumulates the
    # 4 per-batch partials in fp32
    out_d = nc.dram_tensor("out", [S, D], BF16, kind="ExternalOutput")

    with tile.TileContext(nc) as tc:
        with tc.tile_pool(name="big", bufs=1) as big, \
             tc.tile_pool(name="work", bufs=4) as work, \
             tc.tile_pool(name="ropet", bufs=3) as ropet, \
             tc.tile_pool(name="pex", bufs=4) as pex:
            # ---- resident tensors ----
            xt = big.tile([128, KD, S], BF16)
            wqk = big.tile([128, KD, 2 * GDIM], BF16)
            wv = big.tile([128, KD, GDIM], BF16)
            wo = big.tile([128, 2, D], BF16)
            maskT = big.tile([128, 128], BF16)
            csr = big.tile([128, SB, 64], BF16)
            qkt4 = big.tile([128, 4, S], BF16)
            # qkt4 blocks: 0: Qt heads 0-1, 1: Qt heads 2-3, 2: Kt 0-1, 3: Kt 2-3
            vsb = big.tile([128, SB, HEADS_PER_CORE * 65], BF16)
            yt2 = [big.tile([128, S], BF16, tag=f"yt2{i}", name=f"yt2{i}")
                   for i in range(2)]

            # ones-rows of vsb (column 64 of each 65-wide head slot)
            ones_view = vsb[:].rearrange("p m (h c) -> p m h c", h=4)[:, :, :, 64:65]
            nc.vector.memset(ones_view, 1.0)

            # ---- chunked input loads, ordered so x streams ahead of use ----
            xt_r = xt_d.rearrange("(k p) s -> p k s", p=128)
            wqk_r = wqk_d.rearrange("(k p) o -> p k o", p=128)
            nc.sync.dma_start(xt[:, 0:4, 0:128], xt_r[:, 0:4, 0:128])
            nc.sync.dma_start(wqk[:, 0:2, :], wqk_r[:, 0:2, :])
            nc.sync.dma_start(csr[:], cs_d.rearrange("(m p) f -> p m f", p=128))
            nc.sync.dma_start(wqk[:, 2:4, :], wqk_r[:, 2:4, :])
            nc.sync.dma_start(xt[:, 4:8, 0:128], xt_r[:, 4:8, 0:128])
            nc.sync.dma_start(wqk[:, 4:6, :], wqk_r[:, 4:6, :])
            nc.sync.dma_start(wqk[:, 6:8, :], wqk_r[:, 6:8, :])
            nc.sync.dma_start(xt[:, :, 128:384], xt_r[:, :, 128:384])
            nc.sync.dma_start(xt[:, :, 384:640], xt_r[:, :, 384:640])
            nc.sync.dma_start(wv[:], wv_d.rearrange("(k p) o -> p k o", p=128))
            nc.sync.dma_start(maskT[:], mask_d[:])
            nc.sync.dma_start(xt[:, :, 640:1536], xt_r[:, :, 640:1536])
            nc.sync.dma_start(xt[:, :, 1536:2048], xt_r[:, :, 1536:2048])
            nc.sync.dma_start(wo[:], wo_d.rearrange("(k p) o -> p k o", p=128))

            # shared QK-tile body: projection psum -> bf16 -> rope -> one
            # wide transpose into qkt4. copy_dve picks the cast engine so
            # phase-B filler tiles stay off the exp-loaded ACT queue.
            def emit_rope_transpose(ps, m, copy_dve):
                ms = slice(m * 128, (m + 1) * 128)
                qkr = ropet.tile([128, 2 * GDIM], BF16, tag="qkr")
                if use_rope:
                    qkf = ropet.tile([128, 2 * GDIM], BF16, tag="qkf")
                    if copy_dve:
                        nc.vector.tensor_copy(qkf[:], ps)
                    else:
                        nc.scalar.copy(qkf[:], ps)
                    qv = qkf[:].rearrange("p (g e c) -> p g e c", e=2, c=32)
                    ov = qkr[:].rearrange("p (g e c) -> p g e c", e=2, c=32)
                    E, O = qv[:, :, 0, :], qv[:, :, 1, :]
                    C = csr[:, m, 0:32].unsqueeze(1).broadcast_to((128, 8, 32))
                    Sn = csr[:, m, 32:64].unsqueeze(1).broadcast_to((128, 8, 32))
                    g32 = lambda t: t[:].rearrange("p (g c) -> p g c", c=32)
                    ta = ropet.tile([128, 256], BF16, tag="ta")
                    tb = ropet.tile([128, 256], BF16, tag="tb")
                    nc.vector.tensor_mul(g32(ta), E, C)
                    nc.vector.tensor_mul(g32(tb), O, Sn)
                    nc.vector.tensor_sub(ov[:, :, 0, :], g32(ta), g32(tb))
                    tc_ = ropet.tile([128, 256], BF16, tag="tc")
                    td = ropet.tile([128, 256], BF16, tag="td")
                    nc.vector.tensor_mul(g32(tc_), O, C)
                    nc.vector.tensor_mul(g32(td), E, Sn)
                    nc.vector.tensor_add(ov[:, :, 1, :], g32(tc_), g32(td))
                else:
                    if copy_dve:
                        nc.vector.tensor_copy(qkr[:], ps)
                    else:
                        nc.scalar.copy(qkr[:], ps)
                return nc.sync.dma_start_transpose(qkt4[:, :, ms], qkr[:])

            # ---- phase A: QK projections for the first 8 s-tiles (the
            # rest run inside phase B as PE filler) ----
            act_warm = False
            with tc.tile_pool(name="pa", bufs=2, space="PSUM") as pa:
                for m in range(SB // 2):
                    ms = slice(m * 128, (m + 1) * 128)
                    ps = pa.tile([128, 2 * GDIM], F32, tag="qk")
                    for k in range(KD):
                        nc.tensor.matmul(ps[:], xt[:, k, ms], wqk[:, k, :],
                                         start=(k == 0), stop=(k == KD - 1))
                    emit_rope_transpose(ps[:], m, copy_dve=False)
                    if not act_warm:
                        # preload the Exp table while ACT is idle so the
                        # first phase-B exp doesn't pay the 1.3us load
                        wa = work.tile([1, 1], BF16, tag="wa")
                        nc.scalar.activation(wa[:], wqk[0:1, 0, 0:1],
                                             EXP, scale=0.125)
                        act_warm = True

            # ---- phase B: attention + V projection + interleaved o_proj ----
            with tc.tile_pool(name="sc", bufs=1, space="PSUM") as scp, \
                 tc.tile_pool(name="yb", bufs=1, space="PSUM") as ybp, \
                 tc.tile_pool(name="pv", bufs=2, space="PSUM") as pvp:

                def emit_vproj_tile(m):
                    ms = slice(m * 128, (m + 1) * 128)
                    psv = pvp.tile([128, QCHUNK], F32, tag="v")
                    for k in range(KD):
                        nc.tensor.matmul(psv[:, 0:GDIM], xt[:, k, ms], wv[:, k, :],
                                         start=(k == 0), stop=(k == KD - 1))
                    dst = vsb[:, m, :].rearrange("p (h c) -> p h c", h=4)[:, :, 0:64]
                    nc.scalar.copy(dst, psv[:, 0:GDIM].rearrange("p (h c) -> p h c", h=4))

                def emit_qkproj_tile(m):
                    ms = slice(m * 128, (m + 1) * 128)
                    ps = pvp.tile([128, QCHUNK], F32, tag="v")
                    for k in range(KD):
                        nc.tensor.matmul(ps[:], xt[:, k, ms], wqk[:, k, :],
                                         start=(k == 0), stop=(k == KD - 1))
                    emit_rope_transpose(ps[:], m, copy_dve=True)

                def emit_oproj_tile(m, use_pv=False):
                    ms = slice(m * 128, (m + 1) * 128)
                    so = work.tile([128, 1024], BF16, tag="so")
                    for nb in range(2):
                        if use_pv:
                            # pv-pool accumulator: avoids waiting on the
                            # ytps (big-tag) release behind the normalize
                            po = pvp.tile([128, QCHUNK], F32, tag="v")
                        else:
                            po = ybp.tile([128, QCHUNK], F32, tag=f"big{nb}")
                        for k2 in range(2):
                            nc.tensor.matmul(po[:], yt2[k2][:, ms],
                                             wo[:, k2, nb * 512:(nb + 1) * 512],
                                             start=(k2 == 0), stop=(k2 == 1))
                        # copies alternate DVE/ACT to halve the serial chain
                        if nb == 0:
                            nc.vector.tensor_copy(so[:, 0:512], po[:])
                        else:
                            nc.scalar.copy(so[:, 512:1024], po[:])
                    nc.sync.dma_start(out_d[ms, :], so[:])

                # qc0's diagonal PV needs vsb s-tiles 0-3 immediately;
                # project them first (also keeps PE warm across the
                # phase boundary)
                for mi in range(5):
                    emit_vproj_tile(mi)

                # filler schedule: one PE tile (o_proj of an earlier chunk,
                # V projection, or a deferred QK projection) per unit
                # boundary so ACT's exp stream and PE advance in lockstep.
                # Deadlines: vsb[m] before its chunk's diagonal units, and
                # qkt4[m] transposed before the chunk whose q-range needs it.
                FILLER = {
                    (0, 0, 0): [("qk", 8)], (0, 0, 1): [("v", 5)],
                    (0, 1, 0): [("v", 6)], (0, 1, 1): [("v", 7)],
                    (1, 0, 0): [("op", 0), ("op", 1)],
                    (1, 0, 1): [("qk", 9)], (1, 0, 2): [("qk", 10)],
                    (1, 0, 3): [],
                    (1, 1, 0): [("oppv", 2), ("oppv", 3)],
                    (1, 1, 1): [("qk", 11)], (1, 1, 2): [("v", 8)],
                    (1, 1, 3): [("v", 9)], (1, 1, 4): [],
                    (2, 0, 0): [("op", 4), ("op", 5)],
                    (2, 0, 1): [("v", 10)], (2, 0, 2): [("v", 11)],
                    (2, 0, 3): [("qk", 12)], (2, 0, 4): [("qk", 13)],
                    (2, 1, 0): [("oppv", 6), ("oppv", 7)],
                    (2, 1, 1): [("qk", 14)], (2, 1, 2): [("qk", 15)],
                    (3, 0, 0): [("v", 12)], (3, 0, 1): [("v", 13)],
                    (3, 0, 2): [("v", 14)], (3, 0, 3): [("v", 15)],
                    (3, 1, 0): [("oppv", 10), ("oppv", 11)],
                    (3, 1, 1): [("oppv", 8)], (3, 1, 2): [("oppv", 9)],
                }

                for qc in range(NQC):
                    q0 = qc * QCHUNK
                    # block list: full key blocks then the diagonal in order
                    # 512,384,128,256 so no matmul crosses a 512-col bank
                    order = list(range(4 * qc)) + \
                        [4 * qc, 4 * qc + 1, 4 * qc + 3, 4 * qc + 2]
                    blocks = []
                    for kb in order:
                        r = max(0, kb - 4 * qc)
                        blocks.append((kb, q0 + r * 128, QCHUNK - r * 128))
                    units, cur, cols = [], [], 0
                    for kb, qoff, n in blocks:
                        if cols + n > UNIT:
                            units.append(cur)
                            cur, cols = [], 0
                        cur.append((kb, qoff, n, cols))
                        cols += n
                    units.append(cur)
                    last_kb = units[-1][-1][0]

                    for hp in range(2):
                        ytps = None
                        for ui, unit in enumerate(units):
                            ucols = unit[-1][3] + unit[-1][2]
                            scs = [scp.tile([128, UNIT], F32, tag=f"sc{i}",
                                            name=f"sc{i}") for i in range(2)]
                            for i in range(2):
                                rows = slice(i * 64, i * 64 + 64)
                                for (kb, qoff, n, o) in unit:
                                    nc.tensor.matmul(
                                        scs[i][:, o:o + n],
                                        qkt4[rows, 2 + hp, kb * 128:(kb + 1) * 128],
                                        qkt4[rows, hp, qoff:qoff + n],
                                        start=True, stop=True)
                            # PE filler between scores and PV: previous
                            # chunk's o_proj (must precede this chunk's ytps
                            # allocation — the po tiles reuse the big banks)
                            # and this chunk's V projection, spread across
                            # unit boundaries so ACT's exp stream never
                            # starves. The vsb tiles feed only the diagonal
                            # units, several pipeline steps later.
                            for kind, fm in FILLER.get((qc, hp, ui), ()):
                                if kind == "op":
                                    emit_oproj_tile(fm)
                                elif kind == "oppv":
                                    emit_oproj_tile(fm, use_pv=True)
                                elif kind == "v":
                                    emit_vproj_tile(fm)
                                else:
                                    emit_qkproj_tile(fm)
                            if ytps is None:
                                ytps = [ybp.tile([128, QCHUNK], F32,
                                                 tag=f"big{i}", name=f"big{i}")
                                        for i in range(2)]
                            for i in range(2):
                                h = 2 * hp + i
                                vcol = slice(h * 65, h * 65 + 65)
                                pe = pex.tile([128, UNIT], BF16,
                                              tag=f"pe{i}", name=f"pe{i}")
                                nc.scalar.activation(pe[:, :ucols],
                                                     scs[i][:, :ucols],
                                                     EXP, scale=0.125)
                                for (kb, qoff, n, o) in unit:
                                    if kb >= 4 * qc:  # diagonal: causal mask
                                        nc.vector.tensor_mul(
                                            pe[:, o:o + 128], pe[:, o:o + 128],
                                            maskT[:])
                                    # kb==0 always has n=512: start clears
                                    # the whole [65, QCHUNK] accumulator
                                    nc.tensor.matmul(
                                        ytps[i][0:65, qoff - q0:qoff - q0 + n],
                                        vsb[:, kb, vcol],
                                        pe[:, o:o + n],
                                        start=(kb == 0), stop=(kb == last_kb))
                        if qc == NQC - 1 and hp == 1:
                            # tail: bf16 normalize (denominator precision
                            # cost ~0.4%, well inside the error budget).
                            # ACT casts y to bf16 while DVE does the
                            # reciprocals; the per-m-tile muls then run in
                            # 2x mode and stagger the o_proj k2=1 pairs.
                            # Reading ytps via ybf also releases the big
                            # banks for the tail slots sooner.
                            rcs, ybfs = [], []
                            for i in range(2):
                                rc = work.tile([1, QCHUNK], BF16, tag=f"rcb{i}")
                                with nc.allow_low_precision(
                                        reason="bf16 softmax denominator"):
                                    nc.vector.reciprocal(rc[:],
                                                         ytps[i][64:65, :])
                                rcs.append(rc)
                                ybf = work.tile([64, QCHUNK], BF16,
                                                tag=f"ybf{i}")
                                nc.scalar.copy(ybf[:], ytps[i][0:64, :])
                                ybfs.append(ybf)
                            for mi in range(4):
                                mc = slice(mi * 128, (mi + 1) * 128)
                                for i in range(2):
                                    bc = work.tile([64, 128], BF16, tag="bct")
                                    nc.gpsimd.partition_broadcast(
                                        bc[:], rcs[i][0:1, mc])
                                    nc.vector.tensor_mul(
                                        yt2[hp][i * 64:i * 64 + 64,
                                                q0 + mi * 128:
                                                q0 + (mi + 1) * 128],
                                        ybfs[i][:, mc], bc[:])
                        else:
                            for i in range(2):
                                # normalize straight from PSUM
                                rc = work.tile([1, QCHUNK], F32, tag="rc")
                                nc.vector.reciprocal(rc[:], ytps[i][64:65, :])
                                bc = work.tile([64, QCHUNK], F32, tag="bc")
                                nc.gpsimd.partition_broadcast(bc[:], rc[0:1, :])
                                nc.vector.tensor_mul(
                                    yt2[hp][i * 64:i * 64 + 64, q0:q0 + QCHUNK],
                                    ytps[i][0:64, :], bc[:])
                    if qc == NQC - 1:
                        # tail o_proj with 8 independent PSUM accumulators
                        # (freed scores banks + pv pool + big banks) so no
                        # matmul waits on a copy; all k2=0 matmuls run
                        # during the normalize chains, k2=1 staggers in as
                        # the per-m muls land; copies alternate DVE/ACT
                        slots = []
                        for mi in range(2):
                            pot = scp.tile([128, UNIT], F32,
                                           tag=f"sc{mi}", name=f"sc{mi}")
                            slots.append((pot[:, 0:512], pot[:, 512:1024]))
                        for mi in range(2):
                            a = pvp.tile([128, QCHUNK], F32, tag="v")
                            b = ybp.tile([128, QCHUNK], F32,
                                         tag=f"big{mi}", name=f"big{mi}")
                            slots.append((a[:], b[:]))
                        for k2 in range(2):
                            for mi in range(4):
                                m = qc * 4 + mi
                                ms = slice(m * 128, (m + 1) * 128)
                                for nb in range(2):
                                    nc.tensor.matmul(
                                        slots[mi][nb], yt2[k2][:, ms],
                                        wo[:, k2, nb * 512:(nb + 1) * 512],
                                        start=(k2 == 0), stop=(k2 == 1))
                        for mi in range(4):
                            m = qc * 4 + mi
                            ms = slice(m * 128, (m + 1) * 128)
                            so = work.tile([128, 1024], BF16, tag=f"sot{mi}")
                            for nb in range(2):
                                if (mi + nb) % 2 == 0:
                                    nc.vector.tensor_copy(
                                        so[:, nb * 512:(nb + 1) * 512],
                                        slots[mi][nb])
                                else:
                                    nc.scalar.copy(
                                        so[:, nb * 512:(nb + 1) * 512],
                                        slots[mi][nb])
                            nc.sync.dma_start(out_d[ms, :], so[:])
    nc.compile()
    return nc


def _prep_core_inputs(x, Wq, Wk, Wv, Wo, cos_g, sin_g, use_rope):
    """Host-side shard + layout prep. Returns list of 8 input dicts."""
    maskT = np.tril(np.ones((128, 128), np.float32)).T.astype(_BF16)
    # per-head row permutation: [evens | odds] so rope pairs are
    # (j, j+32) within each head's 64 projection dims
    perm = np.concatenate([np.arange(h * 64, (h + 1) * 64).reshape(32, 2).T.reshape(64)
                           for h in range(H)])
    Wq_p = Wq[perm]
    Wk_p = Wk[perm]
    maps = []
    for c in range(NCORES):
        b, g = divmod(c, HEADS_PER_CORE)
        rows = slice(g * GDIM, (g + 1) * GDIM)
        wqk = np.concatenate([Wq_p[rows], Wk_p[rows]], axis=0).T  # [D, 512]
        maps.append({
            "xt": np.ascontiguousarray(x[b].T).astype(_BF16),
            "wqk": np.ascontiguousarray(wqk).astype(_BF16),
            "wv": np.ascontiguousarray(Wv[rows].T).astype(_BF16),
            "wo": np.ascontiguousarray(Wo[:, rows].T).astype(_BF16),
            "csr": np.concatenate([cos_g, sin_g], axis=1).astype(_BF16),
            "maskT": maskT,
        })
    return maps


def kernel(x, token_positions, use_rope, Wq, Wk, Wv, Wo, cos, sin):
    from concourse.bass_utils import run_bass_kernel_spmd

    x = np.asarray(x, np.float32)
    token_positions = np.asarray(token_positions)
    Wq = np.asarray(Wq, np.float32)
    Wk = np.asarray(Wk, np.float32)
    Wv = np.asarray(Wv, np.float32)
    Wo = np.asarray(Wo, np.float32)
    cos = np.asarray(cos, np.float32)
    sin = np.asarray(sin, np.float32)
    rope = bool(int(use_rope))

    cos_g = cos[token_positions]  # [S, 32]
    sin_g = sin[token_positions]

    if rope not in _cache:
        _cache[rope] = _build(rope)
    nc = _cache[rope]

    in_maps = _prep_core_inputs(x, Wq, Wk, Wv, Wo, cos_g, sin_g, rope)
    res = run_bass_kernel_spmd(nc, in_maps, list(range(NCORES)))

    out = np.zeros((B, S, D), np.float32)
    for c in range(NCORES):
        out[c // HEADS_PER_CORE] += res.results[c]["out"].astype(np.float32)
    return out


# revision 49
# speedup vs baseline: 1.2209x; 1.0094x over previous
"""Causal multi-head attention with RoPE for Trainium2, 8-core SPMD.

Problem: B=2, S=2048, D_MODEL=1024, H=16, HD=64, causal softmax(QK^T/8)V
with interleaved-pair RoPE on q/k, projections Wq/Wk/Wv/Wo.

Sharding (host side): batch x head-group. Core c handles batch b=c//4 and
head group g=c%4 (heads 4g..4g+3, a 256-wide slice of the projection dims).
Each core computes a full [S, D_MODEL] partial of the output (its head
group's contribution through Wo); host sums 4 partials per batch.

Device schedule (all matmuls bf16, fp32 accumulate):
 - host passes x[b].T so the d-contraction sits on SBUF partitions
 - Wq/Wk rows are permuted per head to [evens | odds] so RoPE pairs are
   (col j, col j+32) within each head: the DVE ops run on contiguous
   32-wide groups (packed, 2x mode) and cos/sin tables are the raw
   [S, 32] tables read through stride-0 broadcast views
 - phase A: Q,K projected in [s, o] layout for the first 8 s-tiles only
   (cast to bf16 -> RoPE on DVE -> one wide DMA transpose per s-tile
   into qkt4); the other 8 QK tiles run inside phase B as PE filler
 - phase B: per q-chunk, scores^T[k, q] = Kt.T @ Qt per 128-key block
   (K=64 contraction) into 1024-wide PSUM units; one Exp per unit (ACT),
   causal mask on the diagonal blocks (DVE); PV with lhsT = [V | 1]
   (M=65) so row 64 accumulates the softmax denominator for free;
   normalization reads PSUM directly (reciprocal + gpsimd broadcast +
   DVE mul)
 - the FILLER schedule places one PE tile (deferred QK projection, V
   projection, or the previous chunk's o_proj) at each scores-unit
   boundary so PE and ACT's exp stream advance in lockstep; deadlines:
   vsb[m] before its chunk's diagonal units, qkt4[m] transposed before
   the chunk whose q-range reads it
 - PV accumulators, o_proj PSUM and the filler-projection PSUM share
   banks (tags big0/big1 and the pv pool); the tail o_proj accumulates
   in the freed scores banks with a bf16 normalize chain split per
   m-tile to shorten the critical path
"""

import numpy as np
import ml_dtypes

B, S, D, H = 2, 2048, 1024, 16
HD = 64
NCORES = 8
HEADS_PER_CORE = 4
GDIM = HEADS_PER_CORE * HD          # 256 projection cols per core
SB = S // 128                        # 16 s-tiles
KD = D // 128                        # 8 k-tiles over d
QCHUNK = 512
NQC = S // QCHUNK                    # 4 q-chunks
UNIT = 1024                          # scores psum unit (2 banks)

_BF16 = ml_dtypes.bfloat16
_cache = {}


def _build(use_rope: bool):
    import concourse.bass as bass
    import concourse.mybir as mybir
    import concourse.tile as tile
    from concourse import bacc

    F32 = mybir.dt.float32
    BF16 = mybir.dt.bfloat16
    EXP = mybir.ActivationFunctionType.Exp

    nc = bacc.Bacc(None, target_bir_lowering=False)

    xt_d = nc.dram_tensor("xt", [D, S], BF16, kind="ExternalInput")
    wqk_d = nc.dram_tensor("wqk", [D, 2 * GDIM], BF16, kind="ExternalInput")
    wv_d = nc.dram_tensor("wv", [D, GDIM], BF16, kind="ExternalInput")
    wo_d = nc.dram_tensor("wo", [GDIM, D], BF16, kind="ExternalInput")
    cs_d = nc.dram_tensor("csr", [S, 64], BF16, kind="ExternalInput")
    mask_d = nc.dram_tensor("maskT", [128, 128], BF16, kind="ExternalInput")
    # bf16 output halves the output-DMA bytes; the host accumulates the
    # 4 per-batch partials in fp32
    out_d = nc.dram_tensor("out", [S, D], BF16, kind="ExternalOutput")

    with tile.TileContext(nc) as tc:
        with tc.tile_pool(name="big", bufs=1) as big, \
             tc.tile_pool(name="work", bufs=4) as work, \
             tc.tile_pool(name="ropet", bufs=3) as ropet, \
             tc.tile_pool(name="pex", bufs=4) as pex:
            # ---- resident tensors ----
            xt = big.tile([128, KD, S], BF16)
            wqk = big.tile([128, KD, 2 * GDIM], BF16)
            wv = big.tile([128, KD, GDIM], BF16)
            wo = big.tile([128, 2, D], BF16)
            maskT = big.tile([128, 128], BF16)
            csr = big.tile([128, SB, 64], BF16)
            qkt4 = big.tile([128, 4, S], BF16)
            # qkt4 blocks: 0: Qt heads 0-1, 1: Qt heads 2-3, 2: Kt 0-1, 3: Kt 2-3
            vsb = big.tile([128, SB, HEADS_PER_CORE * 65], BF16)
            yt2 = [big.tile([128, S], BF16, tag=f"yt2{i}", name=f"yt2{i}")
                   for i in range(2)]

            # ones-rows of vsb (column 64 of each 65-wide head slot)
            ones_view = vsb[:].rearrange("p m (h c) -> p m h c", h=4)[:, :, :, 64:65]
            nc.vector.memset(ones_view, 1.0)

            # ---- chunked input loads, ordered so x streams ahead of use ----
            xt_r = xt_d.rearrange("(k p) s -> p k s", p=128)
            wqk_r = wqk_d.rearrange("(k p) o -> p k o", p=128)
            nc.sync.dma_start(xt[:, 0:4, 0:128], xt_r[:, 0:4, 0:128])
            nc.sync.dma_start(wqk[:, 0:2, :], wqk_r[:, 0:2, :])
            nc.sync.dma_start(csr[:], cs_d.rearrange("(m p) f -> p m f", p=128))
            nc.sync.dma_start(wqk[:, 2:4, :], wqk_r[:, 2:4, :])
            nc.sync.dma_start(xt[:, 4:8, 0:128], xt_r[:, 4:8, 0:128])
            nc.sync.dma_start(wqk[:, 4:6, :], wqk_r[:, 4:6, :])
            nc.sync.dma_start(wqk[:, 6:8, :], wqk_r[:, 6:8, :])
            nc.sync.dma_start(xt[:, :, 128:384], xt_r[:, :, 128:384])
            nc.sync.dma_start(xt[:, :, 384:640], xt_r[:, :, 384:640])
            nc.sync.dma_start(wv[:], wv_d.rearrange("(k p) o -> p k o", p=128))
            nc.sync.dma_start(maskT[:], mask_d[:])
            nc.sync.dma_start(xt[:, :, 640:1536], xt_r[:, :, 640:1536])
            nc.sync.dma_start(xt[:, :, 1536:2048], xt_r[:, :, 1536:2048])
            nc.sync.dma_start(wo[:], wo_d.rearrange("(k p) o -> p k o", p=128))

            # shared QK-tile body: projection psum -> bf16 -> rope -> one
            # wide transpose into qkt4. copy_dve picks the cast engine so
            # phase-B filler tiles stay off the exp-loaded ACT queue.
            def emit_rope_transpose(ps, m, copy_dve):
                ms = slice(m * 128, (m + 1) * 128)
                qkr = ropet.tile([128, 2 * GDIM], BF16, tag="qkr")
                if use_rope:
                    qkf = ropet.tile([128, 2 * GDIM], BF16, tag="qkf")
                    if copy_dve:
                        nc.vector.tensor_copy(qkf[:], ps)
                    else:
                        nc.scalar.copy(qkf[:], ps)
                    qv = qkf[:].rearrange("p (g e c) -> p g e c", e=2, c=32)
                    ov = qkr[:].rearrange("p (g e c) -> p g e c", e=2, c=32)
                    E, O = qv[:, :, 0, :], qv[:, :, 1, :]
                    C = csr[:, m, 0:32].unsqueeze(1).broadcast_to((128, 8, 32))
                    Sn = csr[:, m, 32:64].unsqueeze(1).broadcast_to((128, 8, 32))
                    g32 = lambda t: t[:].rearrange("p (g c) -> p g c", c=32)
                    ta = ropet.tile([128, 256], BF16, tag="ta")
                    tb = ropet.tile([128, 256], BF16, tag="tb")
                    nc.vector.tensor_mul(g32(ta), E, C)
                    nc.vector.tensor_mul(g32(tb), O, Sn)
                    nc.vector.tensor_sub(ov[:, :, 0, :], g32(ta), g32(tb))
                    tc_ = ropet.tile([128, 256], BF16, tag="tc")
                    td = ropet.tile([128, 256], BF16, tag="td")
                    nc.vector.tensor_mul(g32(tc_), O, C)
                    nc.vector.tensor_mul(g32(td), E, Sn)
                    nc.vector.tensor_add(ov[:, :, 1, :], g32(tc_), g32(td))
                else:
                    if copy_dve:
                        nc.vector.tensor_copy(qkr[:], ps)
                    else:
                        nc.scalar.copy(qkr[:], ps)
                return nc.sync.dma_start_transpose(qkt4[:, :, ms], qkr[:])

            # ---- phase A: QK projections for the first 8 s-tiles (the
            # rest run inside phase B as PE filler) ----
            act_warm = False
            with tc.tile_pool(name="pa", bufs=2, space="PSUM") as pa:
                for m in range(SB // 2):
                    ms = slice(m * 128, (m + 1) * 128)
                    ps = pa.tile([128, 2 * GDIM], F32, tag="qk")
                    for k in range(KD):
                        nc.tensor.matmul(ps[:], xt[:, k, ms], wqk[:, k, :],
                                         start=(k == 0), stop=(k == KD - 1))
                    emit_rope_transpose(ps[:], m, copy_dve=False)
                    if not act_warm:
                        # preload the Exp table while ACT is idle so the
                        # first phase-B exp doesn't pay the 1.3us load
                        wa = work.tile([1, 1], BF16, tag="wa")
                        nc.scalar.activation(wa[:], wqk[0:1, 0, 0:1],
                                             EXP, scale=0.125)
                        act_warm = True

            # ---- phase B: attention + V projection + interleaved o_proj ----
            with tc.tile_pool(name="sc", bufs=1, space="PSUM") as scp, \
                 tc.tile_pool(name="yb", bufs=1, space="PSUM") as ybp, \
                 tc.tile_pool(name="pv", bufs=2, space="PSUM") as pvp:

                def emit_vproj_tile(m):
                    ms = slice(m * 128, (m + 1) * 128)
                    psv = pvp.tile([128, QCHUNK], F32, tag="v")
                    for k in range(KD):
                        nc.tensor.matmul(psv[:, 0:GDIM], xt[:, k, ms], wv[:, k, :],
                                         start=(k == 0), stop=(k == KD - 1))
                    dst = vsb[:, m, :].rearrange("p (h c) -> p h c", h=4)[:, :, 0:64]
                    nc.scalar.copy(dst, psv[:, 0:GDIM].rearrange("p (h c) -> p h c", h=4))

                def emit_qkproj_tile(m):
                    ms = slice(m * 128, (m + 1) * 128)
                    ps = pvp.tile([128, QCHUNK], F32, tag="v")
                    for k in range(KD):
                        nc.tensor.matmul(ps[:], xt[:, k, ms], wqk[:, k, :],
                                         start=(k == 0), stop=(k == KD - 1))
                    emit_rope_transpose(ps[:], m, copy_dve=True)

                def emit_oproj_tile(m, use_pv=False):
                    ms = slice(m * 128, (m + 1) * 128)
                    so = work.tile([128, 1024], BF16, tag="so")
                    for nb in range(2):
                        if use_pv:
                            # pv-pool accumulator: avoids waiting on the
                            # ytps (big-tag) release behind the normalize
                            po = pvp.tile([128, QCHUNK], F32, tag="v")
                        else:
                            po = ybp.tile([128, QCHUNK], F32, tag=f"big{nb}")
                        for k2 in range(2):
                            nc.tensor.matmul(po[:], yt2[k2][:, ms],
                                             wo[:, k2, nb * 512:(nb + 1) * 512],
                                             start=(k2 == 0), stop=(k2 == 1))
                        # copies alternate DVE/ACT to halve the serial chain
                        if nb == 0:
                            nc.vector.tensor_copy(so[:, 0:512], po[:])
                        else:
                            nc.scalar.copy(so[:, 512:1024], po[:])
                    nc.sync.dma_start(out_d[ms, :], so[:])

                # qc0's diagonal PV needs vsb s-tiles 0-3 immediately;
                # project them first (also keeps PE warm across the
                # phase boundary)
                for mi in range(5):
                    emit_vproj_tile(mi)

                # filler schedule: one PE tile (o_proj of an earlier chunk,
                # V projection, or a deferred QK projection) per unit
                # boundary so ACT's exp stream and PE advance in lockstep.
                # Deadlines: vsb[m] before its chunk's diagonal units, and
                # qkt4[m] transposed before the chunk whose q-range needs it.
                FILLER = {
                    (0, 0, 0): [("qk", 8)], (0, 0, 1): [("v", 5)],
                    (0, 1, 0): [("v", 6)], (0, 1, 1): [("v", 7)],
                    (1, 0, 0): [("qk", 9)], (1, 0, 1): [("qk", 10)],
                    (1, 0, 2): [("v", 8)], (1, 0, 3): [("v", 9)],
                    (1, 1, 0): [("oppv", 2), ("oppv", 3)],
                    (1, 1, 1): [("qk", 11)], (1, 1, 2): [("oppv", 0)],
                    (1, 1, 3): [("oppv", 1)],
                    (2, 0, 0): [("v", 10)], (2, 0, 1): [("v", 11)],
                    (2, 0, 2): [("qk", 12)], (2, 0, 3): [("qk", 13)],
                    (2, 1, 0): [("oppv", 6), ("oppv", 7)],
                    (2, 1, 1): [("qk", 14)], (2, 1, 2): [("qk", 15)],
                    (2, 1, 3): [("oppv", 4)], (2, 1, 4): [("oppv", 5)],
                    (3, 0, 0): [("v", 12)], (3, 0, 1): [("v", 13)],
                    (3, 0, 2): [("v", 14)], (3, 0, 3): [("v", 15)],
                    (3, 1, 0): [("oppv", 10), ("oppv", 11)],
                    (3, 1, 1): [("oppv", 8)], (3, 1, 2): [("oppv", 9)],
                }

                for qc in range(NQC):
                    q0 = qc * QCHUNK
                    # block list: full key blocks then the diagonal in order
                    # 512,384,128,256 so no matmul crosses a 512-col bank
                    order = list(range(4 * qc)) + \
                        [4 * qc, 4 * qc + 1, 4 * qc + 3, 4 * qc + 2]
                    blocks = []
                    for kb in order:
                        r = max(0, kb - 4 * qc)
                        blocks.append((kb, q0 + r * 128, QCHUNK - r * 128))
                    units, cur, cols = [], [], 0
                    for kb, qoff, n in blocks:
                        if cols + n > UNIT:
                            units.append(cur)
                            cur, cols = [], 0
                        cur.append((kb, qoff, n, cols))
                        cols += n
                    units.append(cur)
                    last_kb = units[-1][-1][0]

                    for hp in range(2):
                        ytps = None
                        for ui, unit in enumerate(units):
                            ucols = unit[-1][3] + unit[-1][2]
                            scs = [scp.tile([128, UNIT], F32, tag=f"sc{i}",
                                            name=f"sc{i}") for i in range(2)]
                            for i in range(2):
                                rows = slice(i * 64, i * 64 + 64)
                                for (kb, qoff, n, o) in unit:
                                    nc.tensor.matmul(
                                        scs[i][:, o:o + n],
                                        qkt4[rows, 2 + hp, kb * 128:(kb + 1) * 128],
                                        qkt4[rows, hp, qoff:qoff + n],
                                        start=True, stop=True)
                            # PE filler between scores and PV: previous
                            # chunk's o_proj (must precede this chunk's ytps
                            # allocation — the po tiles reuse the big banks)
                            # and this chunk's V projection, spread across
                            # unit boundaries so ACT's exp stream never
                            # starves. The vsb tiles feed only the diagonal
                            # units, several pipeline steps later.
                            for kind, fm in FILLER.get((qc, hp, ui), ()):
                                if kind == "op":
                                    emit_oproj_tile(fm)
                                elif kind == "oppv":
                                    emit_oproj_tile(fm, use_pv=True)
                                elif kind == "v":
                                    emit_vproj_tile(fm)
                                else:
                                    emit_qkproj_tile(fm)
                            if ytps is None:
                                ytps = [ybp.tile([128, QCHUNK], F32,
                                                 tag=f"big{i}", name=f"big{i}")
                                        for i in range(2)]
                            for i in range(2):
                                h = 2 * hp + i
                                vcol = slice(h * 65, h * 65 + 65)
                                pe = pex.tile([128, UNIT], BF16,
                                              tag=f"pe{i}", name=f"pe{i}")
                                nc.scalar.activation(pe[:, :ucols],
                                                     scs[i][:, :ucols],
                                                     EXP, scale=0.125)
                                for (kb, qoff, n, o) in unit:
                                    if kb >= 4 * qc:  # diagonal: causal mask
                                        nc.vector.tensor_mul(
                                            pe[:, o:o + 128], pe[:, o:o + 128],
                                            maskT[:])
                                    # kb==0 always has n=512: start clears
                                    # the whole [65, QCHUNK] accumulator
                                    nc.tensor.matmul(
                                        ytps[i][0:65, qoff - q0:qoff - q0 + n],
                                        vsb[:, kb, vcol],
                                        pe[:, o:o + n],
                                        start=(kb == 0), stop=(kb == last_kb))
                        if qc == NQC - 1 and hp == 1:
                            # tail: bf16 normalize (denominator precision
                            # cost ~0.4%, well inside the error budget).
                            # ACT casts y to bf16 while DVE does the
                            # reciprocals; the per-m-tile muls then run in
                            # 2x mode and stagger the o_proj k2=1 pairs.
                            # Reading ytps via ybf also releases the big
                            # banks for the tail slots sooner.
                            rcs, ybfs = [], []
                            for i in range(2):
                                rc = work.tile([1, QCHUNK], BF16, tag=f"rcb{i}")
                                with nc.allow_low_precision(
                                        reason="bf16 softmax denominator"):
                                    nc.vector.reciprocal(rc[:],
                                                         ytps[i][64:65, :])
                                rcs.append(rc)
                                ybf = work.tile([64, QCHUNK], BF16,
                                                tag=f"ybf{i}")
                                nc.scalar.copy(ybf[:], ytps[i][0:64, :])
                                ybfs.append(ybf)
                            for mi in range(4):
                                mc = slice(mi * 128, (mi + 1) * 128)
                                for i in range(2):
                                    bc = work.tile([64, 128], BF16, tag="bct")
                                    nc.gpsimd.partition_broadcast(
                                        bc[:], rcs[i][0:1, mc])
                                    nc.vector.tensor_mul(
                                        yt2[hp][i * 64:i * 64 + 64,
                                                q0 + mi * 128:
                                                q0 + (mi + 1) * 128],
                                        ybfs[i][:, mc], bc[:])
                        else:
                            for i in range(2):
                                # normalize straight from PSUM
                                rc = work.tile([1, QCHUNK], F32, tag="rc")
                                nc.vector.reciprocal(rc[:], ytps[i][64:65, :])
                                bc = work.tile([64, QCHUNK], F32, tag="bc")
                                nc.gpsimd.partition_broadcast(bc[:], rc[0:1, :])
                                nc.vector.tensor_mul(
                                    yt2[hp][i * 64:i * 64 + 64, q0:q0 + QCHUNK],
                                    ytps[i][0:64, :], bc[:])
                    if qc == NQC - 1:
                        # tail o_proj with 8 independent PSUM accumulators
                        # (freed scores banks + pv pool + big banks) so no
                        # matmul waits on a copy; all k2=0 matmuls run
                        # during the normalize chains, k2=1 staggers in as
                        # the per-m muls land; copies alternate DVE/ACT
                        slots = []
                        for mi in range(2):
                            pot = scp.tile([128, UNIT], F32,
                                           tag=f"sc{mi}", name=f"sc{mi}")
                            slots.append((pot[:, 0:512], pot[:, 512:1024]))
                        for mi in range(2):
                            a = pvp.tile([128, QCHUNK], F32, tag="v")
                            b = ybp.tile([128, QCHUNK], F32,
                                         tag=f"big{mi}", name=f"big{mi}")
                            slots.append((a[:], b[:]))
                        for k2 in range(2):
                            for mi in range(4):
                                m = qc * 4 + mi
                                ms = slice(m * 128, (m + 1) * 128)
                                for nb in range(2):
                                    nc.tensor.matmul(
                                        slots[mi][nb], yt2[k2][:, ms],
                                        wo[:, k2, nb * 512:(nb + 1) * 512],
                                        start=(k2 == 0), stop=(k2 == 1))
                        for mi in range(4):
                            m = qc * 4 + mi
                            ms = slice(m * 128, (m + 1) * 128)
                            so = work.tile([128, 1024], BF16, tag=f"sot{mi}")
                            for nb in range(2):
                                if (mi + nb) % 2 == 0:
                                    nc.vector.tensor_copy(
                                        so[:, nb * 512:(nb + 1) * 512],
                                        slots[mi][nb])
                                else:
                                    nc.scalar.copy(
                                        so[:, nb * 512:(nb + 1) * 512],
                                        slots[mi][nb])
                            nc.sync.dma_start(out_d[ms, :], so[:])
    nc.compile()
    return nc


def _prep_core_inputs(x, Wq, Wk, Wv, Wo, cos_g, sin_g, use_rope):
    """Host-side shard + layout prep. Returns list of 8 input dicts."""
    maskT = np.tril(np.ones((128, 128), np.float32)).T.astype(_BF16)
    # per-head row permutation: [evens | odds] so rope pairs are
    # (j, j+32) within each head's 64 projection dims
    perm = np.concatenate([np.arange(h * 64, (h + 1) * 64).reshape(32, 2).T.reshape(64)
                           for h in range(H)])
    Wq_p = Wq[perm]
    Wk_p = Wk[perm]
    maps = []
    for c in range(NCORES):
        b, g = divmod(c, HEADS_PER_CORE)
        rows = slice(g * GDIM, (g + 1) * GDIM)
        wqk = np.concatenate([Wq_p[rows], Wk_p[rows]], axis=0).T  # [D, 512]
        maps.append({
            "xt": np.ascontiguousarray(x[b].T).astype(_BF16),
            "wqk": np.ascontiguousarray(wqk).astype(_BF16),
            "wv": np.ascontiguousarray(Wv[rows].T).astype(_BF16),
            "wo": np.ascontiguousarray(Wo[:, rows].T).astype(_BF16),
            "csr": np.concatenate([cos_g, sin_g], axis=1).astype(_BF16),
            "maskT": maskT,
        })
    return maps


def kernel(x, token_positions, use_rope, Wq, Wk, Wv, Wo, cos, sin):
    from concourse.bass_utils import run_bass_kernel_spmd

    x = np.asarray(x, np.float32)
    token_positions = np.asarray(token_positions)
    Wq = np.asarray(Wq, np.float32)
    Wk = np.asarray(Wk, np.float32)
    Wv = np.asarray(Wv, np.float32)
    Wo = np.asarray(Wo, np.float32)
    cos = np.asarray(cos, np.float32)
    sin = np.asarray(sin, np.float32)
    rope = bool(int(use_rope))

    cos_g = cos[token_positions]  # [S, 32]
    sin_g = sin[token_positions]

    if rope not in _cache:
        _cache[rope] = _build(rope)
    nc = _cache[rope]

    in_maps = _prep_core_inputs(x, Wq, Wk, Wv, Wo, cos_g, sin_g, rope)
    res = run_bass_kernel_spmd(nc, in_maps, list(range(NCORES)))

    out = np.zeros((B, S, D), np.float32)
    for c in range(NCORES):
        out[c // HEADS_PER_CORE] += res.results[c]["out"].astype(np.float32)
    return out


# revision 50
# speedup vs baseline: 1.2298x; 1.0073x over previous
"""Causal multi-head attention with RoPE for Trainium2, 8-core SPMD.

Problem: B=2, S=2048, D_MODEL=1024, H=16, HD=64, causal softmax(QK^T/8)V
with interleaved-pair RoPE on q/k, projections Wq/Wk/Wv/Wo.

Sharding (host side): batch x head-group. Core c handles batch b=c//4 and
head group g=c%4 (heads 4g..4g+3, a 256-wide slice of the projection dims).
Each core computes a full [S, D_MODEL] partial of the output (its head
group's contribution through Wo); host sums 4 partials per batch.

Device schedule (all matmuls bf16, fp32 accumulate):
 - host passes x[b].T so the d-contraction sits on SBUF partitions
 - Wq/Wk rows are permuted per head to [evens | odds] so RoPE pairs are
   (col j, col j+32) within each head: the DVE ops run on contiguous
   32-wide groups (packed, 2x mode) and cos/sin tables are the raw
   [S, 32] tables read through stride-0 broadcast views
 - phase A: Q,K projected in [s, o] layout for the first 8 s-tiles only
   (cast to bf16 -> RoPE on DVE -> one wide DMA transpose per s-tile
   into qkt4); the other 8 QK tiles run inside phase B as PE filler
 - phase B: per q-chunk, scores^T[k, q] = Kt.T @ Qt per 128-key block
   (K=64 contraction) into 1024-wide PSUM units; one Exp per unit (ACT),
   causal mask on the diagonal blocks (DVE); PV with lhsT = [V | 1]
   (M=65) so row 64 accumulates the softmax denominator for free;
   normalization reads PSUM directly (reciprocal + gpsimd broadcast +
   DVE mul)
 - the FILLER schedule places one PE tile (deferred QK projection, V
   projection, or the previous chunk's o_proj) at each scores-unit
   boundary so PE and ACT's exp stream advance in lockstep; deadlines:
   vsb[m] before its chunk's diagonal units, qkt4[m] transposed before
   the chunk whose q-range reads it
 - PV accumulators, o_proj PSUM and the filler-projection PSUM share
   banks (tags big0/big1 and the pv pool); the tail o_proj accumulates
   in the freed scores banks with a bf16 normalize chain split per
   m-tile to shorten the critical path
"""

import numpy as np
import ml_dtypes

B, S, D, H = 2, 2048, 1024, 16
HD = 64
NCORES = 8
HEADS_PER_CORE = 4
GDIM = HEADS_PER_CORE * HD          # 256 projection cols per core
SB = S // 128                        # 16 s-tiles
KD = D // 128                        # 8 k-tiles over d
QCHUNK = 512
NQC = S // QCHUNK                    # 4 q-chunks
UNIT = 1024                          # scores psum unit (2 banks)

_BF16 = ml_dtypes.bfloat16
_cache = {}


def _build(use_rope: bool):
    import concourse.bass as bass
    import concourse.mybir as mybir
    import concourse.tile as tile
    from concourse import bacc

    F32 = mybir.dt.float32
    BF16 = mybir.dt.bfloat16
    EXP = mybir.ActivationFunctionType.Exp

    nc = bacc.Bacc(None, target_bir_lowering=False)

    xt_d = nc.dram_tensor("xt", [D, S], BF16, kind="ExternalInput")
    wqk_d = nc.dram_tensor("wqk", [D, 2 * GDIM], BF16, kind="ExternalInput")
    wv_d = nc.dram_tensor("wv", [D, GDIM], BF16, kind="ExternalInput")
    wo_d = nc.dram_tensor("wo", [GDIM, D], BF16, kind="ExternalInput")
    cs_d = nc.dram_tensor("csr", [S, 64], BF16, kind="ExternalInput")
    mask_d = nc.dram_tensor("maskT", [128, 128], BF16, kind="ExternalInput")
    # bf16 output halves the output-DMA bytes; the host accumulates the
    # 4 per-batch partials in fp32
    out_d = nc.dram_tensor("out", [S, D], BF16, kind="ExternalOutput")

    with tile.TileContext(nc) as tc:
        with tc.tile_pool(name="big", bufs=1) as big, \
             tc.tile_pool(name="work", bufs=4) as work, \
             tc.tile_pool(name="ropet", bufs=3) as ropet, \
             tc.tile_pool(name="pex", bufs=4) as pex:
            # ---- resident tensors ----
            xt = big.tile([128, KD, S], BF16)
            wqk = big.tile([128, KD, 2 * GDIM], BF16)
            wv = big.tile([128, KD, GDIM], BF16)
            wo = big.tile([128, 2, D], BF16)
            maskT = big.tile([128, 128], BF16)
            csr = big.tile([128, SB, 64], BF16)
            qkt4 = big.tile([128, 4, S], BF16)
            # qkt4 blocks: 0: Qt heads 0-1, 1: Qt heads 2-3, 2: Kt 0-1, 3: Kt 2-3
            vsb = big.tile([128, SB, HEADS_PER_CORE * 65], BF16)
            yt2 = [big.tile([128, S], BF16, tag=f"yt2{i}", name=f"yt2{i}")
                   for i in range(2)]

            # ones-rows of vsb (column 64 of each 65-wide head slot)
            ones_view = vsb[:].rearrange("p m (h c) -> p m h c", h=4)[:, :, :, 64:65]
            nc.vector.memset(ones_view, 1.0)

            # ---- chunked input loads, ordered so x streams ahead of use ----
            xt_r = xt_d.rearrange("(k p) s -> p k s", p=128)
            wqk_r = wqk_d.rearrange("(k p) o -> p k o", p=128)
            nc.sync.dma_start(xt[:, 0:4, 0:128], xt_r[:, 0:4, 0:128])
            nc.sync.dma_start(wqk[:, 0:2, :], wqk_r[:, 0:2, :])
            csr_r = cs_d.rearrange("(m p) f -> p m f", p=128)
            nc.sync.dma_start(csr[:, 0:8, :], csr_r[:, 0:8, :])
            nc.sync.dma_start(wqk[:, 2:4, :], wqk_r[:, 2:4, :])
            nc.sync.dma_start(xt[:, 4:8, 0:128], xt_r[:, 4:8, 0:128])
            nc.sync.dma_start(wqk[:, 4:6, :], wqk_r[:, 4:6, :])
            nc.sync.dma_start(wqk[:, 6:8, :], wqk_r[:, 6:8, :])
            nc.sync.dma_start(xt[:, :, 128:384], xt_r[:, :, 128:384])
            nc.sync.dma_start(xt[:, :, 384:640], xt_r[:, :, 384:640])
            nc.sync.dma_start(wv[:], wv_d.rearrange("(k p) o -> p k o", p=128))
            nc.sync.dma_start(maskT[:], mask_d[:])
            nc.sync.dma_start(xt[:, :, 640:1536], xt_r[:, :, 640:1536])
            nc.sync.dma_start(xt[:, :, 1536:2048], xt_r[:, :, 1536:2048])
            nc.sync.dma_start(wo[:], wo_d.rearrange("(k p) o -> p k o", p=128))

            # shared QK-tile body: projection psum -> bf16 -> rope -> one
            # wide transpose into qkt4. copy_dve picks the cast engine so
            # phase-B filler tiles stay off the exp-loaded ACT queue.
            def emit_rope_transpose(ps, m, copy_dve):
                ms = slice(m * 128, (m + 1) * 128)
                qkr = ropet.tile([128, 2 * GDIM], BF16, tag="qkr")
                if use_rope:
                    qkf = ropet.tile([128, 2 * GDIM], BF16, tag="qkf")
                    if copy_dve:
                        nc.vector.tensor_copy(qkf[:], ps)
                    else:
                        nc.scalar.copy(qkf[:], ps)
                    qv = qkf[:].rearrange("p (g e c) -> p g e c", e=2, c=32)
                    ov = qkr[:].rearrange("p (g e c) -> p g e c", e=2, c=32)
                    E, O = qv[:, :, 0, :], qv[:, :, 1, :]
                    C = csr[:, m, 0:32].unsqueeze(1).broadcast_to((128, 8, 32))
                    Sn = csr[:, m, 32:64].unsqueeze(1).broadcast_to((128, 8, 32))
                    g32 = lambda t: t[:].rearrange("p (g c) -> p g c", c=32)
                    ta = ropet.tile([128, 256], BF16, tag="ta")
                    tb = ropet.tile([128, 256], BF16, tag="tb")
                    nc.vector.tensor_mul(g32(ta), E, C)
                    nc.vector.tensor_mul(g32(tb), O, Sn)
                    nc.vector.tensor_sub(ov[:, :, 0, :], g32(ta), g32(tb))
                    tc_ = ropet.tile([128, 256], BF16, tag="tc")
                    td = ropet.tile([128, 256], BF16, tag="td")
                    nc.vector.tensor_mul(g32(tc_), O, C)
                    nc.vector.tensor_mul(g32(td), E, Sn)
                    nc.vector.tensor_add(ov[:, :, 1, :], g32(tc_), g32(td))
                else:
                    if copy_dve:
                        nc.vector.tensor_copy(qkr[:], ps)
                    else:
                        nc.scalar.copy(qkr[:], ps)
                return nc.sync.dma_start_transpose(qkt4[:, :, ms], qkr[:])

            # ---- phase A: QK projections for the first 8 s-tiles (the
            # rest run inside phase B as PE filler) ----
            act_warm = False
            with tc.tile_pool(name="pa", bufs=2, space="PSUM") as pa:
                for m in range(SB // 2):
                    ms = slice(m * 128, (m + 1) * 128)
                    ps = pa.tile([128, 2 * GDIM], F32, tag="qk")
                    for k in range(KD):
                        nc.tensor.matmul(ps[:], xt[:, k, ms], wqk[:, k, :],
                                         start=(k == 0), stop=(k == KD - 1))
                    emit_rope_transpose(ps[:], m, copy_dve=False)
                    if not act_warm:
                        # preload the Exp table while ACT is idle so the
                        # first phase-B exp doesn't pay the 1.3us load
                        wa = work.tile([1, 1], BF16, tag="wa")
                        nc.scalar.activation(wa[:], wqk[0:1, 0, 0:1],
                                             EXP, scale=0.125)
                        act_warm = True

            # ---- phase B: attention + V projection + interleaved o_proj ----
            with tc.tile_pool(name="sc", bufs=1, space="PSUM") as scp, \
                 tc.tile_pool(name="yb", bufs=1, space="PSUM") as ybp, \
                 tc.tile_pool(name="pv", bufs=2, space="PSUM") as pvp:

                def emit_vproj_tile(m):
                    ms = slice(m * 128, (m + 1) * 128)
                    psv = pvp.tile([128, QCHUNK], F32, tag="v")
                    for k in range(KD):
                        nc.tensor.matmul(psv[:, 0:GDIM], xt[:, k, ms], wv[:, k, :],
                                         start=(k == 0), stop=(k == KD - 1))
                    dst = vsb[:, m, :].rearrange("p (h c) -> p h c", h=4)[:, :, 0:64]
                    nc.scalar.copy(dst, psv[:, 0:GDIM].rearrange("p (h c) -> p h c", h=4))

                def emit_qkproj_tile(m):
                    ms = slice(m * 128, (m + 1) * 128)
                    ps = pvp.tile([128, QCHUNK], F32, tag="v")
                    for k in range(KD):
                        nc.tensor.matmul(ps[:], xt[:, k, ms], wqk[:, k, :],
                                         start=(k == 0), stop=(k == KD - 1))
                    emit_rope_transpose(ps[:], m, copy_dve=True)

                def emit_oproj_tile(m, use_pv=False):
                    ms = slice(m * 128, (m + 1) * 128)
                    so = work.tile([128, 1024], BF16, tag="so")
                    for nb in range(2):
                        if use_pv:
                            # pv-pool accumulator: avoids waiting on the
                            # ytps (big-tag) release behind the normalize
                            po = pvp.tile([128, QCHUNK], F32, tag="v")
                        else:
                            po = ybp.tile([128, QCHUNK], F32, tag=f"big{nb}")
                        for k2 in range(2):
                            nc.tensor.matmul(po[:], yt2[k2][:, ms],
                                             wo[:, k2, nb * 512:(nb + 1) * 512],
                                             start=(k2 == 0), stop=(k2 == 1))
                        # copies alternate DVE/ACT to halve the serial chain
                        if nb == 0:
                            nc.vector.tensor_copy(so[:, 0:512], po[:])
                        else:
                            nc.scalar.copy(so[:, 512:1024], po[:])
                    nc.sync.dma_start(out_d[ms, :], so[:])

                # qc0's diagonal PV needs vsb s-tiles 0-3 immediately;
                # project them first (also keeps PE warm across the
                # phase boundary)
                for mi in range(5):
                    emit_vproj_tile(mi)

                # filler schedule: one PE tile (o_proj of an earlier chunk,
                # V projection, or a deferred QK projection) per unit
                # boundary so ACT's exp stream and PE advance in lockstep.
                # Deadlines: vsb[m] before its chunk's diagonal units, and
                # qkt4[m] transposed before the chunk whose q-range needs it.
                FILLER = {
                    (0, 0, 0): [("qk", 8)], (0, 0, 1): [("v", 5)],
                    (0, 1, 0): [("v", 6)], (0, 1, 1): [("v", 7)],
                    (1, 0, 0): [("qk", 9)], (1, 0, 1): [("qk", 10)],
                    (1, 0, 2): [("v", 8)], (1, 0, 3): [("v", 9)],
                    (1, 1, 0): [("oppv", 2), ("oppv", 3)],
                    (1, 1, 1): [("qk", 11)], (1, 1, 2): [("oppv", 0)],
                    (1, 1, 3): [("oppv", 1)],
                    (2, 0, 0): [("v", 10)], (2, 0, 1): [("v", 11)],
                    (2, 0, 2): [("qk", 12)], (2, 0, 3): [("qk", 13)],
                    (2, 1, 0): [("oppv", 6), ("oppv", 7)],
                    (2, 1, 1): [("qk", 14)], (2, 1, 2): [("qk", 15)],
                    (2, 1, 3): [("oppv", 4)], (2, 1, 4): [("oppv", 5)],
                    (3, 0, 0): [("v", 12)], (3, 0, 1): [("v", 13)],
                    (3, 0, 2): [("v", 14)], (3, 0, 3): [("v", 15)],
                    (3, 1, 0): [("oppv", 10), ("oppv", 11)],
                    (3, 1, 1): [("oppv", 8)], (3, 1, 2): [("oppv", 9)],
                }

                for qc in range(NQC):
                    q0 = qc * QCHUNK
                    # block list: full key blocks then the diagonal in order
                    # 512,384,128,256 so no matmul crosses a 512-col bank
                    order = list(range(4 * qc)) + \
                        [4 * qc, 4 * qc + 1, 4 * qc + 3, 4 * qc + 2]
                    blocks = []
                    for kb in order:
                        r = max(0, kb - 4 * qc)
                        blocks.append((kb, q0 + r * 128, QCHUNK - r * 128))
                    units, cur, cols = [], [], 0
                    for kb, qoff, n in blocks:
                        if cols + n > UNIT:
                            units.append(cur)
                            cur, cols = [], 0
                        cur.append((kb, qoff, n, cols))
                        cols += n
                    units.append(cur)
                    last_kb = units[-1][-1][0]

                    for hp in range(2):
                        ytps = None
                        for ui, unit in enumerate(units):
                            ucols = unit[-1][3] + unit[-1][2]
                            scs = [scp.tile([128, UNIT], F32, tag=f"sc{i}",
                                            name=f"sc{i}") for i in range(2)]
                            for i in range(2):
                                rows = slice(i * 64, i * 64 + 64)
                                for (kb, qoff, n, o) in unit:
                                    nc.tensor.matmul(
                                        scs[i][:, o:o + n],
                                        qkt4[rows, 2 + hp, kb * 128:(kb + 1) * 128],
                                        qkt4[rows, hp, qoff:qoff + n],
                                        start=True, stop=True)
                            # PE filler between scores and PV: previous
                            # chunk's o_proj (must precede this chunk's ytps
                            # allocation — the po tiles reuse the big banks)
                            # and this chunk's V projection, spread across
                            # unit boundaries so ACT's exp stream never
                            # starves. The vsb tiles feed only the diagonal
                            # units, several pipeline steps later.
                            for kind, fm in FILLER.get((qc, hp, ui), ()):
                                if kind == "op":
                                    emit_oproj_tile(fm)
                                elif kind == "oppv":
                                    emit_oproj_tile(fm, use_pv=True)
                                elif kind == "v":
                                    emit_vproj_tile(fm)
                                else:
                                    emit_qkproj_tile(fm)
                            if ytps is None:
                                ytps = [ybp.tile([128, QCHUNK], F32,
                                                 tag=f"big{i}", name=f"big{i}")
                                        for i in range(2)]
                            for i in range(2):
                                h = 2 * hp + i
                                vcol = slice(h * 65, h * 65 + 65)
                                pe = pex.tile([128, UNIT], BF16,
                                              tag=f"pe{i}", name=f"pe{i}")
                                nc.scalar.activation(pe[:, :ucols],
                                                     scs[i][:, :ucols],
                                                     EXP, scale=0.125)
                                for (kb, qoff, n, o) in unit:
                                    if kb >= 4 * qc:  # diagonal: causal mask
                                        nc.vector.tensor_mul(
                                            pe[:, o:o + 128], pe[:, o:o + 128],
                                            maskT[:])
                                    # kb==0 always has n=512: start clears
                                    # the whole [65, QCHUNK] accumulator
                                    nc.tensor.matmul(
                                        ytps[i][0:65, qoff - q0:qoff - q0 + n],
                                        vsb[:, kb, vcol],
                                        pe[:, o:o + n],
                                        start=(kb == 0), stop=(kb == last_kb))
                        if qc == NQC - 1 and hp == 1:
                            # tail: bf16 normalize (denominator precision
                            # cost ~0.4%, well inside the error budget).
                            # ACT casts y to bf16 while DVE does the
                            # reciprocals; the per-m-tile muls then run in
                            # 2x mode and stagger the o_proj k2=1 pairs.
                            # Reading ytps via ybf also releases the big
                            # banks for the tail slots sooner.
                            rcs, ybfs = [], []
                            for i in range(2):
                                rc = work.tile([1, QCHUNK], BF16, tag=f"rcb{i}")
                                with nc.allow_low_precision(
                                        reason="bf16 softmax denominator"):
                                    nc.vector.reciprocal(rc[:],
                                                         ytps[i][64:65, :])
                                rcs.append(rc)
                                ybf = work.tile([64, QCHUNK], BF16,
                                                tag=f"ybf{i}")
                                nc.scalar.copy(ybf[:], ytps[i][0:64, :])
                                ybfs.append(ybf)
                            for mi in range(4):
                                mc = slice(mi * 128, (mi + 1) * 128)
                                for i in range(2):
                                    bc = work.tile([64, 128], BF16, tag="bct")
                                    nc.gpsimd.partition_broadcast(
                                        bc[:], rcs[i][0:1, mc])
                                    nc.vector.tensor_mul(
                                        yt2[hp][i * 64:i * 64 + 64,
                                                q0 + mi * 128:
                                                q0 + (mi + 1) * 128],
                                        ybfs[i][:, mc], bc[:])
                        else:
                            for i in range(2):
                                # normalize straight from PSUM
                                rc = work.tile([1, QCHUNK], F32, tag="rc")
                                nc.vector.reciprocal(rc[:], ytps[i][64:65, :])
                                bc = work.tile([64, QCHUNK], F32, tag="bc")
                                nc.gpsimd.partition_broadcast(bc[:], rc[0:1, :])
                                nc.vector.tensor_mul(
                                    yt2[hp][i * 64:i * 64 + 64, q0:q0 + QCHUNK],
                                    ytps[i][0:64, :], bc[:])
                    if qc == NQC - 1:
                        # tail o_proj with 8 independent PSUM accumulators
                        # (freed scores banks + pv pool + big banks) so no
                        # matmul waits on a copy; all k2=0 matmuls run
                        # during the normalize chains, k2=1 staggers in as
                        # the per-m muls land; copies alternate DVE/ACT
                        slots = []
                        for mi in range(2):
                            pot = scp.tile([128, UNIT], F32,
                                           tag=f"sc{mi}", name=f"sc{mi}")
                            slots.append((pot[:, 0:512], pot[:, 512:1024]))
                        for mi in range(2):
                            a = pvp.tile([128, QCHUNK], F32, tag="v")
                            b = ybp.tile([128, QCHUNK], F32,
                                         tag=f"big{mi}", name=f"big{mi}")
                            slots.append((a[:], b[:]))
                        for k2 in range(2):
                            for mi in range(4):
                                m = qc * 4 + mi
                                ms = slice(m * 128, (m + 1) * 128)
                                for nb in range(2):
                                    nc.tensor.matmul(
                                        slots[mi][nb], yt2[k2][:, ms],
                                        wo[:, k2, nb * 512:(nb + 1) * 512],
                                        start=(k2 == 0), stop=(k2 == 1))
                        for mi in range(4):
                            m = qc * 4 + mi
                            ms = slice(m * 128, (m + 1) * 128)
                            so = work.tile([128, 1024], BF16, tag=f"sot{mi}")
                            for nb in range(2):
                                if (mi + nb) % 2 == 0:
                                    nc.vector.tensor_copy(
                                        so[:, nb * 512:(nb + 1) * 512],
                                        slots[mi][nb])
                                else:
                                    nc.scalar.copy(
                                        so[:, nb * 512:(nb + 1) * 512],
                                        slots[mi][nb])
                            nc.sync.dma_start(out_d[ms, :], so[:])
    nc.compile()
    return nc


def _prep_core_inputs(x, Wq, Wk, Wv, Wo, cos_g, sin_g, use_rope):
    """Host-side shard + layout prep. Returns list of 8 input dicts."""
    maskT = np.tril(np.ones((128, 128), np.float32)).T.astype(_BF16)
    # per-head row permutation: [evens | odds] so rope pairs are
    # (j, j+32) within each head's 64 projection dims
    perm = np.concatenate([np.arange(h * 64, (h + 1) * 64).reshape(32, 2).T.reshape(64)
                           for h in range(H)])
    Wq_p = Wq[perm]
    Wk_p = Wk[perm]
    maps = []
    for c in range(NCORES):
        b, g = divmod(c, HEADS_PER_CORE)
        rows = slice(g * GDIM, (g + 1) * GDIM)
        wqk = np.concatenate([Wq_p[rows], Wk_p[rows]], axis=0).T  # [D, 512]
        maps.append({
            "xt": np.ascontiguousarray(x[b].T).astype(_BF16),
            "wqk": np.ascontiguousarray(wqk).astype(_BF16),
            "wv": np.ascontiguousarray(Wv[rows].T).astype(_BF16),
            "wo": np.ascontiguousarray(Wo[:, rows].T).astype(_BF16),
            "csr": np.concatenate([cos_g, sin_g], axis=1).astype(_BF16),
            "maskT": maskT,
        })
    return maps


def kernel(x, token_positions, use_rope, Wq, Wk, Wv, Wo, cos, sin):
    from concourse.bass_utils import run_bass_kernel_spmd

    x = np.asarray(x, np.float32)
    token_positions = np.asarray(token_positions)
    Wq = np.asarray(Wq, np.float32)
    Wk = np.asarray(Wk, np.float32)
    Wv = np.asarray(Wv, np.float32)
    Wo = np.asarray(Wo, np.float32)
    cos = np.asarray(cos, np.float32)
    sin = np.asarray(sin, np.float32)
    rope = bool(int(use_rope))

    cos_g = cos[token_positions]  # [S, 32]
    sin_g = sin[token_positions]

    if rope not in _cache:
        _cache[rope] = _build(rope)
    nc = _cache[rope]

    in_maps = _prep_core_inputs(x, Wq, Wk, Wv, Wo, cos_g, sin_g, rope)
    res = run_bass_kernel_spmd(nc, in_maps, list(range(NCORES)))

    out = np.zeros((B, S, D), np.float32)
    for c in range(NCORES):
        out[c // HEADS_PER_CORE] += res.results[c]["out"].astype(np.float32)
    return out


# revision 55
# speedup vs baseline: 1.2435x; 1.0112x over previous
"""Causal multi-head attention with RoPE for Trainium2, 8-core SPMD.

Problem: B=2, S=2048, D_MODEL=1024, H=16, HD=64, causal softmax(QK^T/8)V
with interleaved-pair RoPE on q/k, projections Wq/Wk/Wv/Wo.

Sharding (host side): batch x head-group. Core c handles batch b=c//4 and
head group g=c%4 (heads 4g..4g+3, a 256-wide slice of the projection dims).
Each core computes a full [S, D_MODEL] partial of the output (its head
group's contribution through Wo); host sums 4 partials per batch.

Device schedule (all matmuls bf16, fp32 accumulate):
 - host passes x[b].T so the d-contraction sits on SBUF partitions
 - Wq/Wk rows are permuted per head to [evens | odds] so RoPE pairs are
   (col j, col j+32) within each head: the DVE ops run on contiguous
   32-wide groups (packed, 2x mode) and cos/sin tables are the raw
   [S, 32] tables read through stride-0 broadcast views
 - phase A: Q,K projected in [s, o] layout for the first 8 s-tiles only
   (cast to bf16 -> RoPE on DVE -> one wide DMA transpose per s-tile
   into qkt4); the other 8 QK tiles run inside phase B as PE filler
 - phase B: per q-chunk, scores^T[k, q] = Kt.T @ Qt per 128-key block
   (K=64 contraction) into 1024-wide PSUM units; one Exp per unit (ACT),
   causal mask on the diagonal blocks (DVE); PV with lhsT = [V | 1]
   (M=65) so row 64 accumulates the softmax denominator for free;
   normalization reads PSUM directly (reciprocal + gpsimd broadcast +
   DVE mul)
 - the FILLER schedule places one PE tile (deferred QK projection, V
   projection, or the previous chunk's o_proj) at each scores-unit
   boundary so PE and ACT's exp stream advance in lockstep; deadlines:
   vsb[m] before its chunk's diagonal units, qkt4[m] transposed before
   the chunk whose q-range reads it
 - PV accumulators, o_proj PSUM and the filler-projection PSUM share
   banks (tags big0/big1 and the pv pool); the tail o_proj accumulates
   in the freed scores banks with a bf16 normalize chain split per
   m-tile to shorten the critical path
"""

import numpy as np
import ml_dtypes

B, S, D, H = 2, 2048, 1024, 16
HD = 64
NCORES = 8
HEADS_PER_CORE = 4
GDIM = HEADS_PER_CORE * HD          # 256 projection cols per core
SB = S // 128                        # 16 s-tiles
KD = D // 128                        # 8 k-tiles over d
QCHUNK = 512
NQC = S // QCHUNK                    # 4 q-chunks
UNIT = 1024                          # scores psum unit (2 banks)

_BF16 = ml_dtypes.bfloat16
_cache = {}


def _build(use_rope: bool):
    import concourse.bass as bass
    import concourse.mybir as mybir
    import concourse.tile as tile
    from concourse import bacc

    F32 = mybir.dt.float32
    BF16 = mybir.dt.bfloat16
    EXP = mybir.ActivationFunctionType.Exp

    nc = bacc.Bacc(None, target_bir_lowering=False)

    xt_d = nc.dram_tensor("xt", [D, S], BF16, kind="ExternalInput")
    wqk_d = nc.dram_tensor("wqk", [D, 2 * GDIM], BF16, kind="ExternalInput")
    wv_d = nc.dram_tensor("wv", [D, GDIM], BF16, kind="ExternalInput")
    wo_d = nc.dram_tensor("wo", [GDIM, D], BF16, kind="ExternalInput")
    cs_d = nc.dram_tensor("csr", [S, 64], BF16, kind="ExternalInput")
    mask_d = nc.dram_tensor("maskT", [128, 128], BF16, kind="ExternalInput")
    # bf16 output halves the output-DMA bytes; the host accumulates the
    # 4 per-batch partials in fp32
    out_d = nc.dram_tensor("out", [S, D], BF16, kind="ExternalOutput")

    with tile.TileContext(nc) as tc:
        with tc.tile_pool(name="big", bufs=1) as big, \
             tc.tile_pool(name="work", bufs=4) as work, \
             tc.tile_pool(name="ropet", bufs=5) as ropet, \
             tc.tile_pool(name="pex", bufs=6) as pex:
            # ---- resident tensors ----
            xt = big.tile([128, KD, S], BF16)
            wqk = big.tile([128, KD, 2 * GDIM], BF16)
            wv = big.tile([128, KD, GDIM], BF16)
            wo = big.tile([128, 2, D], BF16)
            maskT = big.tile([128, 128], BF16)
            csr = big.tile([128, SB, 64], BF16)
            qkt4 = big.tile([128, 4, S], BF16)
            # qkt4 blocks: 0: Qt heads 0-1, 1: Qt heads 2-3, 2: Kt 0-1, 3: Kt 2-3
            vsb = big.tile([128, SB, HEADS_PER_CORE * 65], BF16)
            yt2 = [big.tile([128, S], BF16, tag=f"yt2{i}", name=f"yt2{i}")
                   for i in range(2)]

            # ones-rows of vsb (column 64 of each 65-wide head slot)
            ones_view = vsb[:].rearrange("p m (h c) -> p m h c", h=4)[:, :, :, 64:65]
            nc.vector.memset(ones_view, 1.0)

            # ---- chunked input loads, ordered so x streams ahead of use ----
            xt_r = xt_d.rearrange("(k p) s -> p k s", p=128)
            wqk_r = wqk_d.rearrange("(k p) o -> p k o", p=128)
            nc.sync.dma_start(xt[:, 0:4, 0:128], xt_r[:, 0:4, 0:128])
            nc.sync.dma_start(wqk[:, 0:2, :], wqk_r[:, 0:2, :])
            csr_r = cs_d.rearrange("(m p) f -> p m f", p=128)
            nc.sync.dma_start(csr[:, 0:8, :], csr_r[:, 0:8, :])
            nc.sync.dma_start(wqk[:, 2:4, :], wqk_r[:, 2:4, :])
            nc.sync.dma_start(xt[:, 4:8, 0:128], xt_r[:, 4:8, 0:128])
            nc.sync.dma_start(wqk[:, 4:6, :], wqk_r[:, 4:6, :])
            nc.sync.dma_start(wqk[:, 6:8, :], wqk_r[:, 6:8, :])
            nc.sync.dma_start(xt[:, :, 128:384], xt_r[:, :, 128:384])
            nc.sync.dma_start(xt[:, :, 384:640], xt_r[:, :, 384:640])
            nc.sync.dma_start(wv[:], wv_d.rearrange("(k p) o -> p k o", p=128))
            nc.sync.dma_start(maskT[:], mask_d[:])
            nc.sync.dma_start(xt[:, :, 640:1536], xt_r[:, :, 640:1536])
            nc.sync.dma_start(xt[:, :, 1536:2048], xt_r[:, :, 1536:2048])
            nc.sync.dma_start(wo[:], wo_d.rearrange("(k p) o -> p k o", p=128))

            # shared QK-tile body: projection psum -> bf16 -> rope -> one
            # wide transpose into qkt4. copy_dve picks the cast engine so
            # phase-B filler tiles stay off the exp-loaded ACT queue.
            def emit_rope_transpose(ps, m, copy_dve):
                ms = slice(m * 128, (m + 1) * 128)
                qkr = ropet.tile([128, 2 * GDIM], BF16, tag="qkr")
                if use_rope:
                    qkf = ropet.tile([128, 2 * GDIM], BF16, tag="qkf")
                    if copy_dve:
                        nc.vector.tensor_copy(qkf[:], ps)
                    else:
                        nc.scalar.copy(qkf[:], ps)
                    qv = qkf[:].rearrange("p (g e c) -> p g e c", e=2, c=32)
                    ov = qkr[:].rearrange("p (g e c) -> p g e c", e=2, c=32)
                    E, O = qv[:, :, 0, :], qv[:, :, 1, :]
                    C = csr[:, m, 0:32].unsqueeze(1).broadcast_to((128, 8, 32))
                    Sn = csr[:, m, 32:64].unsqueeze(1).broadcast_to((128, 8, 32))
                    g32 = lambda t: t[:].rearrange("p (g c) -> p g c", c=32)
                    ta = ropet.tile([128, 256], BF16, tag="ta")
                    tb = ropet.tile([128, 256], BF16, tag="tb")
                    nc.vector.tensor_mul(g32(ta), E, C)
                    nc.vector.tensor_mul(g32(tb), O, Sn)
                    nc.vector.tensor_sub(ov[:, :, 0, :], g32(ta), g32(tb))
                    tc_ = ropet.tile([128, 256], BF16, tag="tc")
                    td = ropet.tile([128, 256], BF16, tag="td")
                    nc.vector.tensor_mul(g32(tc_), O, C)
                    nc.vector.tensor_mul(g32(td), E, Sn)
                    nc.vector.tensor_add(ov[:, :, 1, :], g32(tc_), g32(td))
                else:
                    if copy_dve:
                        nc.vector.tensor_copy(qkr[:], ps)
                    else:
                        nc.scalar.copy(qkr[:], ps)
                return nc.sync.dma_start_transpose(qkt4[:, :, ms], qkr[:])

            # ---- phase A: QK projections for the first 8 s-tiles (the
            # rest run inside phase B as PE filler) ----
            act_warm = False
            with tc.tile_pool(name="pa", bufs=2, space="PSUM") as pa:
                for m in range(SB // 2):
                    ms = slice(m * 128, (m + 1) * 128)
                    ps = pa.tile([128, 2 * GDIM], F32, tag="qk")
                    for k in range(KD):
                        nc.tensor.matmul(ps[:], xt[:, k, ms], wqk[:, k, :],
                                         start=(k == 0), stop=(k == KD - 1))
                    emit_rope_transpose(ps[:], m, copy_dve=False)
                    if not act_warm:
                        # preload the Exp table while ACT is idle so the
                        # first phase-B exp doesn't pay the 1.3us load
                        wa = work.tile([1, 1], BF16, tag="wa")
                        nc.scalar.activation(wa[:], wqk[0:1, 0, 0:1],
                                             EXP, scale=0.125)
                        act_warm = True

            # ---- phase B: attention + V projection + interleaved o_proj ----
            with tc.tile_pool(name="sc", bufs=1, space="PSUM") as scp, \
                 tc.tile_pool(name="yb", bufs=1, space="PSUM") as ybp, \
                 tc.tile_pool(name="pv", bufs=2, space="PSUM") as pvp:

                def emit_vproj_tile(m):
                    ms = slice(m * 128, (m + 1) * 128)
                    psv = pvp.tile([128, QCHUNK], F32, tag="v")
                    for k in range(KD):
                        nc.tensor.matmul(psv[:, 0:GDIM], xt[:, k, ms], wv[:, k, :],
                                         start=(k == 0), stop=(k == KD - 1))
                    dst = vsb[:, m, :].rearrange("p (h c) -> p h c", h=4)[:, :, 0:64]
                    nc.scalar.copy(dst, psv[:, 0:GDIM].rearrange("p (h c) -> p h c", h=4))

                def emit_qkproj_tile(m):
                    ms = slice(m * 128, (m + 1) * 128)
                    ps = pvp.tile([128, QCHUNK], F32, tag="v")
                    for k in range(KD):
                        nc.tensor.matmul(ps[:], xt[:, k, ms], wqk[:, k, :],
                                         start=(k == 0), stop=(k == KD - 1))
                    emit_rope_transpose(ps[:], m, copy_dve=True)

                def emit_oproj_tile(m, use_pv=False):
                    ms = slice(m * 128, (m + 1) * 128)
                    so = work.tile([128, 1024], BF16, tag="so")
                    for nb in range(2):
                        if use_pv:
                            # pv-pool accumulator: avoids waiting on the
                            # ytps (big-tag) release behind the normalize
                            po = pvp.tile([128, QCHUNK], F32, tag="v")
                        else:
                            po = ybp.tile([128, QCHUNK], F32, tag=f"big{nb}")
                        for k2 in range(2):
                            nc.tensor.matmul(po[:], yt2[k2][:, ms],
                                             wo[:, k2, nb * 512:(nb + 1) * 512],
                                             start=(k2 == 0), stop=(k2 == 1))
                        # copies alternate DVE/ACT to halve the serial chain
                        if nb == 0:
                            nc.vector.tensor_copy(so[:, 0:512], po[:])
                        else:
                            nc.scalar.copy(so[:, 512:1024], po[:])
                    nc.sync.dma_start(out_d[ms, :], so[:])

                # qc0's diagonal PV needs vsb s-tiles 0-3 immediately;
                # project them first (also keeps PE warm across the
                # phase boundary)
                for mi in range(5):
                    emit_vproj_tile(mi)
                emit_qkproj_tile(8)
                emit_qkproj_tile(9)

                # filler schedule: one PE tile (o_proj of an earlier chunk,
                # V projection, or a deferred QK projection) per unit
                # boundary so ACT's exp stream and PE advance in lockstep.
                # Deadlines: vsb[m] before its chunk's diagonal units, and
                # qkt4[m] transposed before the chunk whose q-range needs it.
                FILLER = {
                    (0, 0, 0): [("qk", 8)], (0, 0, 1): [("v", 5)],
                    (0, 1, 0): [("v", 6)], (0, 1, 1): [("v", 7)],
                    (1, 0, 0): [("qk", 9)], (1, 0, 1): [("qk", 10)],
                    (1, 0, 2): [("v", 8)], (1, 0, 3): [("v", 9)],
                    (1, 1, 0): [("oppv", 2), ("oppv", 3)],
                    (1, 1, 1): [("qk", 11)], (1, 1, 2): [("oppv", 0)],
                    (1, 1, 3): [("oppv", 1)],
                    (2, 0, 0): [("v", 10)], (2, 0, 1): [("v", 11)],
                    (2, 0, 3): [("qk", 12)], (2, 0, 4): [("qk", 13)],
                    (2, 1, 0): [("oppv", 6), ("oppv", 7)],
                    (2, 1, 1): [("qk", 14)], (2, 1, 2): [("qk", 15)],
                    (2, 1, 3): [("oppv", 4)], (2, 1, 4): [("oppv", 5)],
                    (3, 0, 0): [("v", 12)], (3, 0, 2): [("v", 13)],
                    (3, 0, 4): [("v", 14)], (3, 0, 5): [("v", 15)],
                    (3, 1, 0): [("oppv", 10), ("oppv", 11)],
                    (3, 1, 1): [("oppv", 8)], (3, 1, 2): [("oppv", 9)],
                }

                for qc in range(NQC):
                    q0 = qc * QCHUNK
                    # block list: full key blocks then the diagonal in order
                    # 512,384,128,256 so no matmul crosses a 512-col bank
                    order = list(range(4 * qc)) + \
                        [4 * qc, 4 * qc + 1, 4 * qc + 3, 4 * qc + 2]
                    blocks = []
                    for kb in order:
                        r = max(0, kb - 4 * qc)
                        blocks.append((kb, q0 + r * 128, QCHUNK - r * 128))
                    units, cur, cols = [], [], 0
                    for kb, qoff, n in blocks:
                        if cols + n > UNIT:
                            units.append(cur)
                            cur, cols = [], 0
                        cur.append((kb, qoff, n, cols))
                        cols += n
                    units.append(cur)
                    last_kb = units[-1][-1][0]

                    for hp in range(2):
                        ytps = None
                        for ui, unit in enumerate(units):
                            ucols = unit[-1][3] + unit[-1][2]
                            scs = [scp.tile([128, UNIT], F32, tag=f"sc{i}",
                                            name=f"sc{i}") for i in range(2)]
                            for i in range(2):
                                rows = slice(i * 64, i * 64 + 64)
                                for (kb, qoff, n, o) in unit:
                                    nc.tensor.matmul(
                                        scs[i][:, o:o + n],
                                        qkt4[rows, 2 + hp, kb * 128:(kb + 1) * 128],
                                        qkt4[rows, hp, qoff:qoff + n],
                                        start=True, stop=True)
                            # PE filler between scores and PV: previous
                            # chunk's o_proj (must precede this chunk's ytps
                            # allocation — the po tiles reuse the big banks)
                            # and this chunk's V projection, spread across
                            # unit boundaries so ACT's exp stream never
                            # starves. The vsb tiles feed only the diagonal
                            # units, several pipeline steps later.
                            for kind, fm in FILLER.get((qc, hp, ui), ()):
                                if kind == "op":
                                    emit_oproj_tile(fm)
                                elif kind == "oppv":
                                    emit_oproj_tile(fm, use_pv=True)
                                elif kind == "v":
                                    emit_vproj_tile(fm)
                                else:
                                    emit_qkproj_tile(fm)
                            if ytps is None:
                                ytps = [ybp.tile([128, QCHUNK], F32,
                                                 tag=f"big{i}", name=f"big{i}")
                                        for i in range(2)]
                            for i in range(2):
                                h = 2 * hp + i
                                vcol = slice(h * 65, h * 65 + 65)
                                pe = pex.tile([128, UNIT], BF16,
                                              tag=f"pe{i}", name=f"pe{i}")
                                nc.scalar.activation(pe[:, :ucols],
                                                     scs[i][:, :ucols],
                                                     EXP, scale=0.125)
                                for (kb, qoff, n, o) in unit:
                                    if kb >= 4 * qc:  # diagonal: causal mask
                                        nc.vector.tensor_mul(
                                            pe[:, o:o + 128], pe[:, o:o + 128],
                                            maskT[:])
                                    # kb==0 always has n=512: start clears
                                    # the whole [65, QCHUNK] accumulator
                                    nc.tensor.matmul(
                                        ytps[i][0:65, qoff - q0:qoff - q0 + n],
                                        vsb[:, kb, vcol],
                                        pe[:, o:o + n],
                                        start=(kb == 0), stop=(kb == last_kb))
                        if qc == NQC - 1 and hp == 1:
                            # tail: bf16 normalize (denominator precision
                            # cost ~0.4%, well inside the error budget).
                            # ACT casts y to bf16 while DVE does the
                            # reciprocals; the per-m-tile muls then run in
                            # 2x mode and stagger the o_proj k2=1 pairs.
                            # Reading ytps via ybf also releases the big
                            # banks for the tail slots sooner.
                            rcs, ybfs = [], []
                            for i in range(2):
                                rc = work.tile([1, QCHUNK], BF16, tag=f"rcb{i}")
                                with nc.allow_low_precision(
                                        reason="bf16 softmax denominator"):
                                    nc.vector.reciprocal(rc[:],
                                                         ytps[i][64:65, :])
                                rcs.append(rc)
                                ybf = work.tile([64, QCHUNK], BF16,
                                                tag=f"ybf{i}")
                                nc.scalar.copy(ybf[:], ytps[i][0:64, :])
                                ybfs.append(ybf)
                            for mi in range(4):
                                mc = slice(mi * 128, (mi + 1) * 128)
                                for i in range(2):
                                    bc = work.tile([64, 128], BF16, tag="bct")
                                    nc.gpsimd.partition_broadcast(
                                        bc[:], rcs[i][0:1, mc])
                                    nc.vector.tensor_mul(
                                        yt2[hp][i * 64:i * 64 + 64,
                                                q0 + mi * 128:
                                                q0 + (mi + 1) * 128],
                                        ybfs[i][:, mc], bc[:])
                        else:
                            for i in range(2):
                                # normalize straight from PSUM
                                rc = work.tile([1, QCHUNK], F32, tag="rc")
                                nc.vector.reciprocal(rc[:], ytps[i][64:65, :])
                                bc = work.tile([64, QCHUNK], F32, tag="bc")
                                nc.gpsimd.partition_broadcast(bc[:], rc[0:1, :])
                                nc.vector.tensor_mul(
                                    yt2[hp][i * 64:i * 64 + 64, q0:q0 + QCHUNK],
                                    ytps[i][0:64, :], bc[:])
                    if qc == NQC - 1:
                        # tail o_proj with 8 independent PSUM accumulators
                        # (freed scores banks + pv pool + big banks) so no
                        # matmul waits on a copy; all k2=0 matmuls run
                        # during the normalize chains, k2=1 staggers in as
                        # the per-m muls land; copies alternate DVE/ACT
                        slots = []
                        for mi in range(2):
                            pot = scp.tile([128, UNIT], F32,
                                           tag=f"sc{mi}", name=f"sc{mi}")
                            slots.append((pot[:, 0:512], pot[:, 512:1024]))
                        for mi in range(2):
                            a = pvp.tile([128, QCHUNK], F32, tag="v")
                            b = ybp.tile([128, QCHUNK], F32,
                                         tag=f"big{mi}", name=f"big{mi}")
                            slots.append((a[:], b[:]))
                        for k2 in range(2):
                            for mi in range(4):
                                m = qc * 4 + mi
                                ms = slice(m * 128, (m + 1) * 128)
                                for nb in range(2):
                                    nc.tensor.matmul(
                                        slots[mi][nb], yt2[k2][:, ms],
                                        wo[:, k2, nb * 512:(nb + 1) * 512],
                                        start=(k2 == 0), stop=(k2 == 1))
                        for mi in range(4):
                            m = qc * 4 + mi
                            ms = slice(m * 128, (m + 1) * 128)
                            so = work.tile([128, 1024], BF16, tag=f"sot{mi}")
                            for nb in range(2):
                                if (mi + nb) % 2 == 0:
                                    nc.vector.tensor_copy(
                                        so[:, nb * 512:(nb + 1) * 512],
                                        slots[mi][nb])
                                else:
                                    nc.scalar.copy(
                                        so[:, nb * 512:(nb + 1) * 512],
                                        slots[mi][nb])
                            nc.sync.dma_start(out_d[ms, :], so[:])
    nc.compile()
    return nc


def _prep_core_inputs(x, Wq, Wk, Wv, Wo, cos_g, sin_g, use_rope):
    """Host-side shard + layout prep. Returns list of 8 input dicts."""
    maskT = np.tril(np.ones((128, 128), np.float32)).T.astype(_BF16)
    # per-head row permutation: [evens | odds] so rope pairs are
    # (j, j+32) within each head's 64 projection dims
    perm = np.concatenate([np.arange(h * 64, (h + 1) * 64).reshape(32, 2).T.reshape(64)
                           for h in range(H)])
    Wq_p = Wq[perm]
    Wk_p = Wk[perm]
    maps = []
    for c in range(NCORES):
        b, g = divmod(c, HEADS_PER_CORE)
        rows = slice(g * GDIM, (g + 1) * GDIM)
        wqk = np.concatenate([Wq_p[rows], Wk_p[rows]], axis=0).T  # [D, 512]
        maps.append({
            "xt": np.ascontiguousarray(x[b].T).astype(_BF16),
            "wqk": np.ascontiguousarray(wqk).astype(_BF16),
            "wv": np.ascontiguousarray(Wv[rows].T).astype(_BF16),
            "wo": np.ascontiguousarray(Wo[:, rows].T).astype(_BF16),
            "csr": np.concatenate([cos_g, sin_g], axis=1).astype(_BF16),
            "maskT": maskT,
        })
    return maps


def kernel(x, token_positions, use_rope, Wq, Wk, Wv, Wo, cos, sin):
    from concourse.bass_utils import run_bass_kernel_spmd

    x = np.asarray(x, np.float32)
    token_positions = np.asarray(token_positions)
    Wq = np.asarray(Wq, np.float32)
    Wk = np.asarray(Wk, np.float32)
    Wv = np.asarray(Wv, np.float32)
    Wo = np.asarray(Wo, np.float32)
    cos = np.asarray(cos, np.float32)
    sin = np.asarray(sin, np.float32)
    rope = bool(int(use_rope))

    cos_g = cos[token_positions]  # [S, 32]
    sin_g = sin[token_positions]

    if rope not in _cache:
        _cache[rope] = _build(rope)
    nc = _cache[rope]

    in_maps = _prep_core_inputs(x, Wq, Wk, Wv, Wo, cos_g, sin_g, rope)
    res = run_bass_kernel_spmd(nc, in_maps, list(range(NCORES)))

    out = np.zeros((B, S, D), np.float32)
    for c in range(NCORES):
        out[c // HEADS_PER_CORE] += res.results[c]["out"].astype(np.float32)
    return out


# revision 56
# speedup vs baseline: 1.2504x; 1.0055x over previous
"""Causal multi-head attention with RoPE for Trainium2, 8-core SPMD.

Problem: B=2, S=2048, D_MODEL=1024, H=16, HD=64, causal softmax(QK^T/8)V
with interleaved-pair RoPE on q/k, projections Wq/Wk/Wv/Wo.

Sharding (host side): batch x head-group. Core c handles batch b=c//4 and
head group g=c%4 (heads 4g..4g+3, a 256-wide slice of the projection dims).
Each core computes a full [S, D_MODEL] partial of the output (its head
group's contribution through Wo); host sums 4 partials per batch.

Device schedule (all matmuls bf16, fp32 accumulate):
 - host passes x[b].T so the d-contraction sits on SBUF partitions
 - Wq/Wk rows are permuted per head to [evens | odds] so RoPE pairs are
   (col j, col j+32) within each head: the DVE ops run on contiguous
   32-wide groups (packed, 2x mode) and cos/sin tables are the raw
   [S, 32] tables read through stride-0 broadcast views
 - phase A: Q,K projected in [s, o] layout for the first 8 s-tiles only
   (cast to bf16 -> RoPE on DVE -> one wide DMA transpose per s-tile
   into qkt4); the other 8 QK tiles run inside phase B as PE filler
 - phase B: per q-chunk, scores^T[k, q] = Kt.T @ Qt per 128-key block
   (K=64 contraction) into 1024-wide PSUM units; one Exp per unit (ACT),
   causal mask on the diagonal blocks (DVE); PV with lhsT = [V | 1]
   (M=65) so row 64 accumulates the softmax denominator for free;
   normalization reads PSUM directly (reciprocal + gpsimd broadcast +
   DVE mul)
 - the FILLER schedule places one PE tile (deferred QK projection, V
   projection, or the previous chunk's o_proj) at each scores-unit
   boundary so PE and ACT's exp stream advance in lockstep; deadlines:
   vsb[m] before its chunk's diagonal units, qkt4[m] transposed before
   the chunk whose q-range reads it
 - PV accumulators, o_proj PSUM and the filler-projection PSUM share
   banks (tags big0/big1 and the pv pool); the tail o_proj accumulates
   in the freed scores banks with a bf16 normalize chain split per
   m-tile to shorten the critical path
"""

import numpy as np
import ml_dtypes

B, S, D, H = 2, 2048, 1024, 16
HD = 64
NCORES = 8
HEADS_PER_CORE = 4
GDIM = HEADS_PER_CORE * HD          # 256 projection cols per core
SB = S // 128                        # 16 s-tiles
KD = D // 128                        # 8 k-tiles over d
QCHUNK = 512
NQC = S // QCHUNK                    # 4 q-chunks
UNIT = 1024                          # scores psum unit (2 banks)

_BF16 = ml_dtypes.bfloat16
_cache = {}


def _build(use_rope: bool):
    import concourse.bass as bass
    import concourse.mybir as mybir
    import concourse.tile as tile
    from concourse import bacc

    F32 = mybir.dt.float32
    BF16 = mybir.dt.bfloat16
    EXP = mybir.ActivationFunctionType.Exp

    nc = bacc.Bacc(None, target_bir_lowering=False)

    xt_d = nc.dram_tensor("xt", [D, S], BF16, kind="ExternalInput")
    wqk_d = nc.dram_tensor("wqk", [D, 2 * GDIM], BF16, kind="ExternalInput")
    wv_d = nc.dram_tensor("wv", [D, GDIM], BF16, kind="ExternalInput")
    wo_d = nc.dram_tensor("wo", [GDIM, D], BF16, kind="ExternalInput")
    cs_d = nc.dram_tensor("csr", [S, 64], BF16, kind="ExternalInput")
    mask_d = nc.dram_tensor("maskT", [128, 128], BF16, kind="ExternalInput")
    # bf16 output halves the output-DMA bytes; the host accumulates the
    # 4 per-batch partials in fp32
    out_d = nc.dram_tensor("out", [S, D], BF16, kind="ExternalOutput")

    with tile.TileContext(nc) as tc:
        with tc.tile_pool(name="big", bufs=1) as big, \
             tc.tile_pool(name="work", bufs=4) as work, \
             tc.tile_pool(name="ropet", bufs=5) as ropet, \
             tc.tile_pool(name="pex", bufs=6) as pex:
            # ---- resident tensors ----
            xt = big.tile([128, KD, S], BF16)
            wqk = big.tile([128, KD, 2 * GDIM], BF16)
            wv = big.tile([128, KD, GDIM], BF16)
            wo = big.tile([128, 2, D], BF16)
            maskT = big.tile([128, 128], BF16)
            csr = big.tile([128, SB, 64], BF16)
            qkt4 = big.tile([128, 4, S], BF16)
            # qkt4 blocks: 0: Qt heads 0-1, 1: Qt heads 2-3, 2: Kt 0-1, 3: Kt 2-3
            vsb = big.tile([128, SB, HEADS_PER_CORE * 65], BF16)
            yt2 = [big.tile([128, S], BF16, tag=f"yt2{i}", name=f"yt2{i}")
                   for i in range(2)]

            # ones-rows of vsb (column 64 of each 65-wide head slot)
            ones_view = vsb[:].rearrange("p m (h c) -> p m h c", h=4)[:, :, :, 64:65]
            nc.vector.memset(ones_view, 1.0)

            # ---- chunked input loads, ordered so x streams ahead of use ----
            xt_r = xt_d.rearrange("(k p) s -> p k s", p=128)
            wqk_r = wqk_d.rearrange("(k p) o -> p k o", p=128)
            nc.sync.dma_start(xt[:, 0:4, 0:128], xt_r[:, 0:4, 0:128])
            nc.sync.dma_start(wqk[:, 0:2, :], wqk_r[:, 0:2, :])
            csr_r = cs_d.rearrange("(m p) f -> p m f", p=128)
            nc.sync.dma_start(csr[:, 0:8, :], csr_r[:, 0:8, :])
            nc.sync.dma_start(wqk[:, 2:4, :], wqk_r[:, 2:4, :])
            nc.sync.dma_start(xt[:, 4:8, 0:128], xt_r[:, 4:8, 0:128])
            nc.sync.dma_start(wqk[:, 4:6, :], wqk_r[:, 4:6, :])
            nc.sync.dma_start(wqk[:, 6:8, :], wqk_r[:, 6:8, :])
            nc.sync.dma_start(xt[:, :, 128:384], xt_r[:, :, 128:384])
            nc.sync.dma_start(xt[:, :, 384:640], xt_r[:, :, 384:640])
            nc.sync.dma_start(wv[:], wv_d.rearrange("(k p) o -> p k o", p=128))
            nc.sync.dma_start(maskT[:], mask_d[:])
            nc.sync.dma_start(xt[:, :, 640:1536], xt_r[:, :, 640:1536])
            nc.sync.dma_start(xt[:, :, 1536:2048], xt_r[:, :, 1536:2048])
            nc.sync.dma_start(wo[:], wo_d.rearrange("(k p) o -> p k o", p=128))

            # shared QK-tile body: projection psum -> bf16 -> rope -> one
            # wide transpose into qkt4. copy_dve picks the cast engine so
            # phase-B filler tiles stay off the exp-loaded ACT queue.
            def emit_rope_transpose(ps, m, copy_dve):
                ms = slice(m * 128, (m + 1) * 128)
                qkr = ropet.tile([128, 2 * GDIM], BF16, tag="qkr")
                if use_rope:
                    qkf = ropet.tile([128, 2 * GDIM], BF16, tag="qkf")
                    if copy_dve:
                        nc.vector.tensor_copy(qkf[:], ps)
                    else:
                        nc.scalar.copy(qkf[:], ps)
                    qv = qkf[:].rearrange("p (g e c) -> p g e c", e=2, c=32)
                    ov = qkr[:].rearrange("p (g e c) -> p g e c", e=2, c=32)
                    E, O = qv[:, :, 0, :], qv[:, :, 1, :]
                    C = csr[:, m, 0:32].unsqueeze(1).broadcast_to((128, 8, 32))
                    Sn = csr[:, m, 32:64].unsqueeze(1).broadcast_to((128, 8, 32))
                    g32 = lambda t: t[:].rearrange("p (g c) -> p g c", c=32)
                    ta = ropet.tile([128, 256], BF16, tag="ta")
                    tb = ropet.tile([128, 256], BF16, tag="tb")
                    nc.vector.tensor_mul(g32(ta), E, C)
                    nc.vector.tensor_mul(g32(tb), O, Sn)
                    nc.vector.tensor_sub(ov[:, :, 0, :], g32(ta), g32(tb))
                    tc_ = ropet.tile([128, 256], BF16, tag="tc")
                    td = ropet.tile([128, 256], BF16, tag="td")
                    nc.vector.tensor_mul(g32(tc_), O, C)
                    nc.vector.tensor_mul(g32(td), E, Sn)
                    nc.vector.tensor_add(ov[:, :, 1, :], g32(tc_), g32(td))
                else:
                    if copy_dve:
                        nc.vector.tensor_copy(qkr[:], ps)
                    else:
                        nc.scalar.copy(qkr[:], ps)
                return nc.sync.dma_start_transpose(qkt4[:, :, ms], qkr[:])

            # ---- phase A: QK projections for the first 8 s-tiles (the
            # rest run inside phase B as PE filler) ----
            act_warm = False
            with tc.tile_pool(name="pa", bufs=2, space="PSUM") as pa:
                for m in range(SB // 2):
                    ms = slice(m * 128, (m + 1) * 128)
                    ps = pa.tile([128, 2 * GDIM], F32, tag="qk")
                    for k in range(KD):
                        nc.tensor.matmul(ps[:], xt[:, k, ms], wqk[:, k, :],
                                         start=(k == 0), stop=(k == KD - 1))
                    emit_rope_transpose(ps[:], m, copy_dve=False)
                    if not act_warm:
                        # preload the Exp table while ACT is idle so the
                        # first phase-B exp doesn't pay the 1.3us load
                        wa = work.tile([1, 1], BF16, tag="wa")
                        nc.scalar.activation(wa[:], wqk[0:1, 0, 0:1],
                                             EXP, scale=0.125)
                        act_warm = True

            # ---- phase B: attention + V projection + interleaved o_proj ----
            with tc.tile_pool(name="sc", bufs=1, space="PSUM") as scp, \
                 tc.tile_pool(name="yb", bufs=1, space="PSUM") as ybp, \
                 tc.tile_pool(name="pv", bufs=2, space="PSUM") as pvp:

                def emit_vproj_tile(m):
                    ms = slice(m * 128, (m + 1) * 128)
                    psv = pvp.tile([128, QCHUNK], F32, tag="v")
                    for k in range(KD):
                        nc.tensor.matmul(psv[:, 0:GDIM], xt[:, k, ms], wv[:, k, :],
                                         start=(k == 0), stop=(k == KD - 1))
                    dst = vsb[:, m, :].rearrange("p (h c) -> p h c", h=4)[:, :, 0:64]
                    nc.scalar.copy(dst, psv[:, 0:GDIM].rearrange("p (h c) -> p h c", h=4))

                def emit_qkproj_tile(m):
                    ms = slice(m * 128, (m + 1) * 128)
                    ps = pvp.tile([128, QCHUNK], F32, tag="v")
                    for k in range(KD):
                        nc.tensor.matmul(ps[:], xt[:, k, ms], wqk[:, k, :],
                                         start=(k == 0), stop=(k == KD - 1))
                    emit_rope_transpose(ps[:], m, copy_dve=True)

                def emit_oproj_tile(m, use_pv=False):
                    ms = slice(m * 128, (m + 1) * 128)
                    so = work.tile([128, 1024], BF16, tag="so")
                    for nb in range(2):
                        if use_pv:
                            # pv-pool accumulator: avoids waiting on the
                            # ytps (big-tag) release behind the normalize
                            po = pvp.tile([128, QCHUNK], F32, tag="v")
                        else:
                            po = ybp.tile([128, QCHUNK], F32, tag=f"big{nb}")
                        for k2 in range(2):
                            nc.tensor.matmul(po[:], yt2[k2][:, ms],
                                             wo[:, k2, nb * 512:(nb + 1) * 512],
                                             start=(k2 == 0), stop=(k2 == 1))
                        # copies alternate DVE/ACT to halve the serial chain
                        if nb == 0:
                            nc.vector.tensor_copy(so[:, 0:512], po[:])
                        else:
                            nc.scalar.copy(so[:, 512:1024], po[:])
                    nc.sync.dma_start(out_d[ms, :], so[:])

                # qc0's diagonal PV needs vsb s-tiles 0-3 immediately;
                # project them first (also keeps PE warm across the
                # phase boundary)
                for mi in range(5):
                    emit_vproj_tile(mi)
                emit_qkproj_tile(8)
                emit_qkproj_tile(9)

                # filler schedule: one PE tile (o_proj of an earlier chunk,
                # V projection, or a deferred QK projection) per unit
                # boundary so ACT's exp stream and PE advance in lockstep.
                # Deadlines: vsb[m] before its chunk's diagonal units, and
                # qkt4[m] transposed before the chunk whose q-range needs it.
                FILLER = {
                    (0, 0, 0): [("qk", 8)], (0, 0, 1): [("v", 5)],
                    (0, 1, 0): [("v", 6)], (0, 1, 1): [("v", 7)],
                    (1, 0, 0): [("qk", 9)], (1, 0, 1): [("qk", 10)],
                    (1, 0, 2): [("v", 8)], (1, 0, 3): [("v", 9)],
                    (1, 1, 0): [("oppv", 2)], (1, 1, 1): [("qk", 11)],
                    (1, 1, 2): [("oppv", 3)], (1, 1, 3): [("oppv", 0),
                                                          ("oppv", 1)],
                    (2, 0, 0): [("v", 10)], (2, 0, 1): [("v", 11)],
                    (2, 0, 3): [("qk", 12)], (2, 0, 4): [("qk", 13)],
                    (2, 1, 0): [("oppv", 6)], (2, 1, 1): [("qk", 14)],
                    (2, 1, 2): [("qk", 15)], (2, 1, 3): [("oppv", 7)],
                    (2, 1, 4): [("oppv", 4)], (2, 1, 5): [("oppv", 5)],
                    (3, 0, 0): [("v", 12)], (3, 0, 2): [("v", 13)],
                    (3, 0, 4): [("v", 14)], (3, 0, 5): [("v", 15)],
                    (3, 1, 0): [("oppv", 10)], (3, 1, 1): [("oppv", 11)],
                    (3, 1, 2): [("oppv", 8)], (3, 1, 4): [("oppv", 9)],
                }

                for qc in range(NQC):
                    q0 = qc * QCHUNK
                    # block list: full key blocks then the diagonal in order
                    # 512,384,128,256 so no matmul crosses a 512-col bank
                    order = list(range(4 * qc)) + \
                        [4 * qc, 4 * qc + 1, 4 * qc + 3, 4 * qc + 2]
                    blocks = []
                    for kb in order:
                        r = max(0, kb - 4 * qc)
                        blocks.append((kb, q0 + r * 128, QCHUNK - r * 128))
                    units, cur, cols = [], [], 0
                    for kb, qoff, n in blocks:
                        if cols + n > UNIT:
                            units.append(cur)
                            cur, cols = [], 0
                        cur.append((kb, qoff, n, cols))
                        cols += n
                    units.append(cur)
                    last_kb = units[-1][-1][0]

                    for hp in range(2):
                        ytps = None
                        for ui, unit in enumerate(units):
                            ucols = unit[-1][3] + unit[-1][2]
                            scs = [scp.tile([128, UNIT], F32, tag=f"sc{i}",
                                            name=f"sc{i}") for i in range(2)]
                            for i in range(2):
                                rows = slice(i * 64, i * 64 + 64)
                                for (kb, qoff, n, o) in unit:
                                    nc.tensor.matmul(
                                        scs[i][:, o:o + n],
                                        qkt4[rows, 2 + hp, kb * 128:(kb + 1) * 128],
                                        qkt4[rows, hp, qoff:qoff + n],
                                        start=True, stop=True)
                            # PE filler between scores and PV: previous
                            # chunk's o_proj (must precede this chunk's ytps
                            # allocation — the po tiles reuse the big banks)
                            # and this chunk's V projection, spread across
                            # unit boundaries so ACT's exp stream never
                            # starves. The vsb tiles feed only the diagonal
                            # units, several pipeline steps later.
                            for kind, fm in FILLER.get((qc, hp, ui), ()):
                                if kind == "op":
                                    emit_oproj_tile(fm)
                                elif kind == "oppv":
                                    emit_oproj_tile(fm, use_pv=True)
                                elif kind == "v":
                                    emit_vproj_tile(fm)
                                else:
                                    emit_qkproj_tile(fm)
                            if ytps is None:
                                ytps = [ybp.tile([128, QCHUNK], F32,
                                                 tag=f"big{i}", name=f"big{i}")
                                        for i in range(2)]
                            for i in range(2):
                                h = 2 * hp + i
                                vcol = slice(h * 65, h * 65 + 65)
                                pe = pex.tile([128, UNIT], BF16,
                                              tag=f"pe{i}", name=f"pe{i}")
                                nc.scalar.activation(pe[:, :ucols],
                                                     scs[i][:, :ucols],
                                                     EXP, scale=0.125)
                                for (kb, qoff, n, o) in unit:
                                    if kb >= 4 * qc:  # diagonal: causal mask
                                        nc.vector.tensor_mul(
                                            pe[:, o:o + 128], pe[:, o:o + 128],
                                            maskT[:])
                                    # kb==0 always has n=512: start clears
                                    # the whole [65, QCHUNK] accumulator
                                    nc.tensor.matmul(
                                        ytps[i][0:65, qoff - q0:qoff - q0 + n],
                                        vsb[:, kb, vcol],
                                        pe[:, o:o + n],
                                        start=(kb == 0), stop=(kb == last_kb))
                        if qc == NQC - 1 and hp == 1:
                            # tail: bf16 normalize (denominator precision
                            # cost ~0.4%, well inside the error budget).
                            # ACT casts y to bf16 while DVE does the
                            # reciprocals; the per-m-tile muls then run in
                            # 2x mode and stagger the o_proj k2=1 pairs.
                            # Reading ytps via ybf also releases the big
                            # banks for the tail slots sooner.
                            rcs, ybfs = [], []
                            for i in range(2):
                                rc = work.tile([1, QCHUNK], BF16, tag=f"rcb{i}")
                                with nc.allow_low_precision(
                                        reason="bf16 softmax denominator"):
                                    nc.vector.reciprocal(rc[:],
                                                         ytps[i][64:65, :])
                                rcs.append(rc)
                                ybf = work.tile([64, QCHUNK], BF16,
                                                tag=f"ybf{i}")
                                nc.scalar.copy(ybf[:], ytps[i][0:64, :])
                                ybfs.append(ybf)
                            for mi in range(4):
                                mc = slice(mi * 128, (mi + 1) * 128)
                                for i in range(2):
                                    bc = work.tile([64, 128], BF16, tag="bct")
                                    nc.gpsimd.partition_broadcast(
                                        bc[:], rcs[i][0:1, mc])
                                    nc.vector.tensor_mul(
                                        yt2[hp][i * 64:i * 64 + 64,
                                                q0 + mi * 128:
                                                q0 + (mi + 1) * 128],
                                        ybfs[i][:, mc], bc[:])
                        else:
                            for i in range(2):
                                # normalize straight from PSUM
                                rc = work.tile([1, QCHUNK], F32, tag="rc")
                                nc.vector.reciprocal(rc[:], ytps[i][64:65, :])
                                bc = work.tile([64, QCHUNK], F32, tag="bc")
                                nc.gpsimd.partition_broadcast(bc[:], rc[0:1, :])
                                nc.vector.tensor_mul(
                                    yt2[hp][i * 64:i * 64 + 64, q0:q0 + QCHUNK],
                                    ytps[i][0:64, :], bc[:])
                    if qc == NQC - 1:
                        # tail o_proj with 8 independent PSUM accumulators
                        # (freed scores banks + pv pool + big banks) so no
                        # matmul waits on a copy; all k2=0 matmuls run
                        # during the normalize chains, k2=1 staggers in as
                        # the per-m muls land; copies alternate DVE/ACT
                        slots = []
                        for mi in range(2):
                            pot = scp.tile([128, UNIT], F32,
                                           tag=f"sc{mi}", name=f"sc{mi}")
                            slots.append((pot[:, 0:512], pot[:, 512:1024]))
                        for mi in range(2):
                            a = pvp.tile([128, QCHUNK], F32, tag="v")
                            b = ybp.tile([128, QCHUNK], F32,
                                         tag=f"big{mi}", name=f"big{mi}")
                            slots.append((a[:], b[:]))
                        for k2 in range(2):
                            for mi in range(4):
                                m = qc * 4 + mi
                                ms = slice(m * 128, (m + 1) * 128)
                                for nb in range(2):
                                    nc.tensor.matmul(
                                        slots[mi][nb], yt2[k2][:, ms],
                                        wo[:, k2, nb * 512:(nb + 1) * 512],
                                        start=(k2 == 0), stop=(k2 == 1))
                        for mi in range(4):
                            m = qc * 4 + mi
                            ms = slice(m * 128, (m + 1) * 128)
                            so = work.tile([128, 1024], BF16, tag=f"sot{mi}")
                            for nb in range(2):
                                if (mi + nb) % 2 == 0:
                                    nc.vector.tensor_copy(
                                        so[:, nb * 512:(nb + 1) * 512],
                                        slots[mi][nb])
                                else:
                                    nc.scalar.copy(
                                        so[:, nb * 512:(nb + 1) * 512],
                                        slots[mi][nb])
                            nc.sync.dma_start(out_d[ms, :], so[:])
    nc.compile()
    return nc


def _prep_core_inputs(x, Wq, Wk, Wv, Wo, cos_g, sin_g, use_rope):
    """Host-side shard + layout prep. Returns list of 8 input dicts."""
    maskT = np.tril(np.ones((128, 128), np.float32)).T.astype(_BF16)
    # per-head row permutation: [evens | odds] so rope pairs are
    # (j, j+32) within each head's 64 projection dims
    perm = np.concatenate([np.arange(h * 64, (h + 1) * 64).reshape(32, 2).T.reshape(64)
                           for h in range(H)])
    Wq_p = Wq[perm]
    Wk_p = Wk[perm]
    maps = []
    for c in range(NCORES):
        b, g = divmod(c, HEADS_PER_CORE)
        rows = slice(g * GDIM, (g + 1) * GDIM)
        wqk = np.concatenate([Wq_p[rows], Wk_p[rows]], axis=0).T  # [D, 512]
        maps.append({
            "xt": np.ascontiguousarray(x[b].T).astype(_BF16),
            "wqk": np.ascontiguousarray(wqk).astype(_BF16),
            "wv": np.ascontiguousarray(Wv[rows].T).astype(_BF16),
            "wo": np.ascontiguousarray(Wo[:, rows].T).astype(_BF16),
            "csr": np.concatenate([cos_g, sin_g], axis=1).astype(_BF16),
            "maskT": maskT,
        })
    return maps


def kernel(x, token_positions, use_rope, Wq, Wk, Wv, Wo, cos, sin):
    from concourse.bass_utils import run_bass_kernel_spmd

    x = np.asarray(x, np.float32)
    token_positions = np.asarray(token_positions)
    Wq = np.asarray(Wq, np.float32)
    Wk = np.asarray(Wk, np.float32)
    Wv = np.asarray(Wv, np.float32)
    Wo = np.asarray(Wo, np.float32)
    cos = np.asarray(cos, np.float32)
    sin = np.asarray(sin, np.float32)
    rope = bool(int(use_rope))

    cos_g = cos[token_positions]  # [S, 32]
    sin_g = sin[token_positions]

    if rope not in _cache:
        _cache[rope] = _build(rope)
    nc = _cache[rope]

    in_maps = _prep_core_inputs(x, Wq, Wk, Wv, Wo, cos_g, sin_g, rope)
    res = run_bass_kernel_spmd(nc, in_maps, list(range(NCORES)))

    out = np.zeros((B, S, D), np.float32)
    for c in range(NCORES):
        out[c // HEADS_PER_CORE] += res.results[c]["out"].astype(np.float32)
    return out


# revision 57
# speedup vs baseline: 1.2554x; 1.0040x over previous
"""Causal multi-head attention with RoPE for Trainium2, 8-core SPMD.

Problem: B=2, S=2048, D_MODEL=1024, H=16, HD=64, causal softmax(QK^T/8)V
with interleaved-pair RoPE on q/k, projections Wq/Wk/Wv/Wo.

Sharding (host side): batch x head-group. Core c handles batch b=c//4 and
head group g=c%4 (heads 4g..4g+3, a 256-wide slice of the projection dims).
Each core computes a full [S, D_MODEL] partial of the output (its head
group's contribution through Wo); host sums 4 partials per batch.

Device schedule (all matmuls bf16, fp32 accumulate):
 - host passes x[b].T so the d-contraction sits on SBUF partitions
 - Wq/Wk rows are permuted per head to [evens | odds] so RoPE pairs are
   (col j, col j+32) within each head: the DVE ops run on contiguous
   32-wide groups (packed, 2x mode) and cos/sin tables are the raw
   [S, 32] tables read through stride-0 broadcast views
 - phase A: Q,K projected in [s, o] layout for the first 8 s-tiles only
   (cast to bf16 -> RoPE on DVE -> one wide DMA transpose per s-tile
   into qkt4); the other 8 QK tiles run inside phase B as PE filler
 - phase B: per q-chunk, scores^T[k, q] = Kt.T @ Qt per 128-key block
   (K=64 contraction) into 1024-wide PSUM units; one Exp per unit (ACT),
   causal mask on the diagonal blocks (DVE); PV with lhsT = [V | 1]
   (M=65) so row 64 accumulates the softmax denominator for free;
   normalization reads PSUM directly (reciprocal + gpsimd broadcast +
   DVE mul)
 - the FILLER schedule places one PE tile (deferred QK projection, V
   projection, or the previous chunk's o_proj) at each scores-unit
   boundary so PE and ACT's exp stream advance in lockstep; deadlines:
   vsb[m] before its chunk's diagonal units, qkt4[m] transposed before
   the chunk whose q-range reads it
 - PV accumulators, o_proj PSUM and the filler-projection PSUM share
   banks (tags big0/big1 and the pv pool); the tail o_proj accumulates
   in the freed scores banks with a bf16 normalize chain split per
   m-tile to shorten the critical path
"""

import numpy as np
import ml_dtypes

B, S, D, H = 2, 2048, 1024, 16
HD = 64
NCORES = 8
HEADS_PER_CORE = 4
GDIM = HEADS_PER_CORE * HD          # 256 projection cols per core
SB = S // 128                        # 16 s-tiles
KD = D // 128                        # 8 k-tiles over d
QCHUNK = 512
NQC = S // QCHUNK                    # 4 q-chunks
UNIT = 1024                          # scores psum unit (2 banks)

_BF16 = ml_dtypes.bfloat16
_cache = {}


def _build(use_rope: bool):
    import concourse.bass as bass
    import concourse.mybir as mybir
    import concourse.tile as tile
    from concourse import bacc

    F32 = mybir.dt.float32
    BF16 = mybir.dt.bfloat16
    EXP = mybir.ActivationFunctionType.Exp

    nc = bacc.Bacc(None, target_bir_lowering=False)

    xt_d = nc.dram_tensor("xt", [D, S], BF16, kind="ExternalInput")
    # host-preswizzled first s-column block: contiguous [p, (k s)] layout
    # so the critical first load moves in 2KB runs instead of 256B
    xt0_d = nc.dram_tensor("xt0", [128, KD * 128], BF16, kind="ExternalInput")
    wqk_d = nc.dram_tensor("wqk", [D, 2 * GDIM], BF16, kind="ExternalInput")
    wv_d = nc.dram_tensor("wv", [D, GDIM], BF16, kind="ExternalInput")
    wo_d = nc.dram_tensor("wo", [GDIM, D], BF16, kind="ExternalInput")
    cs_d = nc.dram_tensor("csr", [S, 64], BF16, kind="ExternalInput")
    mask_d = nc.dram_tensor("maskT", [128, 128], BF16, kind="ExternalInput")
    # bf16 output halves the output-DMA bytes; the host accumulates the
    # 4 per-batch partials in fp32
    out_d = nc.dram_tensor("out", [S, D], BF16, kind="ExternalOutput")

    with tile.TileContext(nc) as tc:
        with tc.tile_pool(name="big", bufs=1) as big, \
             tc.tile_pool(name="work", bufs=4) as work, \
             tc.tile_pool(name="ropet", bufs=5) as ropet, \
             tc.tile_pool(name="pex", bufs=6) as pex:
            # ---- resident tensors ----
            xt = big.tile([128, KD, S], BF16)
            xt0 = big.tile([128, KD, 128], BF16)
            wqk = big.tile([128, KD, 2 * GDIM], BF16)
            wv = big.tile([128, KD, GDIM], BF16)
            wo = big.tile([128, 2, D], BF16)
            maskT = big.tile([128, 128], BF16)
            csr = big.tile([128, SB, 64], BF16)
            qkt4 = big.tile([128, 4, S], BF16)
            # qkt4 blocks: 0: Qt heads 0-1, 1: Qt heads 2-3, 2: Kt 0-1, 3: Kt 2-3
            vsb = big.tile([128, SB, HEADS_PER_CORE * 65], BF16)
            yt2 = [big.tile([128, S], BF16, tag=f"yt2{i}", name=f"yt2{i}")
                   for i in range(2)]

            # ones-rows of vsb (column 64 of each 65-wide head slot)
            ones_view = vsb[:].rearrange("p m (h c) -> p m h c", h=4)[:, :, :, 64:65]
            nc.vector.memset(ones_view, 1.0)

            # ---- chunked input loads, ordered so x streams ahead of use ----
            xt_r = xt_d.rearrange("(k p) s -> p k s", p=128)
            wqk_r = wqk_d.rearrange("(k p) o -> p k o", p=128)
            nc.sync.dma_start(xt0[:], xt0_d.rearrange("p (k s) -> p k s", k=KD))
            nc.sync.dma_start(wqk[:, 0:2, :], wqk_r[:, 0:2, :])
            csr_r = cs_d.rearrange("(m p) f -> p m f", p=128)
            nc.sync.dma_start(csr[:, 0:8, :], csr_r[:, 0:8, :])
            nc.sync.dma_start(wqk[:, 2:4, :], wqk_r[:, 2:4, :])
            nc.sync.dma_start(wqk[:, 4:6, :], wqk_r[:, 4:6, :])
            nc.sync.dma_start(wqk[:, 6:8, :], wqk_r[:, 6:8, :])
            nc.sync.dma_start(xt[:, :, 128:384], xt_r[:, :, 128:384])
            nc.sync.dma_start(xt[:, :, 384:640], xt_r[:, :, 384:640])
            nc.sync.dma_start(wv[:], wv_d.rearrange("(k p) o -> p k o", p=128))
            nc.sync.dma_start(maskT[:], mask_d[:])
            nc.sync.dma_start(xt[:, :, 640:1536], xt_r[:, :, 640:1536])
            nc.sync.dma_start(xt[:, :, 1536:2048], xt_r[:, :, 1536:2048])
            nc.sync.dma_start(wo[:], wo_d.rearrange("(k p) o -> p k o", p=128))

            # shared QK-tile body: projection psum -> bf16 -> rope -> one
            # wide transpose into qkt4. copy_dve picks the cast engine so
            # phase-B filler tiles stay off the exp-loaded ACT queue.
            def emit_rope_transpose(ps, m, copy_dve):
                ms = slice(m * 128, (m + 1) * 128)
                qkr = ropet.tile([128, 2 * GDIM], BF16, tag="qkr")
                if use_rope:
                    qkf = ropet.tile([128, 2 * GDIM], BF16, tag="qkf")
                    if copy_dve:
                        nc.vector.tensor_copy(qkf[:], ps)
                    else:
                        nc.scalar.copy(qkf[:], ps)
                    qv = qkf[:].rearrange("p (g e c) -> p g e c", e=2, c=32)
                    ov = qkr[:].rearrange("p (g e c) -> p g e c", e=2, c=32)
                    E, O = qv[:, :, 0, :], qv[:, :, 1, :]
                    C = csr[:, m, 0:32].unsqueeze(1).broadcast_to((128, 8, 32))
                    Sn = csr[:, m, 32:64].unsqueeze(1).broadcast_to((128, 8, 32))
                    g32 = lambda t: t[:].rearrange("p (g c) -> p g c", c=32)
                    ta = ropet.tile([128, 256], BF16, tag="ta")
                    tb = ropet.tile([128, 256], BF16, tag="tb")
                    nc.vector.tensor_mul(g32(ta), E, C)
                    nc.vector.tensor_mul(g32(tb), O, Sn)
                    nc.vector.tensor_sub(ov[:, :, 0, :], g32(ta), g32(tb))
                    tc_ = ropet.tile([128, 256], BF16, tag="tc")
                    td = ropet.tile([128, 256], BF16, tag="td")
                    nc.vector.tensor_mul(g32(tc_), O, C)
                    nc.vector.tensor_mul(g32(td), E, Sn)
                    nc.vector.tensor_add(ov[:, :, 1, :], g32(tc_), g32(td))
                else:
                    if copy_dve:
                        nc.vector.tensor_copy(qkr[:], ps)
                    else:
                        nc.scalar.copy(qkr[:], ps)
                return nc.sync.dma_start_transpose(qkt4[:, :, ms], qkr[:])

            # ---- phase A: QK projections for the first 8 s-tiles (the
            # rest run inside phase B as PE filler) ----
            act_warm = False
            with tc.tile_pool(name="pa", bufs=2, space="PSUM") as pa:
                for m in range(SB // 2):
                    ms = slice(m * 128, (m + 1) * 128)
                    xsrc = (lambda k: xt0[:, k, :]) if m == 0 else \
                        (lambda k, ms=ms: xt[:, k, ms])
                    ps = pa.tile([128, 2 * GDIM], F32, tag="qk")
                    for k in range(KD):
                        nc.tensor.matmul(ps[:], xsrc(k), wqk[:, k, :],
                                         start=(k == 0), stop=(k == KD - 1))
                    emit_rope_transpose(ps[:], m, copy_dve=False)
                    if not act_warm:
                        # preload the Exp table while ACT is idle so the
                        # first phase-B exp doesn't pay the 1.3us load
                        wa = work.tile([1, 1], BF16, tag="wa")
                        nc.scalar.activation(wa[:], wqk[0:1, 0, 0:1],
                                             EXP, scale=0.125)
                        act_warm = True

            # ---- phase B: attention + V projection + interleaved o_proj ----
            with tc.tile_pool(name="sc", bufs=1, space="PSUM") as scp, \
                 tc.tile_pool(name="yb", bufs=1, space="PSUM") as ybp, \
                 tc.tile_pool(name="pv", bufs=2, space="PSUM") as pvp:

                def emit_vproj_tile(m):
                    ms = slice(m * 128, (m + 1) * 128)
                    psv = pvp.tile([128, QCHUNK], F32, tag="v")
                    for k in range(KD):
                        xs = xt0[:, k, :] if m == 0 else xt[:, k, ms]
                        nc.tensor.matmul(psv[:, 0:GDIM], xs, wv[:, k, :],
                                         start=(k == 0), stop=(k == KD - 1))
                    dst = vsb[:, m, :].rearrange("p (h c) -> p h c", h=4)[:, :, 0:64]
                    nc.scalar.copy(dst, psv[:, 0:GDIM].rearrange("p (h c) -> p h c", h=4))

                def emit_qkproj_tile(m):
                    ms = slice(m * 128, (m + 1) * 128)
                    ps = pvp.tile([128, QCHUNK], F32, tag="v")
                    for k in range(KD):
                        nc.tensor.matmul(ps[:], xt[:, k, ms], wqk[:, k, :],
                                         start=(k == 0), stop=(k == KD - 1))
                    emit_rope_transpose(ps[:], m, copy_dve=True)

                def emit_oproj_tile(m, use_pv=False):
                    ms = slice(m * 128, (m + 1) * 128)
                    so = work.tile([128, 1024], BF16, tag="so")
                    for nb in range(2):
                        if use_pv:
                            # pv-pool accumulator: avoids waiting on the
                            # ytps (big-tag) release behind the normalize
                            po = pvp.tile([128, QCHUNK], F32, tag="v")
                        else:
                            po = ybp.tile([128, QCHUNK], F32, tag=f"big{nb}")
                        for k2 in range(2):
                            nc.tensor.matmul(po[:], yt2[k2][:, ms],
                                             wo[:, k2, nb * 512:(nb + 1) * 512],
                                             start=(k2 == 0), stop=(k2 == 1))
                        # copies alternate DVE/ACT to halve the serial chain
                        if nb == 0:
                            nc.vector.tensor_copy(so[:, 0:512], po[:])
                        else:
                            nc.scalar.copy(so[:, 512:1024], po[:])
                    nc.sync.dma_start(out_d[ms, :], so[:])

                # qc0's diagonal PV needs vsb s-tiles 0-3 immediately;
                # project them first (also keeps PE warm across the
                # phase boundary)
                for mi in range(5):
                    emit_vproj_tile(mi)
                emit_qkproj_tile(8)
                emit_qkproj_tile(9)

                # filler schedule: one PE tile (o_proj of an earlier chunk,
                # V projection, or a deferred QK projection) per unit
                # boundary so ACT's exp stream and PE advance in lockstep.
                # Deadlines: vsb[m] before its chunk's diagonal units, and
                # qkt4[m] transposed before the chunk whose q-range needs it.
                FILLER = {
                    (0, 0, 0): [("qk", 8)], (0, 0, 1): [("v", 5)],
                    (0, 1, 0): [("v", 6)], (0, 1, 1): [("v", 7)],
                    (1, 0, 0): [("qk", 9)], (1, 0, 1): [("qk", 10)],
                    (1, 0, 2): [("v", 8)], (1, 0, 3): [("v", 9)],
                    (1, 1, 0): [("oppv", 2)], (1, 1, 1): [("qk", 11)],
                    (1, 1, 2): [("oppv", 3)], (1, 1, 3): [("oppv", 0),
                                                          ("oppv", 1)],
                    (2, 0, 0): [("v", 10)], (2, 0, 1): [("v", 11)],
                    (2, 0, 3): [("qk", 12)], (2, 0, 4): [("qk", 13)],
                    (2, 1, 0): [("oppv", 6)], (2, 1, 1): [("qk", 14)],
                    (2, 1, 2): [("qk", 15)], (2, 1, 3): [("oppv", 7)],
                    (2, 1, 4): [("oppv", 4)], (2, 1, 5): [("oppv", 5)],
                    (3, 0, 0): [("v", 12)], (3, 0, 2): [("v", 13)],
                    (3, 0, 4): [("v", 14)], (3, 0, 5): [("v", 15)],
                    (3, 1, 0): [("oppv", 10)], (3, 1, 1): [("oppv", 11)],
                    (3, 1, 2): [("oppv", 8)], (3, 1, 4): [("oppv", 9)],
                }

                for qc in range(NQC):
                    q0 = qc * QCHUNK
                    # block list: full key blocks then the diagonal in order
                    # 512,384,128,256 so no matmul crosses a 512-col bank
                    order = list(range(4 * qc)) + \
                        [4 * qc, 4 * qc + 1, 4 * qc + 3, 4 * qc + 2]
                    blocks = []
                    for kb in order:
                        r = max(0, kb - 4 * qc)
                        blocks.append((kb, q0 + r * 128, QCHUNK - r * 128))
                    units, cur, cols = [], [], 0
                    for kb, qoff, n in blocks:
                        if cols + n > UNIT:
                            units.append(cur)
                            cur, cols = [], 0
                        cur.append((kb, qoff, n, cols))
                        cols += n
                    units.append(cur)
                    last_kb = units[-1][-1][0]

                    for hp in range(2):
                        ytps = None
                        for ui, unit in enumerate(units):
                            ucols = unit[-1][3] + unit[-1][2]
                            scs = [scp.tile([128, UNIT], F32, tag=f"sc{i}",
                                            name=f"sc{i}") for i in range(2)]
                            for i in range(2):
                                rows = slice(i * 64, i * 64 + 64)
                                for (kb, qoff, n, o) in unit:
                                    nc.tensor.matmul(
                                        scs[i][:, o:o + n],
                                        qkt4[rows, 2 + hp, kb * 128:(kb + 1) * 128],
                                        qkt4[rows, hp, qoff:qoff + n],
                                        start=True, stop=True)
                            # PE filler between scores and PV: previous
                            # chunk's o_proj (must precede this chunk's ytps
                            # allocation — the po tiles reuse the big banks)
                            # and this chunk's V projection, spread across
                            # unit boundaries so ACT's exp stream never
                            # starves. The vsb tiles feed only the diagonal
                            # units, several pipeline steps later.
                            for kind, fm in FILLER.get((qc, hp, ui), ()):
                                if kind == "op":
                                    emit_oproj_tile(fm)
                                elif kind == "oppv":
                                    emit_oproj_tile(fm, use_pv=True)
                                elif kind == "v":
                                    emit_vproj_tile(fm)
                                else:
                                    emit_qkproj_tile(fm)
                            if ytps is None:
                                ytps = [ybp.tile([128, QCHUNK], F32,
                                                 tag=f"big{i}", name=f"big{i}")
                                        for i in range(2)]
                            for i in range(2):
                                h = 2 * hp + i
                                vcol = slice(h * 65, h * 65 + 65)
                                pe = pex.tile([128, UNIT], BF16,
                                              tag=f"pe{i}", name=f"pe{i}")
                                nc.scalar.activation(pe[:, :ucols],
                                                     scs[i][:, :ucols],
                                                     EXP, scale=0.125)
                                for (kb, qoff, n, o) in unit:
                                    if kb >= 4 * qc:  # diagonal: causal mask
                                        nc.vector.tensor_mul(
                                            pe[:, o:o + 128], pe[:, o:o + 128],
                                            maskT[:])
                                    # kb==0 always has n=512: start clears
                                    # the whole [65, QCHUNK] accumulator
                                    nc.tensor.matmul(
                                        ytps[i][0:65, qoff - q0:qoff - q0 + n],
                                        vsb[:, kb, vcol],
                                        pe[:, o:o + n],
                                        start=(kb == 0), stop=(kb == last_kb))
                        if qc == NQC - 1 and hp == 1:
                            # tail: bf16 normalize (denominator precision
                            # cost ~0.4%, well inside the error budget).
                            # ACT casts y to bf16 while DVE does the
                            # reciprocals; the per-m-tile muls then run in
                            # 2x mode and stagger the o_proj k2=1 pairs.
                            # Reading ytps via ybf also releases the big
                            # banks for the tail slots sooner.
                            rcs, ybfs = [], []
                            for i in range(2):
                                rc = work.tile([1, QCHUNK], BF16, tag=f"rcb{i}")
                                with nc.allow_low_precision(
                                        reason="bf16 softmax denominator"):
                                    nc.vector.reciprocal(rc[:],
                                                         ytps[i][64:65, :])
                                rcs.append(rc)
                                ybf = work.tile([64, QCHUNK], BF16,
                                                tag=f"ybf{i}")
                                nc.scalar.copy(ybf[:], ytps[i][0:64, :])
                                ybfs.append(ybf)
                            for mi in range(4):
                                mc = slice(mi * 128, (mi + 1) * 128)
                                for i in range(2):
                                    bc = work.tile([64, 128], BF16, tag="bct")
                                    nc.gpsimd.partition_broadcast(
                                        bc[:], rcs[i][0:1, mc])
                                    nc.vector.tensor_mul(
                                        yt2[hp][i * 64:i * 64 + 64,
                                                q0 + mi * 128:
                                                q0 + (mi + 1) * 128],
                                        ybfs[i][:, mc], bc[:])
                        else:
                            for i in range(2):
                                # normalize straight from PSUM
                                rc = work.tile([1, QCHUNK], F32, tag="rc")
                                nc.vector.reciprocal(rc[:], ytps[i][64:65, :])
                                bc = work.tile([64, QCHUNK], F32, tag="bc")
                                nc.gpsimd.partition_broadcast(bc[:], rc[0:1, :])
                                nc.vector.tensor_mul(
                                    yt2[hp][i * 64:i * 64 + 64, q0:q0 + QCHUNK],
                                    ytps[i][0:64, :], bc[:])
                    if qc == NQC - 1:
                        # tail o_proj with 8 independent PSUM accumulators
                        # (freed scores banks + pv pool + big banks) so no
                        # matmul waits on a copy; all k2=0 matmuls run
                        # during the normalize chains, k2=1 staggers in as
                        # the per-m muls land; copies alternate DVE/ACT
                        slots = []
                        for mi in range(2):
                            pot = scp.tile([128, UNIT], F32,
                                           tag=f"sc{mi}", name=f"sc{mi}")
                            slots.append((pot[:, 0:512], pot[:, 512:1024]))
                        for mi in range(2):
                            a = pvp.tile([128, QCHUNK], F32, tag="v")
                            b = ybp.tile([128, QCHUNK], F32,
                                         tag=f"big{mi}", name=f"big{mi}")
                            slots.append((a[:], b[:]))
                        for k2 in range(2):
                            for mi in range(4):
                                m = qc * 4 + mi
                                ms = slice(m * 128, (m + 1) * 128)
                                for nb in range(2):
                                    nc.tensor.matmul(
                                        slots[mi][nb], yt2[k2][:, ms],
                                        wo[:, k2, nb * 512:(nb + 1) * 512],
                                        start=(k2 == 0), stop=(k2 == 1))
                        for mi in range(4):
                            m = qc * 4 + mi
                            ms = slice(m * 128, (m + 1) * 128)
                            so = work.tile([128, 1024], BF16, tag=f"sot{mi}")
                            for nb in range(2):
                                if (mi + nb) % 2 == 0:
                                    nc.vector.tensor_copy(
                                        so[:, nb * 512:(nb + 1) * 512],
                                        slots[mi][nb])
                                else:
                                    nc.scalar.copy(
                                        so[:, nb * 512:(nb + 1) * 512],
                                        slots[mi][nb])
                            nc.sync.dma_start(out_d[ms, :], so[:])
    nc.compile()
    return nc


def _prep_core_inputs(x, Wq, Wk, Wv, Wo, cos_g, sin_g, use_rope):
    """Host-side shard + layout prep. Returns list of 8 input dicts."""
    maskT = np.tril(np.ones((128, 128), np.float32)).T.astype(_BF16)
    # per-head row permutation: [evens | odds] so rope pairs are
    # (j, j+32) within each head's 64 projection dims
    perm = np.concatenate([np.arange(h * 64, (h + 1) * 64).reshape(32, 2).T.reshape(64)
                           for h in range(H)])
    Wq_p = Wq[perm]
    Wk_p = Wk[perm]
    maps = []
    for c in range(NCORES):
        b, g = divmod(c, HEADS_PER_CORE)
        rows = slice(g * GDIM, (g + 1) * GDIM)
        wqk = np.concatenate([Wq_p[rows], Wk_p[rows]], axis=0).T  # [D, 512]
        xt_h = np.ascontiguousarray(x[b].T).astype(_BF16)
        xt0_h = np.ascontiguousarray(
            xt_h[:, 0:128].reshape(KD, 128, 128).transpose(1, 0, 2)
            .reshape(128, KD * 128))
        maps.append({
            "xt": xt_h,
            "xt0": xt0_h,
            "wqk": np.ascontiguousarray(wqk).astype(_BF16),
            "wv": np.ascontiguousarray(Wv[rows].T).astype(_BF16),
            "wo": np.ascontiguousarray(Wo[:, rows].T).astype(_BF16),
            "csr": np.concatenate([cos_g, sin_g], axis=1).astype(_BF16),
            "maskT": maskT,
        })
    return maps


def kernel(x, token_positions, use_rope, Wq, Wk, Wv, Wo, cos, sin):
    from concourse.bass_utils import run_bass_kernel_spmd

    x = np.asarray(x, np.float32)
    token_positions = np.asarray(token_positions)
    Wq = np.asarray(Wq, np.float32)
    Wk = np.asarray(Wk, np.float32)
    Wv = np.asarray(Wv, np.float32)
    Wo = np.asarray(Wo, np.float32)
    cos = np.asarray(cos, np.float32)
    sin = np.asarray(sin, np.float32)
    rope = bool(int(use_rope))

    cos_g = cos[token_positions]  # [S, 32]
    sin_g = sin[token_positions]

    if rope not in _cache:
        _cache[rope] = _build(rope)
    nc = _cache[rope]

    in_maps = _prep_core_inputs(x, Wq, Wk, Wv, Wo, cos_g, sin_g, rope)
    res = run_bass_kernel_spmd(nc, in_maps, list(range(NCORES)))

    out = np.zeros((B, S, D), np.float32)
    for c in range(NCORES):
        out[c // HEADS_PER_CORE] += res.results[c]["out"].astype(np.float32)
    return out
